# revision 1
# baseline (speedup 1.0000x reference)
"""Banded DTW loss kernel for Trainium2 (Bass/Tile), 8-core data-parallel.

Algorithm (per sample, N=1024, Sakoe-Chiba half-width W=20, band width 41):
  Phase A: forward DP row-by-row. Row recurrence
             D[i,j] = d[i,j] + min(D[i-1,j-1], D[i-1,j], D[i,j-1])
           is computed with ONE tensor_tensor_scan per row
           (state = min(d + state, min(diag,up)+d)), bit-identical values
           to the reference wavefront DP. D rows stream through a rolling
           16-row window and are DMA'd into a [128,*]-partition RE layout.
  Phase B: per-cell backtrack choice bits (argmin with diag>up>left
           preference, replicating the reference bt_step exactly), then a
           per-row scan producing g[row, col] = entry column of row-1 when
           the backtrack enters `row` at `col`.
  Phase C: walk rows 1023..1 with ONE scalar_tensor_tensor per row
           (one-hot extract of g at the current column; accum_out = next
           column).
  Phase D: the path covers a contiguous column interval per row; build
           interval masks and reduce the four path aggregates
           (sum|dx|, sum|dy|, sum bce, count) with big parallel ops.

All compute ops keep every SBUF operand at the same start partition
(0 or 64) to satisfy the birverifier's samePartitionsAll check.

Sharding: batch 32 -> 4 samples per core on 8 cores; host sums partials.
"""

import numpy as np

import concourse.bacc as bacc
import concourse.bass as bass
import concourse.mybir as mybir
import concourse.tile as tile
from concourse.bass_utils import run_bass_kernel_spmd

B, N, NF = 32, 1024, 4
W = 20
NCORES = 8
BC = B // NCORES          # samples per core
BIG = 1e30
NB = 41                   # band width (o = j - i + 20 in [0,40])
CW = 43                   # RE row width (col 0 pad, col c=o+1, col 42 pad)
NBLK = 33                 # RE blocks (r = i+1 in [0,1024], p=r%32, b=r//32)
PPAD_T = 1056             # ppad time length  (ppad[:,1+i,:] = preds[:,i,:])
TPAD_T = 1100             # tpad time length  (tpad[:,21+i,:] = targs[:,i,:])
SKW = 1066                # skewed targ row length
NRING = 16

AL = mybir.AluOpType
DT = mybir.dt.float32

NWIN = 64                 # rolling window depth (rows), ring ditto

# ---- megaQ ([128, QW]) column offsets; quadrant shadows @0 and @64 ----
WIN_O = 0                 # @0: rolling D window, 64 slots * 42 (col 41 BIG)
VR_O = WIN_O + NWIN * 42             # @0: virtual row r=0 (42 cols)
RING_O = VR_O + 42                   # @0: d ring, 64 slots * 41
TMP_O = RING_O + NWIN * NB           # @0: phase-A tmp
DA1_O = TMP_O + 48                   # @0: phase-A data1
WSCL_O = DA1_O + 48                  # @0: walk scratch (lo half)
XHL_O = WSCL_O + 48                  # @0: xhist cols i in [0,512)
GWLO_O = XHL_O + 512                 # @0: g rows i in [0,512), 41 each
QW_LO = GWLO_O + 512 * NB
GWHI_O = 0                # @64: g rows i in [512,1024)
XHH_O = GWHI_O + 512 * NB            # @64: xhist col (i-511), i in [511,1024)
WSCH_O = XHH_O + 513                 # @64: walk scratch (hi half)
QW = max(QW_LO, WSCH_O + 48)

# ---- megaRE ([128, 8*1419 + 448]) regions ----
RE = NBLK * CW            # 1419
R1_O, R2_O, R3_O, R4_O, R5_O, R6_O, R7_O, R8_O = (i * RE for i in range(8))
SM_O = 8 * RE             # small-tensor block (448 cols)
PX_O, PY_O, PZ_O = SM_O, SM_O + 33, SM_O + 66
XC_O, OLO_O = SM_O + 99, SM_O + 132
PCONST_O, COLIO_O = SM_O + 165, SM_O + 166   # colio values 0..42
CLZ_O, SPZ_O, SPN_O, QZ_O, NGZ_O = (SM_O + c for c in (209, 242, 275, 308, 341))
RED_O = SM_O + 374        # Sx, Sy, Sbce, cnt
ROWC_O = SM_O + 378       # per-(p,b) row constant 32b + p - 22
REW = SM_O + 448

_CACHE = {}


def _manual_ap(base, extra_off, dims):
    """AP with base's partition dim and explicit free [stride, count] dims."""
    ap0 = [list(base.ap[0])]
    return bass.AP(base.tensor, base.offset + extra_off,
                   ap0 + [list(d) for d in dims])


def _build_module():
    nc = bacc.Bacc("TRN2", target_bir_lowering=False, debug=False,
                   num_devices=NCORES)
    pre = nc.dram_tensor("pre", [128, 99], DT, kind="ExternalInput")
    tsk = nc.dram_tensor("tsk", [128, 3 * SKW], DT, kind="ExternalInput")
    cst = nc.dram_tensor("cst", [128, 78], DT, kind="ExternalInput")
    partials = nc.dram_tensor("partials", [128, 4], DT, kind="ExternalOutput")
    dram_d = nc.dram_tensor("dscr_d", [BC, N, NB], DT, kind="Internal")
    dram_D = nc.dram_tensor("dscr_D", [BC, N, NB], DT, kind="Internal")

    with tile.TileContext(nc) as tc:
        with tc.tile_pool(name="main", bufs=1) as pool:
            megaQ = pool.tile([128, QW], DT)
            megaRE = pool.tile([128, REW], DT)
            _emit(nc, megaQ, megaRE, pre, tsk, cst, partials, dram_d, dram_D)
    nc.compile()
    return nc


def _emit(nc, megaQ, megaRE, pre, tsk, cst, partials, dram_d, dram_D):
    v = nc.vector

    def cells(off, dc=0):
        """[128, 33, 41] view of RE region cols (b*43 + 1 + dc)."""
        return megaRE[:, off:off + RE].rearrange(
            "p (b c) -> p b c", c=CW)[:, :, 1 + dc:NB + 1 + dc]

    def reblk(off, b, dc=0, w=NB):
        s = off + b * CW + 1 + dc
        return megaRE[:, s:s + w]

    def smb(off):
        """[128, 33] small block broadcast over the 41 band cols."""
        return megaRE[:, off:off + NBLK].unsqueeze(2).broadcast_to([128, NBLK, NB])

    def skwin(off):
        """skewed targ window [128, 33, 41]: u = 32b + (c-1)."""
        base = megaRE[:, off:off + 1]
        return _manual_ap(base, 0, [[32, NBLK], [1, NB]])

    # ---------------- input DMAs (host pre-laid-out) ----------------
    nc.sync.dma_start(out=megaRE[:, PCONST_O:PCONST_O + 44], in_=cst[:, 0:44])
    nc.sync.dma_start(out=megaRE[:, ROWC_O:ROWC_O + NBLK], in_=cst[:, 44:44 + NBLK])
    nc.sync.dma_start(out=megaRE[:, PX_O:PX_O + 99], in_=pre[:])
    for k, off in ((0, R1_O), (1, R2_O), (2, R3_O)):
        for c0, c1 in ((0, 267), (267, 534), (534, 800), (800, SKW)):
            nc.sync.dma_start(out=megaRE[:, off + c0:off + c1],
                              in_=tsk[:, k * SKW + c0:k * SKW + c1])

    # ---------------- d build (all cells, RE layout) ----------------
    ocolv = megaRE[:, COLIO_O + 1:COLIO_O + 1 + NB].unsqueeze(1) \
        .broadcast_to([128, NBLK, NB])
    # jmap = (32b + p - 22) + oc  (the j index of each band cell)
    v.tensor_tensor(out=cells(R5_O), in0=smb(ROWC_O), in1=ocolv, op=AL.add)
    v.tensor_single_scalar(out=cells(R6_O), in_=cells(R5_O),
                           scalar=0.0, op=AL.is_ge)
    v.tensor_single_scalar(out=cells(R7_O), in_=cells(R5_O),
                           scalar=float(N - 1), op=AL.is_le)
    v.tensor_tensor(out=cells(R6_O), in0=cells(R6_O), in1=cells(R7_O), op=AL.mult)
    # vmb = BIG at invalid cells, 0 at valid ones
    v.tensor_scalar(out=cells(R7_O), in0=cells(R6_O),
                    scalar1=-BIG, scalar2=BIG, op0=AL.mult, op1=AL.add)
    # |dx|, |dy| for every cell (also the Sx/Sy metric inputs)
    v.tensor_tensor(out=cells(R5_O), in0=smb(PX_O), in1=skwin(R1_O),
                    op=AL.subtract)
    v.scalar_tensor_tensor(out=cells(R1_O), in0=cells(R5_O), scalar=-1.0,
                           in1=cells(R5_O), op0=AL.mult, op1=AL.max)
    v.tensor_tensor(out=cells(R5_O), in0=smb(PY_O), in1=skwin(R2_O),
                    op=AL.subtract)
    v.scalar_tensor_tensor(out=cells(R2_O), in0=cells(R5_O), scalar=-1.0,
                           in1=cells(R5_O), op0=AL.mult, op1=AL.max)
    v.tensor_tensor(out=cells(R5_O), in0=cells(R1_O), in1=cells(R2_O), op=AL.add)
    # dcost = max(d, vmb): exactly d at valid cells, exactly BIG at invalid
    v.tensor_tensor(out=cells(R6_O), in0=cells(R5_O), in1=cells(R7_O), op=AL.max)

    # ---------------- stage dcost to DRAM (per-p, before phase A) ----------------
    df = dram_d[:]
    Df = dram_D[:]
    for p in range(32):
        bs = [b for b in range(NBLK) if 1 <= 32 * b + p <= N]
        b0, nb = bs[0], len(bs)
        i0 = 32 * b0 + p - 1
        src = _manual_ap(megaRE[4 * p:4 * p + 4,
                                R6_O + b0 * CW + 1:R6_O + b0 * CW + 1 + NB],
                         0, [[CW, nb], [1, NB]])
        dst = bass.AP(df.tensor, i0 * NB, [[N * NB, BC], [32 * NB, nb], [1, NB]])
        nc.sync.dma_start(out=dst, in_=src)

    # ---------------- phase A: forward DP (2 DVE ops per row) ----------------
    # D[oc] = min(mn[oc], D[oc-1]) + d[oc] via tensor_tensor_scan with
    # op0=min, op1=add (state carries D[oc-1]); mn = min(diag, up).
    v.memset(megaQ[0:4, WIN_O:WIN_O + NWIN * 42], BIG)
    v.memset(megaQ[0:4, VR_O:VR_O + 42], BIG)
    v.memset(megaQ[0:4, VR_O + 20:VR_O + 21], 0.0)     # virtual row: D[-1]=0 @ o=20
    v.memset(megaRE[:, R4_O:R4_O + RE], BIG)           # Dre (pads + vrow)
    v.memset(megaRE[0:4, R4_O + 21:R4_O + 22], 0.0)    # vrow in RE (r=0, col 21)

    tmp = megaQ[0:4, TMP_O:TMP_O + NB]
    qbase = megaQ[0:4, 0:1]
    for r in range(1, N + 1):
        i = r - 1
        if i % 32 == 0:
            nrow = min(32, N - i)
            rdst = _manual_ap(qbase, RING_O + (i % NWIN) * NB,
                              [[NB, nrow], [1, NB]])
            rsrc = bass.AP(df.tensor, i * NB, [[N * NB, BC], [NB, nrow], [1, NB]])
            nc.gpsimd.dma_start(out=rdst, in_=rsrc)
        ws = WIN_O + (i % NWIN) * 42
        wp = VR_O if r == 1 else WIN_O + ((i - 1) % NWIN) * 42
        rg0 = RING_O + (i % NWIN) * NB
        dring = megaQ[0:4, rg0:rg0 + NB]
        v.tensor_tensor(out=tmp, in0=megaQ[0:4, wp:wp + NB],
                        in1=megaQ[0:4, wp + 1:wp + NB + 1], op=AL.min)
        v.tensor_tensor_scan(out=megaQ[0:4, ws:ws + NB], data0=tmp,
                             data1=dring, initial=BIG, op0=AL.min, op1=AL.add)
        if i % 32 == 31 or r == N:
            i0 = (i // 32) * 32
            nrow = i - i0 + 1
            k = i // 32
            wsrc = _manual_ap(qbase, WIN_O + (i0 % NWIN) * 42,
                              [[42, nrow], [1, NB]])
            wdst = bass.AP(Df.tensor, i0 * NB, [[N * NB, BC], [NB, nrow], [1, NB]])
            nc.sync.dma_start(out=wdst, in_=wsrc)
            # reload this chunk into Dre right away (overlaps later chunks)
            npp = min(31, N - 1 - 32 * k)          # rows r=32k+1 .. 32k+31
            if npp > 0:
                src = bass.AP(Df.tensor, (32 * k) * NB,
                              [[NB, npp], [N * NB, BC], [1, NB]])
                dst = megaRE[4:4 + 4 * npp, R4_O + k * CW + 1:R4_O + k * CW + 1 + NB]
                nc.sync.dma_start(out=dst, in_=src)
            if 32 * (k + 1) <= N:                  # row r=32(k+1) -> p=0, b=k+1
                src = bass.AP(Df.tensor, (32 * k + 31) * NB,
                              [[N * NB, BC], [1, NB]])
                dst = megaRE[0:4, R4_O + (k + 1) * CW + 1:
                             R4_O + (k + 1) * CW + 1 + NB]
                nc.sync.dma_start(out=dst, in_=src)

    # ---------------- phase B: choice bits + g/L scans ----------------
    v.memset(megaRE[:, R5_O:R5_O + RE], BIG)           # DrePrev
    nc.sync.dma_start(out=megaRE[4:128, R5_O:R5_O + RE],
                      in_=megaRE[0:124, R4_O:R4_O + RE])
    nc.sync.dma_start(out=megaRE[0:4, R5_O + CW:R5_O + RE],
                      in_=megaRE[124:128, R4_O:R4_O + RE - CW])

    diag, up = cells(R5_O, 0), cells(R5_O, 1)
    left = cells(R4_O, -1)
    v.tensor_tensor(out=cells(R7_O), in0=diag, in1=up, op=AL.is_le)
    v.tensor_tensor(out=cells(R8_O), in0=diag, in1=left, op=AL.is_le)
    v.tensor_tensor(out=cells(R8_O), in0=cells(R7_O), in1=cells(R8_O),
                    op=AL.mult)                        # isdiag
    v.tensor_tensor(out=cells(R7_O), in0=left, in1=diag, op=AL.is_lt)
    v.tensor_tensor(out=cells(R6_O), in0=left, in1=up, op=AL.is_lt)
    v.tensor_tensor(out=cells(R7_O), in0=cells(R7_O), in1=cells(R6_O),
                    op=AL.mult)                        # isleft
    v.tensor_single_scalar(out=cells(R6_O), in_=cells(R7_O),
                           scalar=0.0, op=AL.is_equal)  # notleft
    ocp1 = megaRE[:, COLIO_O + 2:COLIO_O + 2 + NB].unsqueeze(1) \
        .broadcast_to([128, NBLK, NB])
    ocol = megaRE[:, COLIO_O + 1:COLIO_O + 1 + NB].unsqueeze(1) \
        .broadcast_to([128, NBLK, NB])
    v.tensor_tensor(out=cells(R8_O), in0=ocp1, in1=cells(R8_O), op=AL.subtract)
    v.tensor_tensor(out=cells(R8_O), in0=cells(R8_O), in1=cells(R6_O),
                    op=AL.mult)                        # gval
    v.tensor_tensor(out=cells(R6_O), in0=ocol, in1=cells(R6_O), op=AL.mult)  # Lval
    for b in range(NBLK):
        v.tensor_tensor_scan(out=reblk(R5_O, b), data0=reblk(R7_O, b),
                             data1=reblk(R8_O, b), initial=0.0,
                             op0=AL.mult, op1=AL.add)  # gfull -> R5
    for b in range(NBLK):
        v.tensor_tensor_scan(out=reblk(R8_O, b), data0=reblk(R7_O, b),
                             data1=reblk(R6_O, b), initial=0.0,
                             op0=AL.mult, op1=AL.add)  # Lfull -> R8

    # ---------------- gwalk copies + walk ----------------
    for half in (1, 0):
        for p in [0] + list(range(31, 0, -1)):     # walk-consumption order
            bs = [b for b in range(NBLK)
                  if 1 <= 32 * b + p <= N
                  and half * 512 <= 32 * b + p - 1 < half * 512 + 512]
            if not bs:
                continue
            b0, nb = bs[0], len(bs)
            i0 = 32 * b0 + p - 1
            src = _manual_ap(
                megaRE[4 * p:4 * p + 4, R5_O + b0 * CW + 1:R5_O + b0 * CW + 1 + NB],
                0, [[CW, nb], [1, NB]])
            q0, go = (0, GWLO_O) if half == 0 else (64, GWHI_O)
            dst = _manual_ap(
                megaQ[q0:q0 + 4, go + (i0 % 512) * NB:go + (i0 % 512) * NB + NB],
                0, [[32 * NB, nb], [1, NB]])
            nc.sync.dma_start(out=dst, in_=src)

    xhl = megaQ[0:4, XHL_O:XHL_O + 512]
    xhh = megaQ[64:68, XHH_O:XHH_O + 513]
    wscl = megaQ[0:4, WSCL_O:WSCL_O + NB]
    wsch = megaQ[64:68, WSCH_O:WSCH_O + NB]
    v.memset(xhh[:, 512:513], 21.0)                    # x_1023 (col coords)
    for i in range(1023, 511, -1):                     # rows 1023..512 (@64)
        g = megaQ[64:68, GWHI_O + (i - 512) * NB:GWHI_O + (i - 512) * NB + NB]
        iot = megaRE[64:68, COLIO_O + 1:COLIO_O + 1 + NB]
        v.scalar_tensor_tensor(out=wsch, in0=iot,
                               scalar=xhh[:, i - 511:i - 510], in1=g,
                               op0=AL.is_equal, op1=AL.mult,
                               accum_out=xhh[:, i - 512:i - 511])
    nc.gpsimd.dma_start(out=xhl[:, 511:512], in_=xhh[:, 0:1])   # x_511
    for i in range(511, 0, -1):                        # rows 511..1 (@0)
        g = megaQ[0:4, GWLO_O + i * NB:GWLO_O + i * NB + NB]
        iot = megaRE[0:4, COLIO_O + 1:COLIO_O + 1 + NB]
        v.scalar_tensor_tensor(out=wscl, in0=iot,
                               scalar=xhl[:, i:i + 1], in1=g,
                               op0=AL.is_equal, op1=AL.mult,
                               accum_out=xhl[:, i - 1:i])

    # ---------------- xcol + olo + mask ----------------
    v.memset(megaRE[:, XC_O:XC_O + NBLK], 0.0)
    for p in range(32):
        for half in (0, 1):
            bs = [b for b in range(NBLK)
                  if 1 <= 32 * b + p <= N
                  and half * 512 <= 32 * b + p - 1 < half * 512 + 512]
            if not bs:
                continue
            b0, nb = bs[0], len(bs)
            i0 = 32 * b0 + p - 1
            if half == 0:
                src = _manual_ap(megaQ[0:4, XHL_O + i0:XHL_O + i0 + 1],
                                 0, [[32, nb]])
            else:
                src = _manual_ap(megaQ[64:68, XHH_O + i0 - 511:XHH_O + i0 - 510],
                                 0, [[32, nb]])
            dst = _manual_ap(megaRE[4 * p:4 * p + 4, XC_O + b0:XC_O + b0 + 1],
                             0, [[1, nb]])
            nc.sync.dma_start(out=dst, in_=src)

    xcolb = smb(XC_O)
    v.tensor_tensor(out=cells(R7_O), in0=ocol, in1=xcolb, op=AL.is_equal)
    v.tensor_tensor(out=cells(R7_O), in0=cells(R7_O), in1=cells(R8_O), op=AL.mult)
    v.tensor_reduce(out=megaRE[:, OLO_O:OLO_O + NBLK], in_=cells(R7_O),
                    axis=mybir.AxisListType.X, op=AL.add)
    v.tensor_tensor(out=cells(R6_O), in0=ocol, in1=smb(OLO_O), op=AL.is_ge)
    v.tensor_tensor(out=cells(R7_O), in0=ocol, in1=xcolb, op=AL.is_le)
    v.tensor_tensor(out=cells(R5_O), in0=cells(R6_O), in1=cells(R7_O),
                    op=AL.mult)                        # mask
    v.memset(megaRE[0:4, R5_O:R5_O + CW], 0.0)         # r=0 virtual slot
    v.memset(megaRE[:, R5_O + 32 * CW:R5_O + 33 * CW], 0.0)  # b=32 junk slots
    # row 1023 (r=1024, p=0, b=32) is real: rebuild its mask (all @0)
    lo1023 = megaRE[0:4, OLO_O + 32:OLO_O + 33]
    hi1023 = megaRE[0:4, XC_O + 32:XC_O + 33]
    ic0 = megaRE[0:4, COLIO_O + 1:COLIO_O + 1 + NB]
    m0 = megaRE[0:4, R5_O + 32 * CW + 1:R5_O + 32 * CW + 1 + NB]
    v.scalar_tensor_tensor(out=wscl, in0=ic0, scalar=lo1023, in1=ic0,
                           op0=AL.is_ge, op1=AL.bypass)
    v.scalar_tensor_tensor(out=m0, in0=ic0, scalar=hi1023, in1=wscl,
                           op0=AL.is_le, op1=AL.mult)

    # ---------------- metrics ----------------
    for src_o, red in ((R1_O, 0), (R2_O, 1)):
        v.tensor_tensor(out=cells(R7_O), in0=cells(src_o), in1=cells(R5_O),
                        op=AL.mult)
        v.tensor_reduce(out=megaRE[:, RED_O + red:RED_O + red + 1],
                        in_=cells(R7_O), axis=mybir.AxisListType.XY, op=AL.add)
    # bce cells: sp(x) + y*(5*sp(-x) - sp(x)),  x = clip(pz, -4, 4)
    v.tensor_scalar(out=megaRE[:, CLZ_O:CLZ_O + NBLK],
                    in0=megaRE[:, PZ_O:PZ_O + NBLK],
                    scalar1=-4.0, scalar2=4.0, op0=AL.max, op1=AL.min)
    nc.scalar.activation(megaRE[:, NGZ_O:NGZ_O + NBLK],
                         megaRE[:, CLZ_O:CLZ_O + NBLK],
                         mybir.ActivationFunctionType.Exp)
    nc.scalar.activation(megaRE[:, SPZ_O:SPZ_O + NBLK],
                         megaRE[:, NGZ_O:NGZ_O + NBLK],
                         mybir.ActivationFunctionType.Ln, bias=1.0)
    nc.scalar.activation(megaRE[:, NGZ_O:NGZ_O + NBLK],
                         megaRE[:, CLZ_O:CLZ_O + NBLK],
                         mybir.ActivationFunctionType.Exp, scale=-1.0)
    nc.scalar.activation(megaRE[:, SPN_O:SPN_O + NBLK],
                         megaRE[:, NGZ_O:NGZ_O + NBLK],
                         mybir.ActivationFunctionType.Ln, bias=1.0)
    v.scalar_tensor_tensor(out=megaRE[:, QZ_O:QZ_O + NBLK],
                           in0=megaRE[:, SPN_O:SPN_O + NBLK], scalar=5.0,
                           in1=megaRE[:, SPZ_O:SPZ_O + NBLK],
                           op0=AL.mult, op1=AL.subtract)
    v.tensor_tensor(out=cells(R7_O), in0=skwin(R3_O), in1=smb(QZ_O), op=AL.mult)
    v.tensor_tensor(out=cells(R7_O), in0=cells(R7_O), in1=smb(SPZ_O), op=AL.add)
    v.tensor_tensor(out=cells(R7_O), in0=cells(R7_O), in1=cells(R5_O), op=AL.mult)
    v.tensor_reduce(out=megaRE[:, RED_O + 2:RED_O + 3], in_=cells(R7_O),
                    axis=mybir.AxisListType.XY, op=AL.add)
    v.tensor_reduce(out=megaRE[:, RED_O + 3:RED_O + 4], in_=cells(R5_O),
                    axis=mybir.AxisListType.XY, op=AL.add)

    nc.sync.dma_start(out=partials[:], in_=megaRE[:, RED_O:RED_O + 4])


def _get_module():
    if "nc" not in _CACHE:
        _CACHE["nc"] = _build_module()
    return _CACHE["nc"]


def _make_inmaps(preds, targs):
    preds = np.ascontiguousarray(preds, dtype=np.float32)
    targs = np.ascontiguousarray(targs, dtype=np.float32)
    cst = np.zeros((128, 78), dtype=np.float32)
    cst[:, 0] = np.arange(128) // 4
    cst[:, 1:44] = np.arange(43)[None, :]
    cst[:, 44:77] = (32 * np.arange(NBLK)[None, :]
                     + (np.arange(128) // 4)[:, None] - 22)
    pp = np.arange(32)
    bb = np.arange(NBLK)
    r_idx = pp[:, None] + 32 * bb[None, :]              # [32, 33]
    r_ok = (r_idx >= 1) & (r_idx <= N)
    r_cl = np.clip(r_idx - 1, 0, N - 1)
    uu = np.arange(SKW)
    t_idx = uu[None, :] + pp[:, None] - 21              # [32, SKW]
    t_ok = (t_idx >= 0) & (t_idx < N)
    t_cl = np.clip(t_idx, 0, N - 1)
    in_maps = []
    for c in range(NCORES):
        ps = preds[c * BC:(c + 1) * BC]
        ts = targs[c * BC:(c + 1) * BC]
        pre = np.zeros((32, BC, 3 * NBLK), dtype=np.float32)
        tskv = np.zeros((32, BC, 3 * SKW), dtype=np.float32)
        for k in range(3):
            vv = ps[:, :, k][:, r_cl]                   # [BC, 32, NBLK]
            pre[:, :, k * NBLK:(k + 1) * NBLK] = \
                np.where(r_ok[None], vv, 0.0).transpose(1, 0, 2)
            ww = ts[:, :, k][:, t_cl]                   # [BC, 32, SKW]
            tskv[:, :, k * SKW:(k + 1) * SKW] = \
                np.where(t_ok[None], ww, 0.0).transpose(1, 0, 2)
        in_maps.append({"pre": pre.reshape(128, 3 * NBLK),
                        "tsk": tskv.reshape(128, 3 * SKW), "cst": cst})
    return in_maps


def _reduce_host(parts_list, subcoef):
    c0, c1 = float(subcoef[0]), float(subcoef[1])
    loss = 0.0
    for parts in parts_list:
        m = parts.reshape(32, BC, 4).sum(axis=0)        # [BC, (Sx,Sy,Sb,cnt)]
        for s in range(BC):
            sx, sy, sb, cnt = (float(m[s, k]) for k in range(4))
            loss += c0 * sx + c1 * sy + 0.1 * sb / cnt
    return np.float32(loss)


def run(preds, targs, subcoef, trace=False):
    nc = _get_module()
    in_maps = _make_inmaps(preds, targs)
    res = run_bass_kernel_spmd(nc, in_maps, core_ids=list(range(NCORES)),
                               trace=trace)
    parts = [r["partials"] for r in res.results]
    return _reduce_host(parts, np.asarray(subcoef)), res


def kernel(preds, targs, subcoef):
    out, _ = run(preds, targs, subcoef)
    return out



# revision 10
# speedup vs baseline: 1.3837x; 1.3837x over previous
"""Banded DTW loss kernel for Trainium2 (Bass/Tile), 8-core data-parallel.

Layout: sample-major partitions q = 32*s + p (s = sample 0..3, p = row%32).

Phase A (forward DP) uses ONE fused 82-wide tensor_tensor_scan per row:
the row slot holds d interleaved with zeros (d[o] at odd col 2o+1), and
data0 is a 3-D overlapping-pair window over the previous row's slot
(odd cols hold D).  Scan semantics state = min(data0[t], state) + data1[t]
then give, per band cell o:
    t=2o   : m   = min(D'[o],   D[o-1]) + 0
    t=2o+1 : D[o]= min(D'[o+1], m     ) + d[o]
which is exactly the banded DTW recurrence.  The scan writes the slot in
place (d -> interleaved m/D).  Rows are distributed over 4 partition
quadrants (0/32/64/96) of 256 rows each, p-major slot order, so the
RE<->walk transposes are single rectangular DMAs per quadrant.

Band-invalid cells get cost ~2e15 via host-side target padding (1e15
fill outside [0,N)), standing in for the explicit +inf band mask.

Backtrack: choice bits + g/L tables built in RE layout (42-pitch cell
regions whose per-block pad column doubles as the band pad and as the
scan-state reset), two full-width scans, g copied to walk layout by two
rectangular DMAs, then two serial walk loops (one STT+accum per row).
Path masks + L1/BCE metrics reduce in RE; host sums the partials.
"""

import numpy as np

import concourse.bacc as bacc
import concourse.bass as bass
import concourse.mybir as mybir
import concourse.tile as tile
from concourse.bass_utils import run_bass_kernel_spmd

B, N = 32, 1024
NB = 41                   # band width
PITCH = 42                # RE cell-block pitch (41 cells + 1 pad)
SP = 84                   # phase-A slot pitch (interleaved, 82 + 2 pads)
NBLK = 33                 # RE blocks (r = i+1 in [0,1056))
SKW = 1066                # skewed targ row length
NCORES = 8
BC = B // NCORES
BIGP = 1e30               # pad / DP "infinity"
FILL = 1e15               # targ pad fill -> invalid-cell cost ~2e15

AL = mybir.AluOpType
DT = mybir.dt.float32

RW = 1388                 # 42-pitch region width (lead pad + 33*42 + spare)
RW84 = 2773               # 84-pitch region width (lead pad + 33*84)

# ---- megaRE column offsets ----
TSK_O = 0                         # 3 * 1066 skewed targs (x, y, z)
PRE_O = TSK_O + 3 * SKW           # 3 * 33 preds (x, y, z)
CONST_O = PRE_O + 99              # iota 0..43
R1_O = CONST_O + 44               # |dx| cells (42-pitch)
R2_O = R1_O + RW                  # |dy| cells
SA_O = R2_O + RW                  # scratch A (isleft / mask)
SB_O = SA_O + RW                  # scratch B (gval -> gfull)
SC_O = SB_O + RW                  # scratch C (Lval -> Lfull -> metric tmp)
DS_O = SC_O + RW                  # d staging, 84-pitch interleaved (no lead)
DRE_O = DS_O + NBLK * SP          # D cells, 84-pitch (lead pad)
DREP_O = DRE_O + RW84             # D prev-row cells, 84-pitch (lead pad)
XC_O = DREP_O + RW84              # xcol [33]
OLO_O = XC_O + 33
CLZ_O = OLO_O + 33
NG1_O = CLZ_O + 33
NG2_O = NG1_O + 33
SPZ_O = NG2_O + 33
SPN_O = SPZ_O + 33
QZ_O = SPN_O + 33
RED_O = QZ_O + 33                 # Sx, Sy, Sbce, cnt
SCX_O = RED_O + 4                 # xcol repack lo (512)
SCX2_O = SCX_O + 512              # xcol repack hi (544)
REW = SCX2_O + 544 + 4

# ---- slots tile ----
# Phase-A D slots: quadrant k on partitions 32k..32k+4 holds rows with
# r//32 in [8k, 8k+BK_k), BK = (8,8,8,9), p-major 84-pitch slots.
BKS = (8, 8, 8, 9)
BSCQ = tuple(bk * 32 * SP for bk in BKS)     # boundary scratch col per quad
# Walk g slots: lo (rows 0..511) on partitions 0:4, 16 blocks; hi (rows
# 512..1055) on 64:68, 17 blocks; 42-pitch p-major.
LOW = 16 * PITCH
HIW = 17 * PITCH
GX = 512 * PITCH          # lo extra slot for g row 512
SLOTW = BSCQ[3] + SP + 4

_CACHE = {}


def _manual_ap(base, extra_off, dims):
    ap0 = [list(base.ap[0])]
    return bass.AP(base.tensor, base.offset + extra_off,
                   ap0 + [list(d) for d in dims])


def _dslot(r):
    k = min(r // 256, 3)
    return k, ((r % 32) * BKS[k] + (r // 32 - 8 * k)) * SP


def _gslot_lo(r):
    return ((r % 32) * 16 + r // 32) * PITCH


def _gslot_hi(r):
    return ((r % 32) * 17 + (r // 32 - 16)) * PITCH


def _build_module():
    nc = bacc.Bacc("TRN2", target_bir_lowering=False, debug=False,
                   num_devices=NCORES, detect_race_conditions=False)
    pre = nc.dram_tensor("pre", [128, 99], DT, kind="ExternalInput")
    tsk = nc.dram_tensor("tsk", [128, 3 * SKW], DT, kind="ExternalInput")
    cst = nc.dram_tensor("cst", [128, 44], DT, kind="ExternalInput")
    partials = nc.dram_tensor("partials", [128, 4], DT, kind="ExternalOutput")

    with tile.TileContext(nc) as tc:
        with tc.tile_pool(name="main", bufs=1) as pool:
            megaRE = pool.tile([128, REW], DT)
            slots = pool.tile([128, SLOTW], DT)
            xhl = pool.tile([128, 520], DT)
            xhh = pool.tile([128, 560], DT)
            wks = pool.tile([128, 96], DT)
            _emit(nc, tc, megaRE, slots, xhl, xhh, wks,
                  pre, tsk, cst, partials)
    nc.compile()
    return nc


def _raw_scan(nc, out, data0, data1, initial, op0, op1):
    eng = nc.vector
    return eng.add_instruction(mybir.InstTensorScalarPtr(
        name=eng.bass.get_next_instruction_name(),
        is_tensor_tensor_scan=True,
        is_scalar_tensor_tensor=True,
        op0=op0, op1=op1,
        ins=[eng.lower_ap(data0), eng.lower_ap_or_imm(initial),
             eng.lower_ap(data1)],
        outs=[eng.lower_ap(out)]))


def _emit(nc, tc, megaRE, slots, xhl, xhh, wks, pre, tsk, cst, partials):
    v = nc.vector

    def cells(off, dc=0):
        s = off + 1 + dc
        return megaRE[:, s:s + NBLK * PITCH].rearrange(
            "p (b c) -> p b c", c=PITCH)[:, :, 0:NB]

    def cells84(base_col):
        return _manual_ap(megaRE[:, base_col:base_col + 1], 0,
                          [[SP, NBLK], [2, NB]])

    def smb(off):
        return megaRE[:, off:off + NBLK].unsqueeze(2) \
            .broadcast_to([128, NBLK, NB])

    def skw(k):
        return _manual_ap(megaRE[:, TSK_O + k * SKW:TSK_O + k * SKW + 1], 0,
                          [[32, NBLK], [1, NB]])

    def padset(off, val):
        v.memset(_manual_ap(megaRE[:, off + 1 + NB:off + 2 + NB], 0,
                            [[PITCH, NBLK]]), val)
        v.memset(megaRE[:, off:off + 1], val)

    # ---------------- input DMAs ----------------
    nc.sync.dma_start(out=megaRE[:, PRE_O:PRE_O + 99], in_=pre[:])
    nc.sync.dma_start(out=megaRE[:, CONST_O:CONST_O + 44], in_=cst[:])
    nc.sync.dma_start(out=wks[:, 0:NB], in_=cst[:, 1:1 + NB])   # iota 1..41
    for k in range(3):
        nc.sync.dma_start(out=megaRE[:, TSK_O + k * SKW:TSK_O + (k + 1) * SKW],
                          in_=tsk[:, k * SKW:(k + 1) * SKW])

    ocol = megaRE[:, CONST_O + 1:CONST_O + 1 + NB].unsqueeze(1) \
        .broadcast_to([128, NBLK, NB])
    ocp1 = megaRE[:, CONST_O + 2:CONST_O + 2 + NB].unsqueeze(1) \
        .broadcast_to([128, NBLK, NB])

    # ---------------- d build (interleaved 84-pitch staging) ----------------
    v.memset(megaRE[:, DS_O:DS_O + NBLK * SP], 0.0)
    v.memset(_manual_ap(megaRE[:, DS_O + 82:DS_O + 83], 0, [[SP, NBLK]]), BIGP)
    v.memset(_manual_ap(megaRE[:, DS_O + 83:DS_O + 84], 0, [[SP, NBLK]]), BIGP)
    v.tensor_tensor(out=cells(SA_O), in0=smb(PRE_O), in1=skw(0),
                    op=AL.subtract)
    v.scalar_tensor_tensor(out=cells(R1_O), in0=cells(SA_O), scalar=-1.0,
                           in1=cells(SA_O), op0=AL.mult, op1=AL.max)
    v.tensor_tensor(out=cells(SA_O), in0=smb(PRE_O + 33), in1=skw(1),
                    op=AL.subtract)
    v.scalar_tensor_tensor(out=cells(R2_O), in0=cells(SA_O), scalar=-1.0,
                           in1=cells(SA_O), op0=AL.mult, op1=AL.max)
    v.tensor_tensor(out=cells84(DS_O + 1), in0=cells(R1_O), in1=cells(R2_O),
                    op=AL.add)

    # ---------------- stage d into quadrant slots ----------------
    for k in range(4):
        w = BKS[k] * SP
        nc.sync.dma_start(
            out=_manual_ap(slots[32 * k:32 * k + 4, 0:1], 0,
                           [[w, 32], [1, w]]),
            in_=_manual_ap(megaRE[:, DS_O + 8 * k * SP:DS_O + 8 * k * SP + 1],
                           0, [[1, w]]))
    # virtual row r=0 (quadrant 0, slot 0): odd cols BIG, col 41 (o=20) = 0
    v.memset(slots[0:4, 0:SP], BIGP)
    v.memset(slots[0:4, 41:42], 0.0)

    # ---------------- phase A: one fused scan per row ----------------
    def arow(q0, cur, prevbase):
        _raw_scan(nc, out=slots[q0:q0 + 4, cur:cur + 82],
                  data0=_manual_ap(slots[q0:q0 + 4, 0:1], prevbase + 1,
                                   [[2, NB], [2, 2]]),
                  data1=slots[q0:q0 + 4, cur:cur + 82],
                  initial=BIGP, op0=AL.min, op1=AL.add)

    v.memset(megaRE[:, DRE_O:DRE_O + 1], BIGP)           # lead pad
    for k in range(4):
        q0 = 32 * k
        r0 = max(1, 256 * k)
        r1 = N if k == 3 else 256 * k + 255
        if k > 0:                                        # boundary from k-1
            _, pc = _dslot(r0 - 1)
            nc.sync.dma_start(out=slots[q0:q0 + 4, BSCQ[k]:BSCQ[k] + SP],
                              in_=slots[q0 - 32:q0 - 28, pc:pc + SP])
            arow(q0, _dslot(r0)[1], BSCQ[k])
            r0 += 1
        for r in range(r0, r1 + 1):
            arow(q0, _dslot(r)[1], _dslot(r - 1)[1])
        # transpose this quadrant's D out (overlaps next quadrant's DP)
        w = BKS[k] * SP
        nc.sync.dma_start(
            out=_manual_ap(megaRE[:, DRE_O + 1 + 8 * k * SP:
                                   DRE_O + 2 + 8 * k * SP], 0, [[1, w]]),
            in_=_manual_ap(slots[q0:q0 + 4, 0:1], 0, [[w, 32], [1, w]]))

    # ---------------- DrePrev: partition shift ----------------
    v.memset(megaRE[:, DREP_O:DREP_O + 1 + SP], BIGP)    # row-0 prev junk
    for s in range(4):
        nc.sync.dma_start(
            out=megaRE[32 * s + 1:32 * s + 32, DREP_O:DREP_O + RW84],
            in_=megaRE[32 * s:32 * s + 31, DRE_O:DRE_O + RW84])
        nc.sync.dma_start(
            out=megaRE[32 * s:32 * s + 1, DREP_O + 1 + SP:DREP_O + RW84],
            in_=megaRE[32 * s + 31:32 * s + 32, DRE_O + 1:DRE_O + RW84 - SP])

    # ---------------- choice bits + g/L scans ----------------
    diag = cells84(DREP_O + 2)
    up = cells84(DREP_O + 4)
    left = cells84(DRE_O)
    v.tensor_tensor(out=cells(SA_O), in0=diag, in1=up, op=AL.is_le)
    v.tensor_tensor(out=cells(SB_O), in0=diag, in1=left, op=AL.is_le)
    v.tensor_tensor(out=cells(SB_O), in0=cells(SA_O), in1=cells(SB_O),
                    op=AL.mult)                          # isdiag
    v.tensor_tensor(out=cells(SA_O), in0=left, in1=diag, op=AL.is_lt)
    v.tensor_tensor(out=cells(SC_O), in0=left, in1=up, op=AL.is_lt)
    v.tensor_tensor(out=cells(SA_O), in0=cells(SA_O), in1=cells(SC_O),
                    op=AL.mult)                          # isleft
    v.tensor_single_scalar(out=cells(SC_O), in_=cells(SA_O), scalar=0.0,
                           op=AL.is_equal)               # notleft
    v.tensor_tensor(out=cells(SB_O), in0=ocp1, in1=cells(SB_O),
                    op=AL.subtract)                      # oc+2-isdiag
    v.tensor_tensor(out=cells(SB_O), in0=cells(SB_O), in1=cells(SC_O),
                    op=AL.mult)                          # gval
    v.tensor_tensor(out=cells(SC_O), in0=ocol, in1=cells(SC_O),
                    op=AL.mult)                          # Lval
    padset(SA_O, 0.0)
    padset(SB_O, 0.0)
    padset(SC_O, 0.0)
    v.tensor_tensor_scan(out=megaRE[:, SB_O:SB_O + 1387],
                         data0=megaRE[:, SA_O:SA_O + 1387],
                         data1=megaRE[:, SB_O:SB_O + 1387],
                         initial=0.0, op0=AL.mult, op1=AL.add)   # gfull
    v.tensor_tensor_scan(out=megaRE[:, SC_O:SC_O + 1387],
                         data0=megaRE[:, SA_O:SA_O + 1387],
                         data1=megaRE[:, SC_O:SC_O + 1387],
                         initial=0.0, op0=AL.mult, op1=AL.add)   # Lfull

    # ---------------- g -> walk slots (42-pitch, p-major) ----------------
    nc.sync.dma_start(
        out=_manual_ap(slots[64:68, 0:1], 0, [[HIW, 32], [1, HIW]]),
        in_=_manual_ap(megaRE[:, SB_O + 1 + LOW:SB_O + 2 + LOW], 0,
                       [[1, HIW]]))
    nc.sync.dma_start(
        out=_manual_ap(slots[0:4, 0:1], 0, [[LOW, 32], [1, LOW]]),
        in_=_manual_ap(megaRE[:, SB_O + 1:SB_O + 2], 0, [[1, LOW]]))
    for s in range(4):      # lo extra: g row 512
        nc.sync.dma_start(
            out=slots[s:s + 1, GX:GX + NB],
            in_=megaRE[32 * s:32 * s + 1, SB_O + 1 + LOW:SB_O + 1 + LOW + NB])

    # ---------------- walk ----------------
    v.memset(xhh[64:68, 512:513], 21.0)                  # x_1023
    v.memset(xhh[64:68, 513:544], 0.0)                   # junk rows > 1023
    v.memset(xhl[0:4, 0:1], 0.0)                         # x_{-1} junk
    for i in range(1023, 511, -1):
        gc = _gslot_hi(i + 1)
        v.scalar_tensor_tensor(
            out=wks[64:68, 41:41 + NB], in0=wks[64:68, 0:NB],
            scalar=xhh[64:68, i - 511:i - 510],
            in1=slots[64:68, gc:gc + NB],
            op0=AL.is_equal, op1=AL.mult,
            accum_out=xhh[64:68, i - 512:i - 511])
    nc.sync.dma_start(out=xhl[0:4, 512:513], in_=xhh[64:68, 0:1])
    for i in range(511, 0, -1):
        gc = GX if i == 511 else _gslot_lo(i + 1)
        v.scalar_tensor_tensor(
            out=wks[0:4, 41:41 + NB], in0=wks[0:4, 0:NB],
            scalar=xhl[0:4, i + 1:i + 2],
            in1=slots[0:4, gc:gc + NB],
            op0=AL.is_equal, op1=AL.mult,
            accum_out=xhl[0:4, i:i + 1])

    # ---------------- xcol repack + DMA ----------------
    nc.vector.tensor_scalar_add(
        out=_manual_ap(megaRE[64:68, SCX2_O:SCX2_O + 1], 0, [[17, 32], [1, 17]]),
        in0=_manual_ap(xhh[64:68, 0:1], 0, [[1, 32], [32, 17]]), scalar1=0.0)
    nc.vector.tensor_scalar_add(
        out=_manual_ap(megaRE[0:4, SCX_O:SCX_O + 1], 0, [[16, 32], [1, 16]]),
        in0=_manual_ap(xhl[0:4, 0:1], 0, [[1, 32], [32, 16]]), scalar1=0.0)
    nc.sync.dma_start(
        out=_manual_ap(megaRE[:, XC_O + 16:XC_O + 17], 0, [[1, 17]]),
        in_=_manual_ap(megaRE[64:68, SCX2_O:SCX2_O + 1], 0, [[17, 32], [1, 17]]))
    nc.sync.dma_start(
        out=_manual_ap(megaRE[:, XC_O:XC_O + 1], 0, [[1, 16]]),
        in_=_manual_ap(megaRE[0:4, SCX_O:SCX_O + 1], 0, [[16, 32], [1, 16]]))

    # ---------------- olo + mask ----------------
    xcolb = smb(XC_O)
    v.tensor_tensor(out=cells(SA_O), in0=ocol, in1=xcolb, op=AL.is_equal)
    v.tensor_tensor(out=cells(SA_O), in0=cells(SA_O), in1=cells(SC_O),
                    op=AL.mult)
    v.tensor_reduce(out=megaRE[:, OLO_O:OLO_O + NBLK], in_=cells(SA_O),
                    axis=mybir.AxisListType.X, op=AL.add)
    v.tensor_tensor(out=cells(SA_O), in0=ocol, in1=smb(OLO_O), op=AL.is_ge)
    v.tensor_tensor(out=cells(SB_O), in0=ocol, in1=xcolb, op=AL.is_le)
    v.tensor_tensor(out=cells(SA_O), in0=cells(SA_O), in1=cells(SB_O),
                    op=AL.mult)                          # mask
    b32 = SA_O + 1 + 32 * PITCH
    v.memset(megaRE[:, b32:b32 + NB], 0.0)               # junk rows 1025..1055
    for s in range(4):                                   # rebuild row 1023 mask
        q = 32 * s
        ic0 = megaRE[q:q + 1, CONST_O + 1:CONST_O + 1 + NB]
        wt = wks[q:q + 1, 41:41 + NB]
        v.scalar_tensor_tensor(out=wt, in0=ic0,
                               scalar=megaRE[q:q + 1, OLO_O + 32:OLO_O + 33],
                               in1=ic0, op0=AL.is_ge, op1=AL.bypass)
        v.scalar_tensor_tensor(out=megaRE[q:q + 1, b32:b32 + NB], in0=ic0,
                               scalar=megaRE[q:q + 1, XC_O + 32:XC_O + 33],
                               in1=wt, op0=AL.is_le, op1=AL.mult)
    for s in range(4):                                   # virtual row 0
        v.memset(megaRE[32 * s:32 * s + 1, SA_O + 1:SA_O + 1 + NB], 0.0)

    # ---------------- metrics ----------------
    v.tensor_tensor(out=cells(SC_O), in0=cells(R1_O), in1=cells(SA_O),
                    op=AL.mult)
    v.tensor_reduce(out=megaRE[:, RED_O:RED_O + 1], in_=cells(SC_O),
                    axis=mybir.AxisListType.XY, op=AL.add)
    v.tensor_tensor(out=cells(SC_O), in0=cells(R2_O), in1=cells(SA_O),
                    op=AL.mult)
    v.tensor_reduce(out=megaRE[:, RED_O + 1:RED_O + 2], in_=cells(SC_O),
                    axis=mybir.AxisListType.XY, op=AL.add)
    v.tensor_scalar(out=megaRE[:, CLZ_O:CLZ_O + NBLK],
                    in0=megaRE[:, PRE_O + 66:PRE_O + 99],
                    scalar1=-4.0, scalar2=4.0, op0=AL.max, op1=AL.min)
    nc.scalar.activation(megaRE[:, NG1_O:NG1_O + NBLK],
                         megaRE[:, CLZ_O:CLZ_O + NBLK],
                         mybir.ActivationFunctionType.Exp)
    nc.scalar.activation(megaRE[:, NG2_O:NG2_O + NBLK],
                         megaRE[:, CLZ_O:CLZ_O + NBLK],
                         mybir.ActivationFunctionType.Exp, scale=-1.0)
    nc.scalar.activation(megaRE[:, SPZ_O:SPZ_O + NBLK],
                         megaRE[:, NG1_O:NG1_O + NBLK],
                         mybir.ActivationFunctionType.Ln, bias=1.0)
    nc.scalar.activation(megaRE[:, SPN_O:SPN_O + NBLK],
                         megaRE[:, NG2_O:NG2_O + NBLK],
                         mybir.ActivationFunctionType.Ln, bias=1.0)
    v.scalar_tensor_tensor(out=megaRE[:, QZ_O:QZ_O + NBLK],
                           in0=megaRE[:, SPN_O:SPN_O + NBLK], scalar=5.0,
                           in1=megaRE[:, SPZ_O:SPZ_O + NBLK],
                           op0=AL.mult, op1=AL.subtract)
    v.tensor_tensor(out=cells(SC_O), in0=skw(2), in1=smb(QZ_O), op=AL.mult)
    v.tensor_tensor(out=cells(SC_O), in0=cells(SC_O), in1=smb(SPZ_O),
                    op=AL.add)
    v.tensor_tensor(out=cells(SC_O), in0=cells(SC_O), in1=cells(SA_O),
                    op=AL.mult)
    v.tensor_reduce(out=megaRE[:, RED_O + 2:RED_O + 3], in_=cells(SC_O),
                    axis=mybir.AxisListType.XY, op=AL.add)
    v.tensor_reduce(out=megaRE[:, RED_O + 3:RED_O + 4], in_=cells(SA_O),
                    axis=mybir.AxisListType.XY, op=AL.add)

    nc.sync.dma_start(out=partials[:], in_=megaRE[:, RED_O:RED_O + 4])


def _get_module():
    if "nc" not in _CACHE:
        _CACHE["nc"] = _build_module()
    return _CACHE["nc"]


def _make_inmaps(preds, targs):
    preds = np.ascontiguousarray(preds, dtype=np.float32)
    targs = np.ascontiguousarray(targs, dtype=np.float32)
    cst = np.tile(np.arange(44, dtype=np.float32), (128, 1))
    pp = np.arange(32)
    bb = np.arange(NBLK)
    r_idx = pp[:, None] + 32 * bb[None, :]              # [32, 33] = i + 1
    r_ok = (r_idx >= 1) & (r_idx <= N)
    r_cl = np.clip(r_idx - 1, 0, N - 1)
    uu = np.arange(SKW)
    t_idx = uu[None, :] + pp[:, None] - 21              # [32, SKW]
    t_ok = (t_idx >= 0) & (t_idx < N)
    t_cl = np.clip(t_idx, 0, N - 1)
    in_maps = []
    for c in range(NCORES):
        ps = preds[c * BC:(c + 1) * BC]
        ts = targs[c * BC:(c + 1) * BC]
        prev = np.zeros((BC, 32, 3 * NBLK), dtype=np.float32)
        tskv = np.zeros((BC, 32, 3 * SKW), dtype=np.float32)
        for k in range(3):
            vv = ps[:, :, k][:, r_cl]                   # [BC, 32, NBLK]
            prev[:, :, k * NBLK:(k + 1) * NBLK] = np.where(r_ok[None], vv, 0.0)
            fill = FILL if k < 2 else 0.0
            ww = ts[:, :, k][:, t_cl]                   # [BC, 32, SKW]
            tskv[:, :, k * SKW:(k + 1) * SKW] = np.where(t_ok[None], ww, fill)
        in_maps.append({"pre": prev.reshape(128, 3 * NBLK),
                        "tsk": tskv.reshape(128, 3 * SKW), "cst": cst})
    return in_maps


def _reduce_host(parts_list, subcoef):
    c0, c1 = float(subcoef[0]), float(subcoef[1])
    loss = 0.0
    for parts in parts_list:
        m = parts.reshape(BC, 32, 4).sum(axis=1)        # [BC, (Sx,Sy,Sb,cnt)]
        for s in range(BC):
            sx, sy, sb, cnt = (float(m[s, k]) for k in range(4))
            loss += c0 * sx + c1 * sy + 0.1 * sb / cnt
    return np.float32(loss)


def run(preds, targs, subcoef, trace=False):
    nc = _get_module()
    in_maps = _make_inmaps(preds, targs)
    res = run_bass_kernel_spmd(nc, in_maps, core_ids=list(range(NCORES)),
                               trace=trace)
    parts = [r["partials"] for r in res.results]
    return _reduce_host(parts, np.asarray(subcoef)), res


def kernel(preds, targs, subcoef):
    out, _ = run(preds, targs, subcoef)
    return out


# revision 14
# speedup vs baseline: 1.4268x; 1.0312x over previous
"""Banded DTW loss kernel for Trainium2 (Bass/Tile), 8-core data-parallel.

Layout: sample-major partitions q = 32*s + p (s = sample 0..3, p = row%32).

Phase A (forward DP) uses ONE fused 82-wide tensor_tensor_scan per row:
the row slot holds d interleaved with zeros (d[o] at odd col 2o+1), and
data0 is a 3-D overlapping-pair window over the previous row's slot
(odd cols hold D).  Scan semantics state = min(data0[t], state) + data1[t]
then give, per band cell o:
    t=2o   : m   = min(D'[o],   D[o-1]) + 0
    t=2o+1 : D[o]= min(D'[o+1], m     ) + d[o]
which is exactly the banded DTW recurrence.  The scan writes the slot in
place (d -> interleaved m/D).  Rows are distributed over 4 partition
quadrants (0/32/64/96) of 256 rows each, p-major slot order, so the
RE<->walk transposes are single rectangular DMAs per quadrant.

Band-invalid cells get cost ~2e15 via host-side target padding (1e15
fill outside [0,N)), standing in for the explicit +inf band mask.

Backtrack: choice bits + g/L tables built in RE layout (42-pitch cell
regions whose per-block pad column doubles as the band pad and as the
scan-state reset), two full-width scans, g copied to walk layout by two
rectangular DMAs, then two serial walk loops (one STT+accum per row).
Path masks + L1/BCE metrics reduce in RE; host sums the partials.
"""

import numpy as np

import concourse.bacc as bacc
import concourse.bass as bass
import concourse.mybir as mybir
import concourse.tile as tile
from concourse.bass_utils import run_bass_kernel_spmd

B, N = 32, 1024
NB = 41                   # band width
PITCH = 42                # RE cell-block pitch (41 cells + 1 pad)
SP = 84                   # phase-A slot pitch (interleaved, 82 + 2 pads)
NBLK = 33                 # RE blocks (r = i+1 in [0,1056))
SKW = 1066                # skewed targ row length
NCORES = 8
BC = B // NCORES
BIGP = 1e30               # pad / DP "infinity"
FILL = 1e15               # targ pad fill -> invalid-cell cost ~2e15

AL = mybir.AluOpType
DT = mybir.dt.float32

RW = 1388                 # 42-pitch region width (lead pad + 33*42 + spare)
RW84 = 2773               # 84-pitch region width (lead pad + 33*84)

# ---- megaRE column offsets ----
TSK_O = 0                         # 3 * 1066 skewed targs (x, y, z)
PRE_O = TSK_O + 3 * SKW           # 3 * 33 preds (x, y, z)
CONST_O = PRE_O + 99              # iota 0..43
R1_O = CONST_O + 44               # |dx| cells (42-pitch)
R2_O = R1_O + RW                  # |dy| cells
SA_O = R2_O + RW                  # scratch A (isleft / mask)
SB_O = SA_O + RW                  # scratch B (gval -> gfull)
SC_O = SB_O + RW                  # scratch C (Lval -> Lfull -> metric tmp)
DS_O = SC_O + RW                  # d staging, 84-pitch interleaved (no lead)
DRE_O = DS_O + NBLK * SP          # D cells, 84-pitch (lead pad)
DREP_O = DRE_O + RW84             # D prev-row cells, 84-pitch (lead pad)
XC_O = DREP_O + RW84              # xcol [33]
OLO_O = XC_O + 33
CLZ_O = OLO_O + 33
NG1_O = CLZ_O + 33
NG2_O = NG1_O + 33
SPZ_O = NG2_O + 33
SPN_O = SPZ_O + 33
QZ_O = SPN_O + 33
RED_O = QZ_O + 33                 # Sx, Sy, Sbce, cnt
SCX_O = RED_O + 4                 # xcol repack lo (512)
SCX2_O = SCX_O + 512              # xcol repack hi (544)
REW = SCX2_O + 544 + 4

# ---- slots tile ----
# Phase-A D slots: quadrant k on partitions 32k..32k+4 holds rows with
# r//32 in [8k, 8k+BK_k), BK = (8,8,8,9), p-major 84-pitch slots.
BKS = (8, 8, 8, 9)
BSCQ = tuple(bk * 32 * SP for bk in BKS)     # boundary scratch col per quad
# Walk g slots: lo (rows 0..511) on partitions 0:4, 16 blocks; hi (rows
# 512..1055) on 64:68, 17 blocks; 42-pitch p-major.
LOW = 16 * PITCH
HIW = 17 * PITCH
GX = 512 * PITCH          # lo extra slot for g row 512
SLOTW = BSCQ[3] + SP + 4

_CACHE = {}


def _manual_ap(base, extra_off, dims):
    ap0 = [list(base.ap[0])]
    return bass.AP(base.tensor, base.offset + extra_off,
                   ap0 + [list(d) for d in dims])


def _dslot(r):
    k = min(r // 256, 3)
    return k, ((r % 32) * BKS[k] + (r // 32 - 8 * k)) * SP


def _gslot_lo(r):
    return ((r % 32) * 16 + r // 32) * PITCH


def _gslot_hi(r):
    return ((r % 32) * 17 + (r // 32 - 16)) * PITCH


def _build_module():
    nc = bacc.Bacc("TRN2", target_bir_lowering=False, debug=False,
                   num_devices=NCORES, detect_race_conditions=False)
    pre = nc.dram_tensor("pre", [128, 99], DT, kind="ExternalInput")
    tsk = nc.dram_tensor("tsk", [128, 3 * SKW], DT, kind="ExternalInput")
    cst = nc.dram_tensor("cst", [128, 44], DT, kind="ExternalInput")
    partials = nc.dram_tensor("partials", [128, 4], DT, kind="ExternalOutput")

    with tile.TileContext(nc) as tc:
        with tc.tile_pool(name="main", bufs=1) as pool:
            megaRE = pool.tile([128, REW], DT)
            slots = pool.tile([128, SLOTW], DT)
            xhl = pool.tile([128, 520], DT)
            xhh = pool.tile([128, 560], DT)
            wks = pool.tile([128, 96], DT)
            _emit(nc, tc, megaRE, slots, xhl, xhh, wks,
                  pre, tsk, cst, partials)
    nc.compile()
    return nc


def _raw_scan(nc, out, data0, data1, initial, op0, op1):
    eng = nc.vector
    return eng.add_instruction(mybir.InstTensorScalarPtr(
        name=eng.bass.get_next_instruction_name(),
        is_tensor_tensor_scan=True,
        is_scalar_tensor_tensor=True,
        op0=op0, op1=op1,
        ins=[eng.lower_ap(data0), eng.lower_ap_or_imm(initial),
             eng.lower_ap(data1)],
        outs=[eng.lower_ap(out)]))


def _emit(nc, tc, megaRE, slots, xhl, xhh, wks, pre, tsk, cst, partials):
    v = nc.vector
    U_O = DS_O        # u2 (bce per-cell) region, 42-pitch, reuses d staging

    def rcells(off, b0, nb, dc=0):
        return _manual_ap(megaRE[:, off + 1 + PITCH * b0 + dc:
                                 off + 2 + PITCH * b0 + dc], 0,
                          [[PITCH, nb], [1, NB]])

    def rsmb(off, b0, nb):
        return megaRE[:, off + b0:off + b0 + nb].unsqueeze(2) \
            .broadcast_to([128, nb, NB])

    def rskw(k, b0, nb):
        return _manual_ap(megaRE[:, TSK_O + k * SKW + 32 * b0:
                                 TSK_O + k * SKW + 32 * b0 + 1], 0,
                          [[32, nb], [1, NB]])

    def rocol(nb, dc=0):
        return megaRE[:, CONST_O + 1 + dc:CONST_O + 1 + dc + NB] \
            .unsqueeze(1).broadcast_to([128, nb, NB])

    def rcells84(base_col, b0, nb):
        return _manual_ap(megaRE[:, base_col + SP * b0:base_col + SP * b0 + 1],
                          0, [[SP, nb], [2, NB]])

    # ---------------- input DMAs ----------------
    nc.sync.dma_start(out=megaRE[:, PRE_O:PRE_O + 99], in_=pre[:])
    nc.sync.dma_start(out=megaRE[:, CONST_O:CONST_O + 44], in_=cst[:])
    nc.sync.dma_start(out=wks[:, 0:NB], in_=cst[:, 1:1 + NB])   # iota 1..41
    for k in range(3):
        nc.sync.dma_start(out=megaRE[:, TSK_O + k * SKW:TSK_O + (k + 1) * SKW],
                          in_=tsk[:, k * SKW:(k + 1) * SKW])

    # ---------------- bce scalars (Act engine; overlaps everything) --------
    v.tensor_scalar(out=megaRE[:, CLZ_O:CLZ_O + NBLK],
                    in0=megaRE[:, PRE_O + 66:PRE_O + 99],
                    scalar1=-4.0, scalar2=4.0, op0=AL.max, op1=AL.min)
    nc.scalar.activation(megaRE[:, NG1_O:NG1_O + NBLK],
                         megaRE[:, CLZ_O:CLZ_O + NBLK],
                         mybir.ActivationFunctionType.Exp)
    nc.scalar.activation(megaRE[:, NG2_O:NG2_O + NBLK],
                         megaRE[:, CLZ_O:CLZ_O + NBLK],
                         mybir.ActivationFunctionType.Exp, scale=-1.0)
    nc.scalar.activation(megaRE[:, SPZ_O:SPZ_O + NBLK],
                         megaRE[:, NG1_O:NG1_O + NBLK],
                         mybir.ActivationFunctionType.Ln, bias=1.0)
    nc.scalar.activation(megaRE[:, SPN_O:SPN_O + NBLK],
                         megaRE[:, NG2_O:NG2_O + NBLK],
                         mybir.ActivationFunctionType.Ln, bias=1.0)
    v.scalar_tensor_tensor(out=megaRE[:, QZ_O:QZ_O + NBLK],
                           in0=megaRE[:, SPN_O:SPN_O + NBLK], scalar=5.0,
                           in1=megaRE[:, SPZ_O:SPZ_O + NBLK],
                           op0=AL.mult, op1=AL.subtract)

    # scratch-region pads/leads: zeroed once (scan-state resets + uninit)
    for off in (SA_O, SB_O, SC_O):
        v.memset(_manual_ap(megaRE[:, off + 1 + NB:off + 2 + NB], 0,
                            [[PITCH, NBLK]]), 0.0)
        v.memset(megaRE[:, off:off + 1], 0.0)

    # ---------------- d build (84-pitch interleaved), quadrant 0 first -----
    def dbuild(b0, nb):
        w0 = DS_O + SP * b0
        v.memset(megaRE[:, w0:w0 + SP * nb], 0.0)
        v.memset(_manual_ap(megaRE[:, w0 + 82:w0 + 83], 0, [[SP, nb]]), BIGP)
        v.memset(_manual_ap(megaRE[:, w0 + 83:w0 + 84], 0, [[SP, nb]]), BIGP)
        v.tensor_tensor(out=rcells(SA_O, b0, nb), in0=rsmb(PRE_O, b0, nb),
                        in1=rskw(0, b0, nb), op=AL.subtract)
        v.scalar_tensor_tensor(out=rcells(R1_O, b0, nb),
                               in0=rcells(SA_O, b0, nb), scalar=-1.0,
                               in1=rcells(SA_O, b0, nb),
                               op0=AL.mult, op1=AL.max)
        v.tensor_tensor(out=rcells(SA_O, b0, nb), in0=rsmb(PRE_O + 33, b0, nb),
                        in1=rskw(1, b0, nb), op=AL.subtract)
        v.scalar_tensor_tensor(out=rcells(R2_O, b0, nb),
                               in0=rcells(SA_O, b0, nb), scalar=-1.0,
                               in1=rcells(SA_O, b0, nb),
                               op0=AL.mult, op1=AL.max)
        v.tensor_tensor(out=rcells84(DS_O + 1, b0, nb),
                        in0=rcells(R1_O, b0, nb), in1=rcells(R2_O, b0, nb),
                        op=AL.add)

    def stage(k):
        w = BKS[k] * SP
        nc.sync.dma_start(
            out=_manual_ap(slots[32 * k:32 * k + 4, 0:1], 0,
                           [[w, 32], [1, w]]),
            in_=_manual_ap(megaRE[:, DS_O + 8 * k * SP:DS_O + 8 * k * SP + 1],
                           0, [[1, w]]))

    dbuild(0, 8)
    stage(0)
    # virtual row r=0 (quadrant 0, slot 0): odd cols BIG, col 41 (o=20) = 0
    v.memset(slots[0:4, 0:SP], BIGP)
    v.memset(slots[0:4, 41:42], 0.0)
    dbuild(8, 25)
    for k in range(1, 4):
        stage(k)

    # ---------------- chunk machinery ----------------
    chunks = []

    def drain(n=1):
        for _ in range(n):
            if chunks:
                chunks.pop(0)()

    # phase-A chunks: u2 = sp + tz*qz per block (2 ops each)
    for b in range(NBLK):
        chunks.append(lambda b=b: v.tensor_tensor(
            out=rcells(U_O, b, 1), in0=rskw(2, b, 1), in1=rsmb(QZ_O, b, 1),
            op=AL.mult))
        chunks.append(lambda b=b: v.tensor_tensor(
            out=rcells(U_O, b, 1), in0=rcells(U_O, b, 1),
            in1=rsmb(SPZ_O, b, 1), op=AL.add))

    # ---------------- phase A: one fused scan per row ----------------
    def arow(q0, cur, prevbase):
        _raw_scan(nc, out=slots[q0:q0 + 4, cur:cur + 82],
                  data0=_manual_ap(slots[q0:q0 + 4, 0:1], prevbase + 1,
                                   [[2, NB], [2, 2]]),
                  data1=slots[q0:q0 + 4, cur:cur + 82],
                  initial=BIGP, op0=AL.min, op1=AL.add)

    v.memset(megaRE[:, DRE_O:DRE_O + 1], BIGP)           # lead pad
    v.memset(megaRE[:, DREP_O:DREP_O + 1 + SP], BIGP)    # row-0 prev junk

    def dshift(c0, cw):
        """DrePrev partition shift for DRE col range [c0, c0+cw)."""
        d0 = max(c0, 1 + SP)      # p=0 rows: dst col x <- src col x - 84
        for s in range(4):
            nc.sync.dma_start(
                out=megaRE[32 * s + 1:32 * s + 32, DREP_O + c0:DREP_O + c0 + cw],
                in_=megaRE[32 * s:32 * s + 31, DRE_O + c0:DRE_O + c0 + cw])
            nc.sync.dma_start(
                out=megaRE[32 * s:32 * s + 1,
                           DREP_O + d0:DREP_O + c0 + cw],
                in_=megaRE[32 * s + 31:32 * s + 32,
                           DRE_O + d0 - SP:DRE_O + c0 + cw - SP])

    for k in range(4):
        q0 = 32 * k
        r0 = max(1, 256 * k)
        r1 = N if k == 3 else 256 * k + 255
        if k > 0:                                        # boundary from k-1
            _, pc = _dslot(r0 - 1)
            nc.sync.dma_start(out=slots[q0:q0 + 4, BSCQ[k]:BSCQ[k] + SP],
                              in_=slots[q0 - 32:q0 - 28, pc:pc + SP])
            arow(q0, _dslot(r0)[1], BSCQ[k])
            r0 += 1
        for r in range(r0, r1 + 1):
            arow(q0, _dslot(r)[1], _dslot(r - 1)[1])
            if r % 4 == 0 and r > 64:
                drain()
        # transpose this quadrant's D out (overlaps next quadrant's DP)
        w = BKS[k] * SP
        nc.sync.dma_start(
            out=_manual_ap(megaRE[:, DRE_O + 1 + 8 * k * SP:
                                   DRE_O + 2 + 8 * k * SP], 0, [[1, w]]),
            in_=_manual_ap(slots[q0:q0 + 4, 0:1], 0, [[w, 32], [1, w]]))
        if k == 1:      # DrePrev shifts for blocks 0..15 (needs DRE b 0..15)
            dshift(0, 1 + 16 * SP)
        if k == 3:      # DrePrev shifts for blocks 16..32
            dshift(1 + 16 * SP, RW84 - 1 - 16 * SP)
    drain(len(chunks))

    # ---------------- choice bits (min-trick, 8 ops per range) -------------
    def choice(b0, nb):
        diag = rcells84(DREP_O + 2, b0, nb)
        up = rcells84(DREP_O + 4, b0, nb)
        left = rcells84(DRE_O, b0, nb)
        sa, sb, sc = (rcells(o, b0, nb) for o in (SA_O, SB_O, SC_O))
        yield lambda: v.tensor_tensor(out=sb, in0=diag, in1=up, op=AL.min)
        yield lambda: v.tensor_tensor(out=sa, in0=left, in1=sb, op=AL.is_lt)
        yield lambda: v.tensor_tensor(out=sb, in0=up, in1=left, op=AL.min)
        yield lambda: v.tensor_tensor(out=sb, in0=diag, in1=sb, op=AL.is_le)
        yield lambda: v.tensor_single_scalar(out=sc, in_=sa, scalar=0.0,
                                             op=AL.is_equal)    # notleft
        yield lambda: v.scalar_tensor_tensor(out=sb, in0=sb, scalar=-1.0,
                                             in1=rocol(nb, 1), op0=AL.mult,
                                             op1=AL.add)        # oc+2-isdiag
        yield lambda: v.tensor_tensor(out=sb, in0=sb, in1=sc, op=AL.mult)
        yield lambda: v.tensor_tensor(out=sc, in0=rocol(nb), in1=sc,
                                      op=AL.mult)               # Lval

    for f in choice(16, 17):                             # hi half: critical
        f()
    # (SA/SB pads pre-zeroed -> scan state resets at block boundaries)
    v.tensor_tensor_scan(out=megaRE[:, SB_O + 1 + 16 * PITCH:SB_O + 1387],
                         data0=megaRE[:, SA_O + 1 + 16 * PITCH:SA_O + 1387],
                         data1=megaRE[:, SB_O + 1 + 16 * PITCH:SB_O + 1387],
                         initial=0.0, op0=AL.mult, op1=AL.add)   # gfull hi
    nc.sync.dma_start(    # g-hi -> walk slots
        out=_manual_ap(slots[64:68, 0:1], 0, [[HIW, 32], [1, HIW]]),
        in_=_manual_ap(megaRE[:, SB_O + 1 + LOW:SB_O + 2 + LOW], 0,
                       [[1, HIW]]))
    for s in range(4):    # lo extra: g row 512
        nc.sync.dma_start(
            out=slots[s:s + 1, GX:GX + NB],
            in_=megaRE[32 * s:32 * s + 1, SB_O + 1 + LOW:SB_O + 1 + LOW + NB])

    # -------- chunks for the hi walk: choice-lo + g-lo/L scans + u2 --------
    for b in range(16):
        for f in choice(b, 1):
            chunks.append(f)
        chunks.append(lambda b=b: v.tensor_tensor_scan(   # g scan, block b
            out=megaRE[:, SB_O + 1 + b * PITCH:SB_O + 1 + b * PITCH + PITCH],
            data0=megaRE[:, SA_O + 1 + b * PITCH:SA_O + 1 + b * PITCH + PITCH],
            data1=megaRE[:, SB_O + 1 + b * PITCH:SB_O + 1 + b * PITCH + PITCH],
            initial=0.0, op0=AL.mult, op1=AL.add))
        chunks.append(lambda b=b: v.tensor_tensor_scan(   # L scan, block b
            out=megaRE[:, SC_O + 1 + b * PITCH:SC_O + 1 + b * PITCH + PITCH],
            data0=megaRE[:, SA_O + 1 + b * PITCH:SA_O + 1 + b * PITCH + PITCH],
            data1=megaRE[:, SC_O + 1 + b * PITCH:SC_O + 1 + b * PITCH + PITCH],
            initial=0.0, op0=AL.mult, op1=AL.add))
    chunks.append(lambda: nc.sync.dma_start(              # g-lo -> walk slots
        out=_manual_ap(slots[0:4, 0:1], 0, [[LOW, 32], [1, LOW]]),
        in_=_manual_ap(megaRE[:, SB_O + 1:SB_O + 2], 0, [[1, LOW]])))
    for b in range(16, NBLK):                             # L scans, hi blocks
        chunks.append(lambda b=b: v.tensor_tensor_scan(
            out=megaRE[:, SC_O + 1 + b * PITCH:SC_O + 1 + b * PITCH + PITCH],
            data0=megaRE[:, SA_O + 1 + b * PITCH:SA_O + 1 + b * PITCH + PITCH],
            data1=megaRE[:, SC_O + 1 + b * PITCH:SC_O + 1 + b * PITCH + PITCH],
            initial=0.0, op0=AL.mult, op1=AL.add))

    # ---------------- walk hi ----------------
    v.memset(xhh[64:68, 512:513], 21.0)                  # x_1023
    v.memset(xhh[64:68, 513:544], 0.0)                   # junk rows > 1023
    v.memset(xhl[0:4, 0:1], 0.0)                         # x_{-1} junk
    for i in range(1023, 511, -1):
        gc = _gslot_hi(i + 1)
        v.scalar_tensor_tensor(
            out=wks[64:68, 41:41 + NB], in0=wks[64:68, 0:NB],
            scalar=xhh[64:68, i - 511:i - 510],
            in1=slots[64:68, gc:gc + NB],
            op0=AL.is_equal, op1=AL.mult,
            accum_out=xhh[64:68, i - 512:i - 511])
        drain()
    drain(len(chunks))

    nc.sync.dma_start(out=xhl[0:4, 512:513], in_=xhh[64:68, 0:1])   # handoff
    nc.vector.tensor_scalar_add(                         # xcol repack hi
        out=_manual_ap(megaRE[64:68, SCX2_O:SCX2_O + 1], 0, [[17, 32], [1, 17]]),
        in0=_manual_ap(xhh[64:68, 0:1], 0, [[1, 32], [32, 17]]), scalar1=0.0)
    nc.sync.dma_start(
        out=_manual_ap(megaRE[:, XC_O + 16:XC_O + 17], 0, [[1, 17]]),
        in_=_manual_ap(megaRE[64:68, SCX2_O:SCX2_O + 1], 0, [[17, 32], [1, 17]]))

    # -------- chunks for the lo walk: hi-half olo/mask/products ------------
    b32 = SA_O + 1 + 32 * PITCH

    def metrics_blk(b):
        sa1 = rcells(SA_O, b, 1)
        yield lambda: v.tensor_tensor(out=sa1, in0=rocol(1),
                                      in1=rsmb(XC_O, b, 1), op=AL.is_equal)
        yield lambda: v.tensor_tensor(out=sa1, in0=sa1,
                                      in1=rcells(SC_O, b, 1), op=AL.mult)
        yield lambda: v.tensor_reduce(out=megaRE[:, OLO_O + b:OLO_O + b + 1],
                                      in_=rcells(SA_O, b, 1),
                                      axis=mybir.AxisListType.X, op=AL.add)
        if b == 32:
            yield lambda: v.memset(megaRE[:, b32:b32 + NB], 0.0)
            for s in range(4):                           # row 1023 mask
                q = 32 * s

                def rebuild(q=q):
                    ic0 = megaRE[q:q + 1, CONST_O + 1:CONST_O + 1 + NB]
                    wt = wks[q:q + 1, 41:41 + NB]
                    v.scalar_tensor_tensor(
                        out=wt, in0=ic0,
                        scalar=megaRE[q:q + 1, OLO_O + 32:OLO_O + 33],
                        in1=ic0, op0=AL.is_ge, op1=AL.bypass)
                    v.scalar_tensor_tensor(
                        out=megaRE[q:q + 1, b32:b32 + NB], in0=ic0,
                        scalar=megaRE[q:q + 1, XC_O + 32:XC_O + 33],
                        in1=wt, op0=AL.is_le, op1=AL.mult)
                yield rebuild
        else:
            yield lambda: v.tensor_tensor(out=sa1, in0=rocol(1),
                                          in1=rsmb(OLO_O, b, 1), op=AL.is_ge)
            yield lambda: v.tensor_tensor(out=rcells(SB_O, b, 1),
                                          in0=rocol(1), in1=rsmb(XC_O, b, 1),
                                          op=AL.is_le)
            yield lambda: v.tensor_tensor(out=sa1, in0=sa1,
                                          in1=rcells(SB_O, b, 1), op=AL.mult)
        for off in (R1_O, R2_O, U_O):                    # products, in place
            yield lambda off=off: v.tensor_tensor(
                out=rcells(off, b, 1), in0=rcells(off, b, 1), in1=sa1,
                op=AL.mult)

    for b in range(16, NBLK):
        for f in metrics_blk(b):
            chunks.append(f)

    # ---------------- walk lo ----------------
    for i in range(511, 0, -1):
        gc = GX if i == 511 else _gslot_lo(i + 1)
        v.scalar_tensor_tensor(
            out=wks[0:4, 41:41 + NB], in0=wks[0:4, 0:NB],
            scalar=xhl[0:4, i + 1:i + 2],
            in1=slots[0:4, gc:gc + NB],
            op0=AL.is_equal, op1=AL.mult,
            accum_out=xhl[0:4, i:i + 1])
        if i < 490:
            drain()
    drain(len(chunks))

    # ---------------- tail: lo-half olo/mask/products + reduces ------------
    nc.vector.tensor_scalar_add(                         # xcol repack lo
        out=_manual_ap(megaRE[0:4, SCX_O:SCX_O + 1], 0, [[16, 32], [1, 16]]),
        in0=_manual_ap(xhl[0:4, 0:1], 0, [[1, 32], [32, 16]]), scalar1=0.0)
    nc.sync.dma_start(
        out=_manual_ap(megaRE[:, XC_O:XC_O + 1], 0, [[1, 16]]),
        in_=_manual_ap(megaRE[0:4, SCX_O:SCX_O + 1], 0, [[16, 32], [1, 16]]))
    salo = rcells(SA_O, 0, 16)
    v.tensor_tensor(out=salo, in0=rocol(16), in1=rsmb(XC_O, 0, 16),
                    op=AL.is_equal)
    v.tensor_tensor(out=salo, in0=salo, in1=rcells(SC_O, 0, 16), op=AL.mult)
    v.tensor_reduce(out=megaRE[:, OLO_O:OLO_O + 16], in_=salo,
                    axis=mybir.AxisListType.X, op=AL.add)
    v.tensor_tensor(out=salo, in0=rocol(16), in1=rsmb(OLO_O, 0, 16),
                    op=AL.is_ge)
    v.tensor_tensor(out=rcells(SB_O, 0, 16), in0=rocol(16),
                    in1=rsmb(XC_O, 0, 16), op=AL.is_le)
    v.tensor_tensor(out=salo, in0=salo, in1=rcells(SB_O, 0, 16), op=AL.mult)
    for s in range(4):                                   # virtual row 0
        v.memset(megaRE[32 * s:32 * s + 1, SA_O + 1:SA_O + 1 + NB], 0.0)
    for off in (R1_O, R2_O, U_O):
        v.tensor_tensor(out=rcells(off, 0, 16), in0=rcells(off, 0, 16),
                        in1=salo, op=AL.mult)
    for j, off in enumerate((R1_O, R2_O, U_O, SA_O)):
        v.tensor_reduce(out=megaRE[:, RED_O + j:RED_O + j + 1],
                        in_=rcells(off, 0, NBLK),
                        axis=mybir.AxisListType.XY, op=AL.add)

    nc.sync.dma_start(out=partials[:], in_=megaRE[:, RED_O:RED_O + 4])


def _get_module():
    if "nc" not in _CACHE:
        _CACHE["nc"] = _build_module()
    return _CACHE["nc"]


def _make_inmaps(preds, targs):
    preds = np.ascontiguousarray(preds, dtype=np.float32)
    targs = np.ascontiguousarray(targs, dtype=np.float32)
    cst = np.tile(np.arange(44, dtype=np.float32), (128, 1))
    pp = np.arange(32)
    bb = np.arange(NBLK)
    r_idx = pp[:, None] + 32 * bb[None, :]              # [32, 33] = i + 1
    r_ok = (r_idx >= 1) & (r_idx <= N)
    r_cl = np.clip(r_idx - 1, 0, N - 1)
    uu = np.arange(SKW)
    t_idx = uu[None, :] + pp[:, None] - 21              # [32, SKW]
    t_ok = (t_idx >= 0) & (t_idx < N)
    t_cl = np.clip(t_idx, 0, N - 1)
    in_maps = []
    for c in range(NCORES):
        ps = preds[c * BC:(c + 1) * BC]
        ts = targs[c * BC:(c + 1) * BC]
        prev = np.zeros((BC, 32, 3 * NBLK), dtype=np.float32)
        tskv = np.zeros((BC, 32, 3 * SKW), dtype=np.float32)
        for k in range(3):
            vv = ps[:, :, k][:, r_cl]                   # [BC, 32, NBLK]
            prev[:, :, k * NBLK:(k + 1) * NBLK] = np.where(r_ok[None], vv, 0.0)
            fill = FILL if k < 2 else 0.0
            ww = ts[:, :, k][:, t_cl]                   # [BC, 32, SKW]
            tskv[:, :, k * SKW:(k + 1) * SKW] = np.where(t_ok[None], ww, fill)
        in_maps.append({"pre": prev.reshape(128, 3 * NBLK),
                        "tsk": tskv.reshape(128, 3 * SKW), "cst": cst})
    return in_maps


def _reduce_host(parts_list, subcoef):
    c0, c1 = float(subcoef[0]), float(subcoef[1])
    loss = 0.0
    for parts in parts_list:
        m = parts.reshape(BC, 32, 4).sum(axis=1)        # [BC, (Sx,Sy,Sb,cnt)]
        for s in range(BC):
            sx, sy, sb, cnt = (float(m[s, k]) for k in range(4))
            loss += c0 * sx + c1 * sy + 0.1 * sb / cnt
    return np.float32(loss)


def run(preds, targs, subcoef, trace=False):
    nc = _get_module()
    in_maps = _make_inmaps(preds, targs)
    res = run_bass_kernel_spmd(nc, in_maps, core_ids=list(range(NCORES)),
                               trace=trace)
    parts = [r["partials"] for r in res.results]
    return _reduce_host(parts, np.asarray(subcoef)), res


def kernel(preds, targs, subcoef):
    out, _ = run(preds, targs, subcoef)
    return out


# revision 19
# speedup vs baseline: 1.4437x; 1.0119x over previous
"""Banded DTW loss kernel for Trainium2 (Bass/Tile), 8-core data-parallel.

Layout: sample-major partitions q = 32*s + p (s = sample 0..3, p = row%32).

Phase A (forward DP) uses ONE fused 82-wide tensor_tensor_scan per row:
the row slot holds d interleaved with zeros (d[o] at odd col 2o+1), and
data0 is a 3-D overlapping-pair window over the previous row's slot
(odd cols hold D).  Scan semantics state = min(data0[t], state) + data1[t]
then give, per band cell o:
    t=2o   : m   = min(D'[o],   D[o-1]) + 0
    t=2o+1 : D[o]= min(D'[o+1], m     ) + d[o]
which is exactly the banded DTW recurrence.  The scan writes the slot in
place (d -> interleaved m/D).  Rows are distributed over 4 partition
quadrants (0/32/64/96) of 256 rows each, p-major slot order, so the
RE<->walk transposes are single rectangular DMAs per quadrant.

Band-invalid cells get cost ~2e15 via host-side target padding (1e15
fill outside [0,N)), standing in for the explicit +inf band mask.

Backtrack: choice bits + g/L tables built in RE layout (42-pitch cell
regions whose per-block pad column doubles as the band pad and as the
scan-state reset), two full-width scans, g copied to walk layout by two
rectangular DMAs, then two serial walk loops (one STT+accum per row).
Path masks + L1/BCE metrics reduce in RE; host sums the partials.
"""

import numpy as np

import concourse.bacc as bacc
import concourse.bass as bass
import concourse.mybir as mybir
import concourse.tile as tile
from concourse.bass_utils import run_bass_kernel_spmd

B, N = 32, 1024
NB = 41                   # band width
PITCH = 42                # RE cell-block pitch (41 cells + 1 pad)
SP = 84                   # phase-A slot pitch (interleaved, 82 + 2 pads)
NBLK = 33                 # RE blocks (r = i+1 in [0,1056))
SKW = 1066                # skewed targ row length
NCORES = 8
BC = B // NCORES
BIGP = 1e30               # pad / DP "infinity"
FILL = 1e15               # targ pad fill -> invalid-cell cost ~2e15

AL = mybir.AluOpType
DT = mybir.dt.float32

RW = 1388                 # 42-pitch region width (lead pad + 33*42 + spare)
RW84 = 2773               # 84-pitch region width (lead pad + 33*84)

# ---- megaRE column offsets ----
TSK_O = 0                         # 3 * 1066 skewed targs (x, y, z)
PRE_O = TSK_O + 3 * SKW           # 3 * 33 preds (x, y, z)
CONST_O = PRE_O + 99              # iota 0..43
R1_O = CONST_O + 44               # |dx| cells (42-pitch)
R2_O = R1_O + RW                  # |dy| cells
SA_O = R2_O + RW                  # scratch A (isleft / mask)
SB_O = SA_O + RW                  # scratch B (gval -> gfull)
SC_O = SB_O + RW                  # scratch C (Lval -> Lfull -> metric tmp)
DS_O = SC_O + RW                  # d staging, 84-pitch interleaved (no lead)
DRE_O = DS_O + NBLK * SP          # D cells, 84-pitch (lead pad)
DREP_O = DRE_O + RW84             # D prev-row cells, 84-pitch (lead pad)
XC_O = DREP_O + RW84              # xcol [33]
OLO_O = XC_O + 33
CLZ_O = OLO_O + 33
NG1_O = CLZ_O + 33
NG2_O = NG1_O + 33
SPZ_O = NG2_O + 33
SPN_O = SPZ_O + 33
QZ_O = SPN_O + 33
RED_O = QZ_O + 33                 # Sx, Sy, Sbce, cnt
SCX_O = RED_O + 4                 # xcol repack lo (512)
SCX2_O = SCX_O + 512              # xcol repack hi (544)
REW = SCX2_O + 544 + 4

# ---- slots tile ----
# Phase-A D slots: quadrant k on partitions 32k..32k+4 holds rows with
# r//32 in [8k, 8k+BK_k), BK = (8,8,8,9), p-major 84-pitch slots.
BKS = (8, 8, 8, 9)
BSCQ = tuple(bk * 32 * SP for bk in BKS)     # boundary scratch col per quad
# Walk g slots: lo (rows 0..511) on partitions 0:4, 16 blocks; hi (rows
# 512..1055) on 64:68, 17 blocks; 42-pitch p-major.
LOW = 16 * PITCH
HIW = 17 * PITCH
GX = 512 * PITCH          # lo extra slot for g row 512
SLOTW = BSCQ[3] + SP + 4

_CACHE = {}


def _manual_ap(base, extra_off, dims):
    ap0 = [list(base.ap[0])]
    return bass.AP(base.tensor, base.offset + extra_off,
                   ap0 + [list(d) for d in dims])


def _dslot(r):
    k = min(r // 256, 3)
    return k, ((r % 32) * BKS[k] + (r // 32 - 8 * k)) * SP


def _gslot_lo(r):
    return ((r % 32) * 16 + r // 32) * PITCH


def _gslot_hi(r):
    return ((r % 32) * 17 + (r // 32 - 16)) * PITCH


def _build_module():
    nc = bacc.Bacc("TRN2", target_bir_lowering=False, debug=False,
                   num_devices=NCORES, detect_race_conditions=False)
    pre = nc.dram_tensor("pre", [128, 99], DT, kind="ExternalInput")
    tsk = nc.dram_tensor("tsk", [128, 3 * SKW], DT, kind="ExternalInput")
    cst = nc.dram_tensor("cst", [128, 44], DT, kind="ExternalInput")
    partials = nc.dram_tensor("partials", [128, 4], DT, kind="ExternalOutput")

    with tile.TileContext(nc) as tc:
        with tc.tile_pool(name="main", bufs=1) as pool:
            megaRE = pool.tile([128, REW], DT)
            slots = pool.tile([128, SLOTW], DT)
            xhl = pool.tile([128, 520], DT)
            xhh = pool.tile([128, 560], DT)
            wks = pool.tile([128, 96], DT)
            _emit(nc, tc, megaRE, slots, xhl, xhh, wks,
                  pre, tsk, cst, partials)
    nc.compile()
    return nc


def _raw_scan(nc, out, data0, data1, initial, op0, op1):
    eng = nc.vector
    return eng.add_instruction(mybir.InstTensorScalarPtr(
        name=eng.bass.get_next_instruction_name(),
        is_tensor_tensor_scan=True,
        is_scalar_tensor_tensor=True,
        op0=op0, op1=op1,
        ins=[eng.lower_ap(data0), eng.lower_ap_or_imm(initial),
             eng.lower_ap(data1)],
        outs=[eng.lower_ap(out)]))


def _emit(nc, tc, megaRE, slots, xhl, xhh, wks, pre, tsk, cst, partials):
    v = nc.vector
    U_O = DS_O        # u2 (bce per-cell) region, 42-pitch, reuses d staging

    def rcells(off, b0, nb, dc=0):
        return _manual_ap(megaRE[:, off + 1 + PITCH * b0 + dc:
                                 off + 2 + PITCH * b0 + dc], 0,
                          [[PITCH, nb], [1, NB]])

    def rsmb(off, b0, nb):
        return megaRE[:, off + b0:off + b0 + nb].unsqueeze(2) \
            .broadcast_to([128, nb, NB])

    def rskw(k, b0, nb):
        return _manual_ap(megaRE[:, TSK_O + k * SKW + 32 * b0:
                                 TSK_O + k * SKW + 32 * b0 + 1], 0,
                          [[32, nb], [1, NB]])

    def rocol(nb, dc=0):
        return megaRE[:, CONST_O + 1 + dc:CONST_O + 1 + dc + NB] \
            .unsqueeze(1).broadcast_to([128, nb, NB])

    def rcells84(base_col, b0, nb):
        return _manual_ap(megaRE[:, base_col + SP * b0:base_col + SP * b0 + 1],
                          0, [[SP, nb], [2, NB]])

    # ---------------- input DMAs (d-build deps first) ----------------
    nc.sync.dma_start(out=megaRE[:, TSK_O:TSK_O + SKW], in_=tsk[:, 0:SKW])
    nc.sync.dma_start(out=megaRE[:, PRE_O:PRE_O + 99], in_=pre[:])
    nc.sync.dma_start(out=megaRE[:, TSK_O + SKW:TSK_O + 2 * SKW],
                      in_=tsk[:, SKW:2 * SKW])
    nc.sync.dma_start(out=megaRE[:, CONST_O:CONST_O + 44], in_=cst[:])
    nc.sync.dma_start(out=wks[:, 0:NB], in_=cst[:, 1:1 + NB])   # iota 1..41
    nc.sync.dma_start(out=megaRE[:, TSK_O + 2 * SKW:TSK_O + 3 * SKW],
                      in_=tsk[:, 2 * SKW:3 * SKW])

    # ---------------- bce scalars (Act engine; overlaps everything) --------
    v.tensor_scalar(out=megaRE[:, CLZ_O:CLZ_O + NBLK],
                    in0=megaRE[:, PRE_O + 66:PRE_O + 99],
                    scalar1=-4.0, scalar2=4.0, op0=AL.max, op1=AL.min)
    nc.scalar.activation(megaRE[:, NG1_O:NG1_O + NBLK],
                         megaRE[:, CLZ_O:CLZ_O + NBLK],
                         mybir.ActivationFunctionType.Exp)
    nc.scalar.activation(megaRE[:, NG2_O:NG2_O + NBLK],
                         megaRE[:, CLZ_O:CLZ_O + NBLK],
                         mybir.ActivationFunctionType.Exp, scale=-1.0)
    nc.scalar.activation(megaRE[:, SPZ_O:SPZ_O + NBLK],
                         megaRE[:, NG1_O:NG1_O + NBLK],
                         mybir.ActivationFunctionType.Ln, bias=1.0)
    nc.scalar.activation(megaRE[:, SPN_O:SPN_O + NBLK],
                         megaRE[:, NG2_O:NG2_O + NBLK],
                         mybir.ActivationFunctionType.Ln, bias=1.0)
    v.scalar_tensor_tensor(out=megaRE[:, QZ_O:QZ_O + NBLK],
                           in0=megaRE[:, SPN_O:SPN_O + NBLK], scalar=5.0,
                           in1=megaRE[:, SPZ_O:SPZ_O + NBLK],
                           op0=AL.mult, op1=AL.subtract)

    # scratch-region pads/leads: zeroed once (scan-state resets + uninit)
    for off in (SA_O, SB_O, SC_O):
        v.memset(_manual_ap(megaRE[:, off + 1 + NB:off + 2 + NB], 0,
                            [[PITCH, NBLK]]), 0.0)
        v.memset(megaRE[:, off:off + 1], 0.0)

    # ---------------- d build (84-pitch interleaved), quadrant 0 first -----
    def dbuild(b0, nb):
        w0 = DS_O + SP * b0
        v.memset(megaRE[:, w0:w0 + SP * nb], 0.0)
        v.memset(_manual_ap(megaRE[:, w0 + 82:w0 + 83], 0, [[SP, nb]]), BIGP)
        v.memset(_manual_ap(megaRE[:, w0 + 83:w0 + 84], 0, [[SP, nb]]), BIGP)
        v.tensor_tensor(out=rcells(SA_O, b0, nb), in0=rsmb(PRE_O, b0, nb),
                        in1=rskw(0, b0, nb), op=AL.subtract)
        v.scalar_tensor_tensor(out=rcells(R1_O, b0, nb),
                               in0=rcells(SA_O, b0, nb), scalar=-1.0,
                               in1=rcells(SA_O, b0, nb),
                               op0=AL.mult, op1=AL.max)
        v.tensor_tensor(out=rcells(SA_O, b0, nb), in0=rsmb(PRE_O + 33, b0, nb),
                        in1=rskw(1, b0, nb), op=AL.subtract)
        v.scalar_tensor_tensor(out=rcells(R2_O, b0, nb),
                               in0=rcells(SA_O, b0, nb), scalar=-1.0,
                               in1=rcells(SA_O, b0, nb),
                               op0=AL.mult, op1=AL.max)
        v.tensor_tensor(out=rcells84(DS_O + 1, b0, nb),
                        in0=rcells(R1_O, b0, nb), in1=rcells(R2_O, b0, nb),
                        op=AL.add)

    def stage(k):
        w = BKS[k] * SP
        nc.sync.dma_start(
            out=_manual_ap(slots[32 * k:32 * k + 4, 0:1], 0,
                           [[w, 32], [1, w]]),
            in_=_manual_ap(megaRE[:, DS_O + 8 * k * SP:DS_O + 8 * k * SP + 1],
                           0, [[1, w]]))

    dbuild(0, 8)
    stage(0)
    # virtual row r=0 (quadrant 0, slot 0): odd cols BIG, col 41 (o=20) = 0
    v.memset(slots[0:4, 0:SP], BIGP)
    v.memset(slots[0:4, 41:42], 0.0)
    dbuild(8, 25)
    for k in range(1, 4):
        stage(k)

    # ---------------- chunk machinery ----------------
    chunks = []

    def drain(n=1):
        for _ in range(n):
            if chunks:
                chunks.pop(0)()

    # phase-A chunks: u2 = sp + tz*qz per block (2 ops each)
    for b in range(NBLK):
        chunks.append(lambda b=b: v.tensor_tensor(
            out=rcells(U_O, b, 1), in0=rskw(2, b, 1), in1=rsmb(QZ_O, b, 1),
            op=AL.mult))
        chunks.append(lambda b=b: v.tensor_tensor(
            out=rcells(U_O, b, 1), in0=rcells(U_O, b, 1),
            in1=rsmb(SPZ_O, b, 1), op=AL.add))

    # ---------------- phase A: one fused scan per row ----------------
    def arow(q0, cur, prevbase):
        _raw_scan(nc, out=slots[q0:q0 + 4, cur:cur + 82],
                  data0=_manual_ap(slots[q0:q0 + 4, 0:1], prevbase + 1,
                                   [[2, NB], [2, 2]]),
                  data1=slots[q0:q0 + 4, cur:cur + 82],
                  initial=BIGP, op0=AL.min, op1=AL.add)

    v.memset(megaRE[:, DRE_O:DRE_O + 1], BIGP)           # lead pad
    v.memset(megaRE[:, DREP_O:DREP_O + 1 + SP], BIGP)    # row-0 prev junk

    def dshift(c0, cw):
        """DrePrev partition shift for DRE col range [c0, c0+cw)."""
        d0 = max(c0, 1 + SP)      # p=0 rows: dst col x <- src col x - 84
        for s in range(4):
            nc.sync.dma_start(
                out=megaRE[32 * s + 1:32 * s + 32, DREP_O + c0:DREP_O + c0 + cw],
                in_=megaRE[32 * s:32 * s + 31, DRE_O + c0:DRE_O + c0 + cw])
            nc.sync.dma_start(
                out=megaRE[32 * s:32 * s + 1,
                           DREP_O + d0:DREP_O + c0 + cw],
                in_=megaRE[32 * s + 31:32 * s + 32,
                           DRE_O + d0 - SP:DRE_O + c0 + cw - SP])

    for k in range(4):
        q0 = 32 * k
        r0 = max(1, 256 * k)
        r1 = N if k == 3 else 256 * k + 255
        if k > 0:
            arow(q0, _dslot(r0)[1], BSCQ[k])
            r0 += 1
        for r in range(r0, r1 + 1):
            arow(q0, _dslot(r)[1], _dslot(r - 1)[1])
            if r % 4 == 0 and r > 64:
                drain()
        if k < 3:                                        # boundary to k+1
            _, pc = _dslot(r1)
            nc.sync.dma_start(
                out=slots[q0 + 32:q0 + 36, BSCQ[k + 1]:BSCQ[k + 1] + SP],
                in_=slots[q0:q0 + 4, pc:pc + SP])
        # transpose this quadrant's D out (overlaps next quadrant's DP)
        w = BKS[k] * SP
        nc.sync.dma_start(
            out=_manual_ap(megaRE[:, DRE_O + 1 + 8 * k * SP:
                                   DRE_O + 2 + 8 * k * SP], 0, [[1, w]]),
            in_=_manual_ap(slots[q0:q0 + 4, 0:1], 0, [[w, 32], [1, w]]))
        if k == 1:      # DrePrev shifts, blocks 0..15 (needs DRE b 0..15)
            dshift(0, 1 + 16 * SP)
        if k == 2:      # DrePrev shifts, blocks 16..23
            dshift(1 + 16 * SP, 8 * SP)
        if k == 3:      # DrePrev shifts, blocks 24..32
            dshift(1 + 24 * SP, 9 * SP)
    drain(len(chunks))

    # ---------------- choice bits (min-trick, 8 ops per range) -------------
    def choice(b0, nb):
        diag = rcells84(DREP_O + 2, b0, nb)
        up = rcells84(DREP_O + 4, b0, nb)
        left = rcells84(DRE_O, b0, nb)
        sa, sb, sc = (rcells(o, b0, nb) for o in (SA_O, SB_O, SC_O))
        yield lambda: v.tensor_tensor(out=sb, in0=diag, in1=up, op=AL.min)
        yield lambda: v.tensor_tensor(out=sa, in0=left, in1=sb, op=AL.is_lt)
        yield lambda: v.tensor_tensor(out=sb, in0=up, in1=left, op=AL.min)
        yield lambda: v.tensor_tensor(out=sb, in0=diag, in1=sb, op=AL.is_le)
        yield lambda: v.tensor_single_scalar(out=sc, in_=sa, scalar=0.0,
                                             op=AL.is_equal)    # notleft
        yield lambda: v.scalar_tensor_tensor(out=sb, in0=sb, scalar=-1.0,
                                             in1=rocol(nb, 1), op0=AL.mult,
                                             op1=AL.add)        # oc+2-isdiag
        yield lambda: v.tensor_tensor(out=sb, in0=sb, in1=sc, op=AL.mult)
        yield lambda: v.tensor_tensor(out=sc, in0=rocol(nb), in1=sc,
                                      op=AL.mult)               # Lval

    def gscan(off, b0, nb):
        return lambda: v.tensor_tensor_scan(
            out=megaRE[:, off + 1 + b0 * PITCH:off + 1 + (b0 + nb) * PITCH],
            data0=megaRE[:, SA_O + 1 + b0 * PITCH:SA_O + 1 + (b0 + nb) * PITCH],
            data1=megaRE[:, off + 1 + b0 * PITCH:off + 1 + (b0 + nb) * PITCH],
            initial=0.0, op0=AL.mult, op1=AL.add)

    for f in choice(24, 9):                              # top blocks: critical
        f()
    gscan(SB_O, 24, 9)()                                 # g, blocks 24..32
    nc.sync.dma_start(    # g top -> walk slots (steps i >= 767)
        out=_manual_ap(slots[64:68, 336:337], 0, [[HIW, 32], [1, 378]]),
        in_=_manual_ap(megaRE[:, SB_O + 1009:SB_O + 1010], 0, [[1, 378]]))

    # -------- chunks for the hi walk: rest of choice/g/L + g DMAs ----------
    for b in range(16, 24):
        for f in choice(b, 1):
            chunks.append(f)
        chunks.append(gscan(SB_O, b, 1))
        chunks.append(gscan(SC_O, b, 1))                  # L scan
    chunks.append(lambda: nc.sync.dma_start(              # g mid -> walk slots
        out=_manual_ap(slots[64:68, 0:1], 0, [[HIW, 32], [1, 336]]),
        in_=_manual_ap(megaRE[:, SB_O + 1 + LOW:SB_O + 2 + LOW], 0,
                       [[1, 336]])))

    def gx_dmas():
        for s in range(4):    # lo extra: g row 512
            nc.sync.dma_start(
                out=slots[s:s + 1, GX:GX + NB],
                in_=megaRE[32 * s:32 * s + 1,
                           SB_O + 1 + LOW:SB_O + 1 + LOW + NB])
    chunks.append(gx_dmas)
    for b in range(16):
        for f in choice(b, 1):
            chunks.append(f)
        chunks.append(gscan(SB_O, b, 1))
        chunks.append(gscan(SC_O, b, 1))
    chunks.append(lambda: nc.sync.dma_start(              # g-lo -> walk slots
        out=_manual_ap(slots[0:4, 0:1], 0, [[LOW, 32], [1, LOW]]),
        in_=_manual_ap(megaRE[:, SB_O + 1:SB_O + 2], 0, [[1, LOW]])))
    for b in range(24, NBLK):                             # L scans, top blocks
        chunks.append(gscan(SC_O, b, 1))

    # ---------------- walk hi ----------------
    v.memset(xhh[64:68, 512:513], 21.0)                  # x_1023
    v.memset(xhh[64:68, 513:544], 0.0)                   # junk rows > 1023
    v.memset(xhl[0:4, 0:1], 0.0)                         # x_{-1} junk
    for i in range(1023, 511, -1):
        gc = _gslot_hi(i + 1)
        v.scalar_tensor_tensor(
            out=wks[64:68, 41:41 + NB], in0=wks[64:68, 0:NB],
            scalar=xhh[64:68, i - 511:i - 510],
            in1=slots[64:68, gc:gc + NB],
            op0=AL.is_equal, op1=AL.mult,
            accum_out=xhh[64:68, i - 512:i - 511])
        drain()
    drain(len(chunks))

    nc.sync.dma_start(out=xhl[0:4, 512:513], in_=xhh[64:68, 0:1])   # handoff
    nc.vector.tensor_scalar_add(                         # xcol repack hi
        out=_manual_ap(megaRE[64:68, SCX2_O:SCX2_O + 1], 0, [[17, 32], [1, 17]]),
        in0=_manual_ap(xhh[64:68, 0:1], 0, [[1, 32], [32, 17]]), scalar1=0.0)
    nc.sync.dma_start(
        out=_manual_ap(megaRE[:, XC_O + 16:XC_O + 17], 0, [[1, 17]]),
        in_=_manual_ap(megaRE[64:68, SCX2_O:SCX2_O + 1], 0, [[17, 32], [1, 17]]))

    # -------- chunks for the lo walk: hi-half olo/mask/products ------------
    b32 = SA_O + 1 + 32 * PITCH

    def metrics_blk(b):
        sa1 = rcells(SA_O, b, 1)
        yield lambda: v.tensor_tensor(out=sa1, in0=rocol(1),
                                      in1=rsmb(XC_O, b, 1), op=AL.is_equal)
        yield lambda: v.tensor_tensor(out=sa1, in0=sa1,
                                      in1=rcells(SC_O, b, 1), op=AL.mult)
        yield lambda: v.tensor_reduce(out=megaRE[:, OLO_O + b:OLO_O + b + 1],
                                      in_=rcells(SA_O, b, 1),
                                      axis=mybir.AxisListType.X, op=AL.add)
        if b == 32:
            yield lambda: v.memset(megaRE[:, b32:b32 + NB], 0.0)
            for s in range(4):                           # row 1023 mask
                q = 32 * s

                def rebuild(q=q):
                    ic0 = megaRE[q:q + 1, CONST_O + 1:CONST_O + 1 + NB]
                    wt = wks[q:q + 1, 41:41 + NB]
                    v.scalar_tensor_tensor(
                        out=wt, in0=ic0,
                        scalar=megaRE[q:q + 1, OLO_O + 32:OLO_O + 33],
                        in1=ic0, op0=AL.is_ge, op1=AL.bypass)
                    v.scalar_tensor_tensor(
                        out=megaRE[q:q + 1, b32:b32 + NB], in0=ic0,
                        scalar=megaRE[q:q + 1, XC_O + 32:XC_O + 33],
                        in1=wt, op0=AL.is_le, op1=AL.mult)
                yield rebuild
        else:
            yield lambda: v.tensor_tensor(out=sa1, in0=rocol(1),
                                          in1=rsmb(OLO_O, b, 1), op=AL.is_ge)
            yield lambda: v.tensor_tensor(out=rcells(SB_O, b, 1),
                                          in0=rocol(1), in1=rsmb(XC_O, b, 1),
                                          op=AL.is_le)
            yield lambda: v.tensor_tensor(out=sa1, in0=sa1,
                                          in1=rcells(SB_O, b, 1), op=AL.mult)
        for off in (R1_O, R2_O, U_O):                    # products, in place
            yield lambda off=off: v.tensor_tensor(
                out=rcells(off, b, 1), in0=rcells(off, b, 1), in1=sa1,
                op=AL.mult)

    for b in range(16, NBLK):
        for f in metrics_blk(b):
            chunks.append(f)

    # ---------------- walk lo ----------------
    chunks2 = []                # mid-walk group: blocks 8..15 metrics
    for b in range(8, 16):
        for f in metrics_blk(b):
            chunks2.append(f)
    for i in range(511, 0, -1):
        gc = GX if i == 511 else _gslot_lo(i + 1)
        v.scalar_tensor_tensor(
            out=wks[0:4, 41:41 + NB], in0=wks[0:4, 0:NB],
            scalar=xhl[0:4, i + 1:i + 2],
            in1=slots[0:4, gc:gc + NB],
            op0=AL.is_equal, op1=AL.mult,
            accum_out=xhl[0:4, i:i + 1])
        if i == 255:            # xcol for rows 256..511 is final
            nc.vector.tensor_scalar_add(
                out=_manual_ap(megaRE[0:4, SCX_O:SCX_O + 1], 0,
                               [[8, 32], [1, 8]]),
                in0=_manual_ap(xhl[0:4, 256:257], 0, [[1, 32], [32, 8]]),
                scalar1=0.0)
            nc.sync.dma_start(
                out=_manual_ap(megaRE[:, XC_O + 8:XC_O + 9], 0, [[1, 8]]),
                in_=_manual_ap(megaRE[0:4, SCX_O:SCX_O + 1], 0,
                               [[8, 32], [1, 8]]))
        if 225 < i < 490:
            drain()
        elif i <= 225 and chunks2:
            chunks2.pop(0)()
    drain(len(chunks))
    for f in chunks2:
        f()

    # ---------------- tail: blocks 0..7 olo/mask/products + reduces --------
    nc.vector.tensor_scalar_add(                         # xcol repack, b 0..7
        out=_manual_ap(megaRE[0:4, SCX_O:SCX_O + 1], 0, [[8, 32], [1, 8]]),
        in0=_manual_ap(xhl[0:4, 0:1], 0, [[1, 32], [32, 8]]), scalar1=0.0)
    nc.sync.dma_start(
        out=_manual_ap(megaRE[:, XC_O:XC_O + 1], 0, [[1, 8]]),
        in_=_manual_ap(megaRE[0:4, SCX_O:SCX_O + 1], 0, [[8, 32], [1, 8]]))
    salo = rcells(SA_O, 0, 8)
    v.tensor_tensor(out=salo, in0=rocol(8), in1=rsmb(XC_O, 0, 8),
                    op=AL.is_equal)
    v.tensor_tensor(out=salo, in0=salo, in1=rcells(SC_O, 0, 8), op=AL.mult)
    v.tensor_reduce(out=megaRE[:, OLO_O:OLO_O + 8], in_=salo,
                    axis=mybir.AxisListType.X, op=AL.add)
    v.tensor_tensor(out=salo, in0=rocol(8), in1=rsmb(OLO_O, 0, 8),
                    op=AL.is_ge)
    v.tensor_tensor(out=rcells(SB_O, 0, 8), in0=rocol(8),
                    in1=rsmb(XC_O, 0, 8), op=AL.is_le)
    v.tensor_tensor(out=salo, in0=salo, in1=rcells(SB_O, 0, 8), op=AL.mult)
    for s in range(4):                                   # virtual row 0
        v.memset(megaRE[32 * s:32 * s + 1, SA_O + 1:SA_O + 1 + NB], 0.0)
    for off in (R1_O, R2_O, U_O):
        v.tensor_tensor(out=rcells(off, 0, 8), in0=rcells(off, 0, 8),
                        in1=salo, op=AL.mult)
    for j, off in enumerate((R1_O, R2_O, U_O, SA_O)):
        v.tensor_reduce(out=megaRE[:, RED_O + j:RED_O + j + 1],
                        in_=rcells(off, 0, NBLK),
                        axis=mybir.AxisListType.XY, op=AL.add)

    nc.sync.dma_start(out=partials[:], in_=megaRE[:, RED_O:RED_O + 4])


def _get_module():
    if "nc" not in _CACHE:
        _CACHE["nc"] = _build_module()
    return _CACHE["nc"]


def _make_inmaps(preds, targs):
    preds = np.ascontiguousarray(preds, dtype=np.float32)
    targs = np.ascontiguousarray(targs, dtype=np.float32)
    cst = np.tile(np.arange(44, dtype=np.float32), (128, 1))
    pp = np.arange(32)
    bb = np.arange(NBLK)
    r_idx = pp[:, None] + 32 * bb[None, :]              # [32, 33] = i + 1
    r_ok = (r_idx >= 1) & (r_idx <= N)
    r_cl = np.clip(r_idx - 1, 0, N - 1)
    uu = np.arange(SKW)
    t_idx = uu[None, :] + pp[:, None] - 21              # [32, SKW]
    t_ok = (t_idx >= 0) & (t_idx < N)
    t_cl = np.clip(t_idx, 0, N - 1)
    in_maps = []
    for c in range(NCORES):
        ps = preds[c * BC:(c + 1) * BC]
        ts = targs[c * BC:(c + 1) * BC]
        prev = np.zeros((BC, 32, 3 * NBLK), dtype=np.float32)
        tskv = np.zeros((BC, 32, 3 * SKW), dtype=np.float32)
        for k in range(3):
            vv = ps[:, :, k][:, r_cl]                   # [BC, 32, NBLK]
            prev[:, :, k * NBLK:(k + 1) * NBLK] = np.where(r_ok[None], vv, 0.0)
            fill = FILL if k < 2 else 0.0
            ww = ts[:, :, k][:, t_cl]                   # [BC, 32, SKW]
            tskv[:, :, k * SKW:(k + 1) * SKW] = np.where(t_ok[None], ww, fill)
        in_maps.append({"pre": prev.reshape(128, 3 * NBLK),
                        "tsk": tskv.reshape(128, 3 * SKW), "cst": cst})
    return in_maps


def _reduce_host(parts_list, subcoef):
    c0, c1 = float(subcoef[0]), float(subcoef[1])
    loss = 0.0
    for parts in parts_list:
        m = parts.reshape(BC, 32, 4).sum(axis=1)        # [BC, (Sx,Sy,Sb,cnt)]
        for s in range(BC):
            sx, sy, sb, cnt = (float(m[s, k]) for k in range(4))
            loss += c0 * sx + c1 * sy + 0.1 * sb / cnt
    return np.float32(loss)


def run(preds, targs, subcoef, trace=False):
    nc = _get_module()
    in_maps = _make_inmaps(preds, targs)
    res = run_bass_kernel_spmd(nc, in_maps, core_ids=list(range(NCORES)),
                               trace=trace)
    parts = [r["partials"] for r in res.results]
    return _reduce_host(parts, np.asarray(subcoef)), res


def kernel(preds, targs, subcoef):
    out, _ = run(preds, targs, subcoef)
    return out


# revision 24
# speedup vs baseline: 1.4519x; 1.0056x over previous
"""Banded DTW loss kernel for Trainium2 (Bass/Tile), 8-core data-parallel.

Layout: sample-major partitions q = 32*s + p (s = sample 0..3, p = row%32).

Phase A (forward DP) uses ONE fused 82-wide tensor_tensor_scan per row:
the row slot holds d interleaved with zeros (d[o] at odd col 2o+1), and
data0 is a 3-D overlapping-pair window over the previous row's slot
(odd cols hold D).  Scan semantics state = min(data0[t], state) + data1[t]
then give, per band cell o:
    t=2o   : m   = min(D'[o],   D[o-1]) + 0
    t=2o+1 : D[o]= min(D'[o+1], m     ) + d[o]
which is exactly the banded DTW recurrence.  The scan writes the slot in
place (d -> interleaved m/D).  Rows are distributed over 4 partition
quadrants (0/32/64/96) of 256 rows each, p-major slot order, so the
RE<->walk transposes are single rectangular DMAs per quadrant.

Band-invalid cells get cost ~2e15 via host-side target padding (1e15
fill outside [0,N)), standing in for the explicit +inf band mask.

Backtrack: choice bits + g/L tables built in RE layout (42-pitch cell
regions whose per-block pad column doubles as the band pad and as the
scan-state reset), two full-width scans, g copied to walk layout by two
rectangular DMAs, then two serial walk loops (one STT+accum per row).
Path masks + L1/BCE metrics reduce in RE; host sums the partials.
"""

import numpy as np

import concourse.bacc as bacc
import concourse.bass as bass
import concourse.mybir as mybir
import concourse.tile as tile
from concourse.bass_utils import run_bass_kernel_spmd

B, N = 32, 1024
NB = 41                   # band width
PITCH = 42                # RE cell-block pitch (41 cells + 1 pad)
SP = 84                   # phase-A slot pitch (interleaved, 82 + 2 pads)
NBLK = 33                 # RE blocks (r = i+1 in [0,1056))
SKW = 1066                # skewed targ row length
NCORES = 8
BC = B // NCORES
BIGP = 1e30               # pad / DP "infinity"
FILL = 1e15               # targ pad fill -> invalid-cell cost ~2e15

AL = mybir.AluOpType
DT = mybir.dt.float32

RW = 1388                 # 42-pitch region width (lead pad + 33*42 + spare)
RW84 = 2773               # 84-pitch region width (lead pad + 33*84)

# ---- megaRE column offsets ----
TSK_O = 0                         # 3 * 1066 skewed targs (x, y, z)
PRE_O = TSK_O + 3 * SKW           # 3 * 33 preds (x, y, z)
CONST_O = PRE_O + 99              # iota 0..43
R1_O = CONST_O + 44               # |dx| cells (42-pitch)
R2_O = R1_O + RW                  # |dy| cells
SA_O = R2_O + RW                  # scratch A (isleft / mask)
SB_O = SA_O + RW                  # scratch B (gval -> gfull)
SC_O = SB_O + RW                  # scratch C (Lval -> Lfull -> metric tmp)
DS_O = SC_O + RW                  # d staging, 84-pitch interleaved (no lead)
DRE_O = DS_O + NBLK * SP          # D cells, 84-pitch (lead pad)
DREP_O = DRE_O + RW84             # D prev-row cells, 84-pitch (lead pad)
XC_O = DREP_O + RW84              # xcol [33]
OLO_O = XC_O + 33
CLZ_O = OLO_O + 33
NG1_O = CLZ_O + 33
NG2_O = NG1_O + 33
SPZ_O = NG2_O + 33
SPN_O = SPZ_O + 33
QZ_O = SPN_O + 33
RED_O = QZ_O + 33                 # Sx, Sy, Sbce, cnt
SCX_O = RED_O + 4                 # xcol repack lo (512)
SCX2_O = SCX_O + 512              # xcol repack hi (544)
RSC_O = SCX2_O + 544              # per-block partial reduces (4 x 33)
REW = RSC_O + 132 + 4

# ---- slots tile ----
# Phase-A D slots: quadrant k on partitions 32k..32k+4 holds rows with
# r//32 in [8k, 8k+BK_k), BK = (8,8,8,9), p-major 84-pitch slots.
BKS = (8, 8, 8, 9)
BSCQ = tuple(bk * 32 * SP for bk in BKS)     # boundary scratch col per quad
# Walk g slots: lo (rows 0..511) on partitions 0:4, 16 blocks; hi (rows
# 512..1055) on 64:68, 17 blocks; 42-pitch p-major.
LOW = 16 * PITCH
HIW = 17 * PITCH
GX = 512 * PITCH          # lo extra slot for g row 512
SLOTW = BSCQ[3] + SP + 4

_CACHE = {}


def _manual_ap(base, extra_off, dims):
    ap0 = [list(base.ap[0])]
    return bass.AP(base.tensor, base.offset + extra_off,
                   ap0 + [list(d) for d in dims])


def _dslot(r):
    k = min(r // 256, 3)
    return k, ((r % 32) * BKS[k] + (r // 32 - 8 * k)) * SP


def _gslot_lo(r):
    return ((r % 32) * 16 + r // 32) * PITCH


def _gslot_hi(r):
    return ((r % 32) * 17 + (r // 32 - 16)) * PITCH


def _build_module():
    nc = bacc.Bacc("TRN2", target_bir_lowering=False, debug=False,
                   num_devices=NCORES, detect_race_conditions=False)
    pre = nc.dram_tensor("pre", [128, 99], DT, kind="ExternalInput")
    tsk = nc.dram_tensor("tsk", [128, 3 * SKW], DT, kind="ExternalInput")
    cst = nc.dram_tensor("cst", [128, 44], DT, kind="ExternalInput")
    partials = nc.dram_tensor("partials", [128, 4], DT, kind="ExternalOutput")

    with tile.TileContext(nc) as tc:
        with tc.tile_pool(name="main", bufs=1) as pool:
            megaRE = pool.tile([128, REW], DT)
            slots = pool.tile([128, SLOTW], DT)
            xhl = pool.tile([128, 520], DT)
            xhh = pool.tile([128, 560], DT)
            wks = pool.tile([128, 96], DT)
            _emit(nc, tc, megaRE, slots, xhl, xhh, wks,
                  pre, tsk, cst, partials)
    nc.compile()
    return nc


def _raw_scan(nc, out, data0, data1, initial, op0, op1):
    eng = nc.vector
    return eng.add_instruction(mybir.InstTensorScalarPtr(
        name=eng.bass.get_next_instruction_name(),
        is_tensor_tensor_scan=True,
        is_scalar_tensor_tensor=True,
        op0=op0, op1=op1,
        ins=[eng.lower_ap(data0), eng.lower_ap_or_imm(initial),
             eng.lower_ap(data1)],
        outs=[eng.lower_ap(out)]))


def _emit(nc, tc, megaRE, slots, xhl, xhh, wks, pre, tsk, cst, partials):
    v = nc.vector
    U_O = DS_O        # u2 (bce per-cell) region, 42-pitch, reuses d staging

    def rcells(off, b0, nb, dc=0):
        return _manual_ap(megaRE[:, off + 1 + PITCH * b0 + dc:
                                 off + 2 + PITCH * b0 + dc], 0,
                          [[PITCH, nb], [1, NB]])

    def rsmb(off, b0, nb):
        return megaRE[:, off + b0:off + b0 + nb].unsqueeze(2) \
            .broadcast_to([128, nb, NB])

    def rskw(k, b0, nb):
        return _manual_ap(megaRE[:, TSK_O + k * SKW + 32 * b0:
                                 TSK_O + k * SKW + 32 * b0 + 1], 0,
                          [[32, nb], [1, NB]])

    def rocol(nb, dc=0):
        return megaRE[:, CONST_O + 1 + dc:CONST_O + 1 + dc + NB] \
            .unsqueeze(1).broadcast_to([128, nb, NB])

    def rcells84(base_col, b0, nb):
        return _manual_ap(megaRE[:, base_col + SP * b0:base_col + SP * b0 + 1],
                          0, [[SP, nb], [2, NB]])

    # ---------------- input DMAs (d-build deps first) ----------------
    nc.sync.dma_start(out=megaRE[:, TSK_O:TSK_O + 300], in_=tsk[:, 0:300])
    nc.sync.dma_start(out=megaRE[:, PRE_O:PRE_O + 99], in_=pre[:])
    nc.sync.dma_start(out=megaRE[:, TSK_O + SKW:TSK_O + SKW + 300],
                      in_=tsk[:, SKW:SKW + 300])
    nc.sync.dma_start(out=megaRE[:, TSK_O + 300:TSK_O + SKW],
                      in_=tsk[:, 300:SKW])
    nc.sync.dma_start(out=megaRE[:, TSK_O + SKW + 300:TSK_O + 2 * SKW],
                      in_=tsk[:, SKW + 300:2 * SKW])
    nc.sync.dma_start(out=megaRE[:, CONST_O:CONST_O + 44], in_=cst[:])
    nc.sync.dma_start(out=wks[:, 0:NB], in_=cst[:, 1:1 + NB])   # iota 1..41
    nc.sync.dma_start(out=megaRE[:, TSK_O + 2 * SKW:TSK_O + 3 * SKW],
                      in_=tsk[:, 2 * SKW:3 * SKW])

    # ---------------- bce scalars (Act engine; overlaps everything) --------
    v.tensor_scalar(out=megaRE[:, CLZ_O:CLZ_O + NBLK],
                    in0=megaRE[:, PRE_O + 66:PRE_O + 99],
                    scalar1=-4.0, scalar2=4.0, op0=AL.max, op1=AL.min)
    nc.scalar.activation(megaRE[:, NG1_O:NG1_O + NBLK],
                         megaRE[:, CLZ_O:CLZ_O + NBLK],
                         mybir.ActivationFunctionType.Exp)
    nc.scalar.activation(megaRE[:, NG2_O:NG2_O + NBLK],
                         megaRE[:, CLZ_O:CLZ_O + NBLK],
                         mybir.ActivationFunctionType.Exp, scale=-1.0)
    nc.scalar.activation(megaRE[:, SPZ_O:SPZ_O + NBLK],
                         megaRE[:, NG1_O:NG1_O + NBLK],
                         mybir.ActivationFunctionType.Ln, bias=1.0)
    nc.scalar.activation(megaRE[:, SPN_O:SPN_O + NBLK],
                         megaRE[:, NG2_O:NG2_O + NBLK],
                         mybir.ActivationFunctionType.Ln, bias=1.0)
    v.scalar_tensor_tensor(out=megaRE[:, QZ_O:QZ_O + NBLK],
                           in0=megaRE[:, SPN_O:SPN_O + NBLK], scalar=5.0,
                           in1=megaRE[:, SPZ_O:SPZ_O + NBLK],
                           op0=AL.mult, op1=AL.subtract)

    # scratch-region pads/leads: zeroed once (scan-state resets + uninit)
    for off in (SA_O, SB_O, SC_O):
        v.memset(_manual_ap(megaRE[:, off + 1 + NB:off + 2 + NB], 0,
                            [[PITCH, NBLK]]), 0.0)
        v.memset(megaRE[:, off:off + 1], 0.0)

    # ---------------- d build (84-pitch interleaved), quadrant 0 first -----
    def dbuild(b0, nb):
        w0 = DS_O + SP * b0
        v.memset(megaRE[:, w0:w0 + SP * nb], 0.0)
        v.memset(_manual_ap(megaRE[:, w0 + 82:w0 + 83], 0, [[SP, nb]]), BIGP)
        v.memset(_manual_ap(megaRE[:, w0 + 83:w0 + 84], 0, [[SP, nb]]), BIGP)
        v.tensor_tensor(out=rcells(SA_O, b0, nb), in0=rsmb(PRE_O, b0, nb),
                        in1=rskw(0, b0, nb), op=AL.subtract)
        v.scalar_tensor_tensor(out=rcells(R1_O, b0, nb),
                               in0=rcells(SA_O, b0, nb), scalar=-1.0,
                               in1=rcells(SA_O, b0, nb),
                               op0=AL.mult, op1=AL.max)
        v.tensor_tensor(out=rcells(SA_O, b0, nb), in0=rsmb(PRE_O + 33, b0, nb),
                        in1=rskw(1, b0, nb), op=AL.subtract)
        v.scalar_tensor_tensor(out=rcells(R2_O, b0, nb),
                               in0=rcells(SA_O, b0, nb), scalar=-1.0,
                               in1=rcells(SA_O, b0, nb),
                               op0=AL.mult, op1=AL.max)
        v.tensor_tensor(out=rcells84(DS_O + 1, b0, nb),
                        in0=rcells(R1_O, b0, nb), in1=rcells(R2_O, b0, nb),
                        op=AL.add)

    def stage(k):
        w = BKS[k] * SP
        nc.sync.dma_start(
            out=_manual_ap(slots[32 * k:32 * k + 4, 0:1], 0,
                           [[w, 32], [1, w]]),
            in_=_manual_ap(megaRE[:, DS_O + 8 * k * SP:DS_O + 8 * k * SP + 1],
                           0, [[1, w]]))

    dbuild(0, 8)
    stage(0)
    # virtual row r=0 (quadrant 0, slot 0): odd cols BIG, col 41 (o=20) = 0
    v.memset(slots[0:4, 0:SP], BIGP)
    v.memset(slots[0:4, 41:42], 0.0)
    dbuild(8, 25)
    for k in range(1, 4):
        stage(k)

    # ---------------- chunk machinery ----------------
    chunks = []

    def drain(n=1):
        for _ in range(n):
            if chunks:
                chunks.pop(0)()

    # phase-A chunks: u2 = sp + tz*qz per block (2 ops each)
    for b in range(NBLK):
        chunks.append(lambda b=b: v.tensor_tensor(
            out=rcells(U_O, b, 1), in0=rskw(2, b, 1), in1=rsmb(QZ_O, b, 1),
            op=AL.mult))
        chunks.append(lambda b=b: v.tensor_tensor(
            out=rcells(U_O, b, 1), in0=rcells(U_O, b, 1),
            in1=rsmb(SPZ_O, b, 1), op=AL.add))

    # ---------------- phase A: one fused scan per row ----------------
    def arow(q0, cur, prevbase):
        _raw_scan(nc, out=slots[q0:q0 + 4, cur:cur + 82],
                  data0=_manual_ap(slots[q0:q0 + 4, 0:1], prevbase + 1,
                                   [[2, NB], [2, 2]]),
                  data1=slots[q0:q0 + 4, cur:cur + 82],
                  initial=BIGP, op0=AL.min, op1=AL.add)

    v.memset(megaRE[:, DRE_O:DRE_O + 1], BIGP)           # lead pad
    v.memset(megaRE[:, DREP_O:DREP_O + 1 + SP], BIGP)    # row-0 prev junk

    def dshift(c0, cw):
        """DrePrev partition shift for DRE col range [c0, c0+cw)."""
        d0 = max(c0, 1 + SP)      # p=0 rows: dst col x <- src col x - 84
        for s in range(4):
            nc.sync.dma_start(
                out=megaRE[32 * s + 1:32 * s + 32, DREP_O + c0:DREP_O + c0 + cw],
                in_=megaRE[32 * s:32 * s + 31, DRE_O + c0:DRE_O + c0 + cw])
            nc.sync.dma_start(
                out=megaRE[32 * s:32 * s + 1,
                           DREP_O + d0:DREP_O + c0 + cw],
                in_=megaRE[32 * s + 31:32 * s + 32,
                           DRE_O + d0 - SP:DRE_O + c0 + cw - SP])

    for k in range(4):
        q0 = 32 * k
        r0 = max(1, 256 * k)
        r1 = N if k == 3 else 256 * k + 255
        if k > 0:
            arow(q0, _dslot(r0)[1], BSCQ[k])
            r0 += 1
        for r in range(r0, r1 + 1):
            arow(q0, _dslot(r)[1], _dslot(r - 1)[1])
            if r % 4 == 0 and r > 64:
                drain()
        if k < 3:                                        # boundary to k+1
            _, pc = _dslot(r1)
            nc.sync.dma_start(
                out=slots[q0 + 32:q0 + 36, BSCQ[k + 1]:BSCQ[k + 1] + SP],
                in_=slots[q0:q0 + 4, pc:pc + SP])
        # transpose this quadrant's D out (overlaps next quadrant's DP)
        w = BKS[k] * SP
        nc.sync.dma_start(
            out=_manual_ap(megaRE[:, DRE_O + 1 + 8 * k * SP:
                                   DRE_O + 2 + 8 * k * SP], 0, [[1, w]]),
            in_=_manual_ap(slots[q0:q0 + 4, 0:1], 0, [[w, 32], [1, w]]))
        if k == 1:      # DrePrev shifts, blocks 0..15 (needs DRE b 0..15)
            dshift(0, 1 + 16 * SP)
        if k == 2:      # DrePrev shifts, blocks 16..23
            dshift(1 + 16 * SP, 8 * SP)
        if k == 3:      # DrePrev shifts, blocks 24..32
            dshift(1 + 24 * SP, 9 * SP)
    drain(len(chunks))

    # ---------------- choice bits (min-trick, 8 ops per range) -------------
    def choice(b0, nb):
        diag = rcells84(DREP_O + 2, b0, nb)
        up = rcells84(DREP_O + 4, b0, nb)
        left = rcells84(DRE_O, b0, nb)
        sa, sb, sc = (rcells(o, b0, nb) for o in (SA_O, SB_O, SC_O))
        yield lambda: v.tensor_tensor(out=sb, in0=diag, in1=up, op=AL.min)
        yield lambda: v.tensor_tensor(out=sa, in0=left, in1=sb, op=AL.is_lt)
        yield lambda: v.tensor_tensor(out=sb, in0=up, in1=left, op=AL.min)
        yield lambda: v.tensor_tensor(out=sb, in0=diag, in1=sb, op=AL.is_le)
        yield lambda: v.tensor_single_scalar(out=sc, in_=sa, scalar=0.0,
                                             op=AL.is_equal)    # notleft
        yield lambda: v.scalar_tensor_tensor(out=sb, in0=sb, scalar=-1.0,
                                             in1=rocol(nb, 1), op0=AL.mult,
                                             op1=AL.add)        # oc+2-isdiag
        yield lambda: v.tensor_tensor(out=sb, in0=sb, in1=sc, op=AL.mult)
        yield lambda: v.tensor_tensor(out=sc, in0=rocol(nb), in1=sc,
                                      op=AL.mult)               # Lval

    def gscan(off, b0, nb):
        return lambda: v.tensor_tensor_scan(
            out=megaRE[:, off + 1 + b0 * PITCH:off + 1 + (b0 + nb) * PITCH],
            data0=megaRE[:, SA_O + 1 + b0 * PITCH:SA_O + 1 + (b0 + nb) * PITCH],
            data1=megaRE[:, off + 1 + b0 * PITCH:off + 1 + (b0 + nb) * PITCH],
            initial=0.0, op0=AL.mult, op1=AL.add)

    for f in choice(29, 4):                              # top blocks: critical
        f()
    gscan(SB_O, 29, 4)()                                 # g, blocks 29..32
    nc.sync.dma_start(    # g top -> walk slots (steps i >= 927)
        out=_manual_ap(slots[64:68, 546:547], 0, [[HIW, 32], [1, 168]]),
        in_=_manual_ap(megaRE[:, SB_O + 1219:SB_O + 1220], 0, [[1, 168]]))
    for f in choice(24, 5):                              # next blocks
        f()
    gscan(SB_O, 24, 5)()                                 # g, blocks 24..28
    nc.sync.dma_start(    # g -> walk slots (steps i >= 767)
        out=_manual_ap(slots[64:68, 336:337], 0, [[HIW, 32], [1, 210]]),
        in_=_manual_ap(megaRE[:, SB_O + 1009:SB_O + 1010], 0, [[1, 210]]))

    # -------- chunks for the hi walk: rest of choice/g/L + g DMAs ----------
    for b in range(16, 24):
        for f in choice(b, 1):
            chunks.append(f)
        chunks.append(gscan(SB_O, b, 1))
        chunks.append(gscan(SC_O, b, 1))                  # L scan
    chunks.append(lambda: nc.sync.dma_start(              # g mid -> walk slots
        out=_manual_ap(slots[64:68, 0:1], 0, [[HIW, 32], [1, 336]]),
        in_=_manual_ap(megaRE[:, SB_O + 1 + LOW:SB_O + 2 + LOW], 0,
                       [[1, 336]])))

    def gx_dmas():
        for s in range(4):    # lo extra: g row 512
            nc.sync.dma_start(
                out=slots[s:s + 1, GX:GX + NB],
                in_=megaRE[32 * s:32 * s + 1,
                           SB_O + 1 + LOW:SB_O + 1 + LOW + NB])
    chunks.append(gx_dmas)
    for b in range(16):
        for f in choice(b, 1):
            chunks.append(f)
        chunks.append(gscan(SB_O, b, 1))
        chunks.append(gscan(SC_O, b, 1))
    chunks.append(lambda: nc.sync.dma_start(              # g-lo -> walk slots
        out=_manual_ap(slots[0:4, 0:1], 0, [[LOW, 32], [1, LOW]]),
        in_=_manual_ap(megaRE[:, SB_O + 1:SB_O + 2], 0, [[1, LOW]])))
    for b in range(24, NBLK):                             # L scans, top blocks
        chunks.append(gscan(SC_O, b, 1))

    # ---------------- walk hi ----------------
    v.memset(xhh[64:68, 512:513], 21.0)                  # x_1023
    v.memset(xhh[64:68, 513:544], 0.0)                   # junk rows > 1023
    v.memset(xhl[0:4, 0:1], 0.0)                         # x_{-1} junk
    for i in range(1023, 511, -1):
        gc = _gslot_hi(i + 1)
        v.scalar_tensor_tensor(
            out=wks[64:68, 41:41 + NB], in0=wks[64:68, 0:NB],
            scalar=xhh[64:68, i - 511:i - 510],
            in1=slots[64:68, gc:gc + NB],
            op0=AL.is_equal, op1=AL.mult,
            accum_out=xhh[64:68, i - 512:i - 511])
        drain()
    drain(len(chunks))

    nc.sync.dma_start(out=xhl[0:4, 512:513], in_=xhh[64:68, 0:1])   # handoff
    nc.vector.tensor_scalar_add(                         # xcol repack hi
        out=_manual_ap(megaRE[64:68, SCX2_O:SCX2_O + 1], 0, [[17, 32], [1, 17]]),
        in0=_manual_ap(xhh[64:68, 0:1], 0, [[1, 32], [32, 17]]), scalar1=0.0)
    nc.sync.dma_start(
        out=_manual_ap(megaRE[:, XC_O + 16:XC_O + 17], 0, [[1, 17]]),
        in_=_manual_ap(megaRE[64:68, SCX2_O:SCX2_O + 1], 0, [[17, 32], [1, 17]]))

    # -------- chunks for the lo walk: hi-half olo/mask/products ------------
    b32 = SA_O + 1 + 32 * PITCH

    def metrics_blk(b):
        sa1 = rcells(SA_O, b, 1)
        yield lambda: v.tensor_tensor(out=sa1, in0=rocol(1),
                                      in1=rsmb(XC_O, b, 1), op=AL.is_equal)
        yield lambda: v.tensor_tensor(out=sa1, in0=sa1,
                                      in1=rcells(SC_O, b, 1), op=AL.mult)
        yield lambda: v.tensor_reduce(out=megaRE[:, OLO_O + b:OLO_O + b + 1],
                                      in_=rcells(SA_O, b, 1),
                                      axis=mybir.AxisListType.X, op=AL.add)
        if b == 32:
            yield lambda: v.memset(megaRE[:, b32:b32 + NB], 0.0)
            for s in range(4):                           # row 1023 mask
                q = 32 * s

                def rebuild(q=q):
                    ic0 = megaRE[q:q + 1, CONST_O + 1:CONST_O + 1 + NB]
                    wt = wks[q:q + 1, 41:41 + NB]
                    v.scalar_tensor_tensor(
                        out=wt, in0=ic0,
                        scalar=megaRE[q:q + 1, OLO_O + 32:OLO_O + 33],
                        in1=ic0, op0=AL.is_ge, op1=AL.bypass)
                    v.scalar_tensor_tensor(
                        out=megaRE[q:q + 1, b32:b32 + NB], in0=ic0,
                        scalar=megaRE[q:q + 1, XC_O + 32:XC_O + 33],
                        in1=wt, op0=AL.is_le, op1=AL.mult)
                yield rebuild
        else:
            yield lambda: v.tensor_tensor(out=sa1, in0=rocol(1),
                                          in1=rsmb(OLO_O, b, 1), op=AL.is_ge)
            yield lambda: v.tensor_tensor(out=rcells(SB_O, b, 1),
                                          in0=rocol(1), in1=rsmb(XC_O, b, 1),
                                          op=AL.is_le)
            yield lambda: v.tensor_tensor(out=sa1, in0=sa1,
                                          in1=rcells(SB_O, b, 1), op=AL.mult)
        for off in (R1_O, R2_O, U_O):                    # products, in place
            yield lambda off=off: v.tensor_tensor(
                out=rcells(off, b, 1), in0=rcells(off, b, 1), in1=sa1,
                op=AL.mult)
        for j, off in enumerate((R1_O, R2_O, U_O, SA_O)):  # partial reduces
            yield lambda j=j, off=off: v.tensor_reduce(
                out=megaRE[:, RSC_O + 33 * j + b:RSC_O + 33 * j + b + 1],
                in_=rcells(off, b, 1), axis=mybir.AxisListType.X, op=AL.add)

    for b in range(16, NBLK):
        for f in metrics_blk(b):
            chunks.append(f)

    # ---------------- walk lo ----------------
    chunks2 = []                # mid-walk group: blocks 8..15 metrics
    for b in range(8, 16):
        for f in metrics_blk(b):
            chunks2.append(f)
    for i in range(511, 0, -1):
        gc = GX if i == 511 else _gslot_lo(i + 1)
        v.scalar_tensor_tensor(
            out=wks[0:4, 41:41 + NB], in0=wks[0:4, 0:NB],
            scalar=xhl[0:4, i + 1:i + 2],
            in1=slots[0:4, gc:gc + NB],
            op0=AL.is_equal, op1=AL.mult,
            accum_out=xhl[0:4, i:i + 1])
        if i == 255:            # xcol for rows 256..511 is final
            nc.vector.tensor_scalar_add(
                out=_manual_ap(megaRE[0:4, SCX_O:SCX_O + 1], 0,
                               [[8, 32], [1, 8]]),
                in0=_manual_ap(xhl[0:4, 256:257], 0, [[1, 32], [32, 8]]),
                scalar1=0.0)
            nc.sync.dma_start(
                out=_manual_ap(megaRE[:, XC_O + 8:XC_O + 9], 0, [[1, 8]]),
                in_=_manual_ap(megaRE[0:4, SCX_O:SCX_O + 1], 0,
                               [[8, 32], [1, 8]]))
        if 225 < i < 490:
            drain()
        elif i <= 225 and chunks2:
            chunks2.pop(0)()
    drain(len(chunks))
    for f in chunks2:
        f()

    # ---------------- tail: blocks 0..7 olo/mask/products + reduces --------
    nc.vector.tensor_scalar_add(                         # xcol repack, b 0..7
        out=_manual_ap(megaRE[0:4, SCX_O:SCX_O + 1], 0, [[8, 32], [1, 8]]),
        in0=_manual_ap(xhl[0:4, 0:1], 0, [[1, 32], [32, 8]]), scalar1=0.0)
    nc.sync.dma_start(
        out=_manual_ap(megaRE[:, XC_O:XC_O + 1], 0, [[1, 8]]),
        in_=_manual_ap(megaRE[0:4, SCX_O:SCX_O + 1], 0, [[8, 32], [1, 8]]))
    salo = rcells(SA_O, 0, 8)
    v.tensor_tensor(out=salo, in0=rocol(8), in1=rsmb(XC_O, 0, 8),
                    op=AL.is_equal)
    v.tensor_tensor(out=salo, in0=salo, in1=rcells(SC_O, 0, 8), op=AL.mult)
    v.tensor_reduce(out=megaRE[:, OLO_O:OLO_O + 8], in_=salo,
                    axis=mybir.AxisListType.X, op=AL.add)
    v.tensor_tensor(out=salo, in0=rocol(8), in1=rsmb(OLO_O, 0, 8),
                    op=AL.is_ge)
    v.tensor_tensor(out=rcells(SB_O, 0, 8), in0=rocol(8),
                    in1=rsmb(XC_O, 0, 8), op=AL.is_le)
    v.tensor_tensor(out=salo, in0=salo, in1=rcells(SB_O, 0, 8), op=AL.mult)
    for s in range(4):                                   # virtual row 0
        v.memset(megaRE[32 * s:32 * s + 1, SA_O + 1:SA_O + 1 + NB], 0.0)
    for off in (R1_O, R2_O, U_O):
        v.tensor_tensor(out=rcells(off, 0, 8), in0=rcells(off, 0, 8),
                        in1=salo, op=AL.mult)
    for j, off in enumerate((R1_O, R2_O, U_O, SA_O)):
        v.tensor_reduce(out=megaRE[:, RSC_O + 33 * j:RSC_O + 33 * j + 8],
                        in_=rcells(off, 0, 8),
                        axis=mybir.AxisListType.X, op=AL.add)
    for j in range(4):
        v.tensor_reduce(out=megaRE[:, RED_O + j:RED_O + j + 1],
                        in_=megaRE[:, RSC_O + 33 * j:RSC_O + 33 * j + NBLK],
                        axis=mybir.AxisListType.X, op=AL.add)

    nc.sync.dma_start(out=partials[:], in_=megaRE[:, RED_O:RED_O + 4])


def _get_module():
    if "nc" not in _CACHE:
        _CACHE["nc"] = _build_module()
    return _CACHE["nc"]


def _make_inmaps(preds, targs):
    preds = np.ascontiguousarray(preds, dtype=np.float32)
    targs = np.ascontiguousarray(targs, dtype=np.float32)
    cst = np.tile(np.arange(44, dtype=np.float32), (128, 1))
    pp = np.arange(32)
    bb = np.arange(NBLK)
    r_idx = pp[:, None] + 32 * bb[None, :]              # [32, 33] = i + 1
    r_ok = (r_idx >= 1) & (r_idx <= N)
    r_cl = np.clip(r_idx - 1, 0, N - 1)
    uu = np.arange(SKW)
    t_idx = uu[None, :] + pp[:, None] - 21              # [32, SKW]
    t_ok = (t_idx >= 0) & (t_idx < N)
    t_cl = np.clip(t_idx, 0, N - 1)
    in_maps = []
    for c in range(NCORES):
        ps = preds[c * BC:(c + 1) * BC]
        ts = targs[c * BC:(c + 1) * BC]
        prev = np.zeros((BC, 32, 3 * NBLK), dtype=np.float32)
        tskv = np.zeros((BC, 32, 3 * SKW), dtype=np.float32)
        for k in range(3):
            vv = ps[:, :, k][:, r_cl]                   # [BC, 32, NBLK]
            prev[:, :, k * NBLK:(k + 1) * NBLK] = np.where(r_ok[None], vv, 0.0)
            fill = FILL if k < 2 else 0.0
            ww = ts[:, :, k][:, t_cl]                   # [BC, 32, SKW]
            tskv[:, :, k * SKW:(k + 1) * SKW] = np.where(t_ok[None], ww, fill)
        in_maps.append({"pre": prev.reshape(128, 3 * NBLK),
                        "tsk": tskv.reshape(128, 3 * SKW), "cst": cst})
    return in_maps


def _reduce_host(parts_list, subcoef):
    c0, c1 = float(subcoef[0]), float(subcoef[1])
    loss = 0.0
    for parts in parts_list:
        m = parts.reshape(BC, 32, 4).sum(axis=1)        # [BC, (Sx,Sy,Sb,cnt)]
        for s in range(BC):
            sx, sy, sb, cnt = (float(m[s, k]) for k in range(4))
            loss += c0 * sx + c1 * sy + 0.1 * sb / cnt
    return np.float32(loss)


def run(preds, targs, subcoef, trace=False):
    nc = _get_module()
    in_maps = _make_inmaps(preds, targs)
    res = run_bass_kernel_spmd(nc, in_maps, core_ids=list(range(NCORES)),
                               trace=trace)
    parts = [r["partials"] for r in res.results]
    return _reduce_host(parts, np.asarray(subcoef)), res


def kernel(preds, targs, subcoef):
    out, _ = run(preds, targs, subcoef)
    return out


# revision 26
# speedup vs baseline: 1.4606x; 1.0060x over previous
"""Banded DTW loss kernel for Trainium2 (Bass/Tile), 8-core data-parallel.

Layout: sample-major partitions q = 32*s + p (s = sample 0..3, p = row%32).

Phase A (forward DP) uses ONE fused 82-wide tensor_tensor_scan per row:
the row slot holds d interleaved with zeros (d[o] at odd col 2o+1), and
data0 is a 3-D overlapping-pair window over the previous row's slot
(odd cols hold D).  Scan semantics state = min(data0[t], state) + data1[t]
then give, per band cell o:
    t=2o   : m   = min(D'[o],   D[o-1]) + 0
    t=2o+1 : D[o]= min(D'[o+1], m     ) + d[o]
which is exactly the banded DTW recurrence.  The scan writes the slot in
place (d -> interleaved m/D).  Rows are distributed over 4 partition
quadrants (0/32/64/96) of 256 rows each, p-major slot order, so the
RE<->walk transposes are single rectangular DMAs per quadrant.

Band-invalid cells get cost ~2e15 via host-side target padding (1e15
fill outside [0,N)), standing in for the explicit +inf band mask.

Backtrack: choice bits + g/L tables built in RE layout (42-pitch cell
regions whose per-block pad column doubles as the band pad and as the
scan-state reset), two full-width scans, g copied to walk layout by two
rectangular DMAs, then two serial walk loops (one STT+accum per row).
Path masks + L1/BCE metrics reduce in RE; host sums the partials.
"""

import numpy as np

import concourse.bacc as bacc
import concourse.bass as bass
import concourse.mybir as mybir
import concourse.tile as tile
from concourse.bass_utils import run_bass_kernel_spmd

B, N = 32, 1024
NB = 41                   # band width
PITCH = 42                # RE cell-block pitch (41 cells + 1 pad)
SP = 84                   # phase-A slot pitch (interleaved, 82 + 2 pads)
NBLK = 33                 # RE blocks (r = i+1 in [0,1056))
SKW = 1066                # skewed targ row length
NCORES = 8
BC = B // NCORES
BIGP = 1e30               # pad / DP "infinity"
FILL = 1e15               # targ pad fill -> invalid-cell cost ~2e15

AL = mybir.AluOpType
DT = mybir.dt.float32

RW = 1388                 # 42-pitch region width (lead pad + 33*42 + spare)
RW84 = 2773               # 84-pitch region width (lead pad + 33*84)

# ---- megaRE column offsets ----
TSK_O = 0                         # 3 * 1066 skewed targs (x, y, z)
PRE_O = TSK_O + 3 * SKW           # 3 * 33 preds (x, y, z)
CONST_O = PRE_O + 99              # iota 0..43
R1_O = CONST_O + 44               # |dx| cells (42-pitch)
R2_O = R1_O + RW                  # |dy| cells
SA_O = R2_O + RW                  # scratch A (isleft / mask)
SB_O = SA_O + RW                  # scratch B (gval -> gfull)
SC_O = SB_O + RW                  # scratch C (Lval -> Lfull -> metric tmp)
DS_O = SC_O + RW                  # d staging, 84-pitch interleaved (no lead)
DRE_O = DS_O + NBLK * SP          # D cells, 84-pitch (lead pad)
DREP_O = DRE_O + RW84             # D prev-row cells, 84-pitch (lead pad)
XC_O = DREP_O + RW84              # xcol [33]
OLO_O = XC_O + 33
CLZ_O = OLO_O + 33
NG1_O = CLZ_O + 33
NG2_O = NG1_O + 33
SPZ_O = NG2_O + 33
SPN_O = SPZ_O + 33
QZ_O = SPN_O + 33
RED_O = QZ_O + 33                 # Sx, Sy, Sbce, cnt
SCX_O = RED_O + 4                 # xcol repack lo (512)
SCX2_O = SCX_O + 512              # xcol repack hi (544)
RSC_O = SCX2_O + 544              # per-block partial reduces (4 x 33)
REW = RSC_O + 132 + 4

# ---- slots tile ----
# Phase-A D slots: quadrant k on partitions 32k..32k+4 holds rows with
# r//32 in [8k, 8k+BK_k), BK = (8,8,8,9), p-major 84-pitch slots.
BKS = (8, 8, 8, 9)
BSCQ = tuple(bk * 32 * SP for bk in BKS)     # boundary scratch col per quad
# Walk g slots: lo (rows 0..511) on partitions 0:4, 16 blocks; hi (rows
# 512..1055) on 64:68, 17 blocks; 42-pitch p-major.
LOW = 16 * PITCH
HIW = 17 * PITCH
GX = 512 * PITCH          # lo extra slot for g row 512
SLOTW = BSCQ[3] + SP + 4

_CACHE = {}


def _manual_ap(base, extra_off, dims):
    ap0 = [list(base.ap[0])]
    return bass.AP(base.tensor, base.offset + extra_off,
                   ap0 + [list(d) for d in dims])


def _dslot(r):
    k = min(r // 256, 3)
    return k, ((r % 32) * BKS[k] + (r // 32 - 8 * k)) * SP


def _gslot_lo(r):
    return ((r % 32) * 16 + r // 32) * PITCH


def _gslot_hi(r):
    return ((r % 32) * 17 + (r // 32 - 16)) * PITCH


def _build_module():
    nc = bacc.Bacc("TRN2", target_bir_lowering=False, debug=False,
                   num_devices=NCORES, detect_race_conditions=False)
    pre = nc.dram_tensor("pre", [128, 99], DT, kind="ExternalInput")
    tsk = nc.dram_tensor("tsk", [128, 3 * SKW], DT, kind="ExternalInput")
    cst = nc.dram_tensor("cst", [128, 44], DT, kind="ExternalInput")
    partials = nc.dram_tensor("partials", [128, 4], DT, kind="ExternalOutput")

    with tile.TileContext(nc) as tc:
        with tc.tile_pool(name="main", bufs=1) as pool:
            megaRE = pool.tile([128, REW], DT)
            slots = pool.tile([128, SLOTW], DT)
            xhl = pool.tile([128, 520], DT)
            xhh = pool.tile([128, 560], DT)
            wks = pool.tile([128, 96], DT)
            _emit(nc, tc, megaRE, slots, xhl, xhh, wks,
                  pre, tsk, cst, partials)
    nc.compile()
    return nc


def _raw_scan(nc, out, data0, data1, initial, op0, op1):
    eng = nc.vector
    return eng.add_instruction(mybir.InstTensorScalarPtr(
        name=eng.bass.get_next_instruction_name(),
        is_tensor_tensor_scan=True,
        is_scalar_tensor_tensor=True,
        op0=op0, op1=op1,
        ins=[eng.lower_ap(data0), eng.lower_ap_or_imm(initial),
             eng.lower_ap(data1)],
        outs=[eng.lower_ap(out)]))


def _emit(nc, tc, megaRE, slots, xhl, xhh, wks, pre, tsk, cst, partials):
    v = nc.vector
    U_O = DS_O        # u2 (bce per-cell) region, 42-pitch, reuses d staging

    def rcells(off, b0, nb, dc=0):
        return _manual_ap(megaRE[:, off + 1 + PITCH * b0 + dc:
                                 off + 2 + PITCH * b0 + dc], 0,
                          [[PITCH, nb], [1, NB]])

    def rsmb(off, b0, nb):
        return megaRE[:, off + b0:off + b0 + nb].unsqueeze(2) \
            .broadcast_to([128, nb, NB])

    def rskw(k, b0, nb):
        return _manual_ap(megaRE[:, TSK_O + k * SKW + 32 * b0:
                                 TSK_O + k * SKW + 32 * b0 + 1], 0,
                          [[32, nb], [1, NB]])

    def rocol(nb, dc=0):
        return megaRE[:, CONST_O + 1 + dc:CONST_O + 1 + dc + NB] \
            .unsqueeze(1).broadcast_to([128, nb, NB])

    def rcells84(base_col, b0, nb):
        return _manual_ap(megaRE[:, base_col + SP * b0:base_col + SP * b0 + 1],
                          0, [[SP, nb], [2, NB]])

    # ---------------- input DMAs (d-build deps first) ----------------
    nc.sync.dma_start(out=megaRE[:, TSK_O:TSK_O + 300], in_=tsk[:, 0:300])
    nc.sync.dma_start(out=megaRE[:, PRE_O:PRE_O + 99], in_=pre[:])
    nc.sync.dma_start(out=megaRE[:, TSK_O + SKW:TSK_O + SKW + 300],
                      in_=tsk[:, SKW:SKW + 300])
    nc.sync.dma_start(out=megaRE[:, TSK_O + 300:TSK_O + SKW],
                      in_=tsk[:, 300:SKW])
    nc.sync.dma_start(out=megaRE[:, TSK_O + SKW + 300:TSK_O + 2 * SKW],
                      in_=tsk[:, SKW + 300:2 * SKW])
    nc.sync.dma_start(out=megaRE[:, CONST_O:CONST_O + 44], in_=cst[:])
    nc.sync.dma_start(out=wks[:, 0:NB], in_=cst[:, 1:1 + NB])   # iota 1..41
    nc.sync.dma_start(out=megaRE[:, TSK_O + 2 * SKW:TSK_O + 3 * SKW],
                      in_=tsk[:, 2 * SKW:3 * SKW])

    # ---------------- bce scalars (Act engine; overlaps everything) --------
    v.tensor_scalar(out=megaRE[:, CLZ_O:CLZ_O + NBLK],
                    in0=megaRE[:, PRE_O + 66:PRE_O + 99],
                    scalar1=-4.0, scalar2=4.0, op0=AL.max, op1=AL.min)
    nc.scalar.activation(megaRE[:, NG1_O:NG1_O + NBLK],
                         megaRE[:, CLZ_O:CLZ_O + NBLK],
                         mybir.ActivationFunctionType.Exp)
    nc.scalar.activation(megaRE[:, NG2_O:NG2_O + NBLK],
                         megaRE[:, CLZ_O:CLZ_O + NBLK],
                         mybir.ActivationFunctionType.Exp, scale=-1.0)
    nc.scalar.activation(megaRE[:, SPZ_O:SPZ_O + NBLK],
                         megaRE[:, NG1_O:NG1_O + NBLK],
                         mybir.ActivationFunctionType.Ln, bias=1.0)
    nc.scalar.activation(megaRE[:, SPN_O:SPN_O + NBLK],
                         megaRE[:, NG2_O:NG2_O + NBLK],
                         mybir.ActivationFunctionType.Ln, bias=1.0)
    v.scalar_tensor_tensor(out=megaRE[:, QZ_O:QZ_O + NBLK],
                           in0=megaRE[:, SPN_O:SPN_O + NBLK], scalar=5.0,
                           in1=megaRE[:, SPZ_O:SPZ_O + NBLK],
                           op0=AL.mult, op1=AL.subtract)

    # scratch-region pads/leads: zeroed once (scan-state resets + uninit)
    for off in (SA_O, SB_O, SC_O):
        v.memset(_manual_ap(megaRE[:, off + 1 + NB:off + 2 + NB], 0,
                            [[PITCH, NBLK]]), 0.0)
        v.memset(megaRE[:, off:off + 1], 0.0)

    # ---------------- d build (84-pitch interleaved), quadrant 0 first -----
    def dbuild(b0, nb):
        w0 = DS_O + SP * b0
        v.memset(megaRE[:, w0:w0 + SP * nb], 0.0)
        v.memset(_manual_ap(megaRE[:, w0 + 82:w0 + 83], 0, [[SP, nb]]), BIGP)
        v.memset(_manual_ap(megaRE[:, w0 + 83:w0 + 84], 0, [[SP, nb]]), BIGP)
        v.tensor_tensor(out=rcells(SA_O, b0, nb), in0=rsmb(PRE_O, b0, nb),
                        in1=rskw(0, b0, nb), op=AL.subtract)
        v.scalar_tensor_tensor(out=rcells(R1_O, b0, nb),
                               in0=rcells(SA_O, b0, nb), scalar=-1.0,
                               in1=rcells(SA_O, b0, nb),
                               op0=AL.mult, op1=AL.max)
        v.tensor_tensor(out=rcells(SA_O, b0, nb), in0=rsmb(PRE_O + 33, b0, nb),
                        in1=rskw(1, b0, nb), op=AL.subtract)
        v.scalar_tensor_tensor(out=rcells(R2_O, b0, nb),
                               in0=rcells(SA_O, b0, nb), scalar=-1.0,
                               in1=rcells(SA_O, b0, nb),
                               op0=AL.mult, op1=AL.max)
        v.tensor_tensor(out=rcells84(DS_O + 1, b0, nb),
                        in0=rcells(R1_O, b0, nb), in1=rcells(R2_O, b0, nb),
                        op=AL.add)

    def stage(k):
        w = BKS[k] * SP
        nc.sync.dma_start(
            out=_manual_ap(slots[32 * k:32 * k + 4, 0:1], 0,
                           [[w, 32], [1, w]]),
            in_=_manual_ap(megaRE[:, DS_O + 8 * k * SP:DS_O + 8 * k * SP + 1],
                           0, [[1, w]]))

    dbuild(0, 8)
    stage(0)
    # virtual row r=0 (quadrant 0, slot 0): odd cols BIG, col 41 (o=20) = 0
    v.memset(slots[0:4, 0:SP], BIGP)
    v.memset(slots[0:4, 41:42], 0.0)
    dbuild(8, 25)
    for k in range(1, 4):
        stage(k)

    # ---------------- chunk machinery ----------------
    chunks = []

    def drain(n=1):
        for _ in range(n):
            if chunks:
                chunks.pop(0)()

    # phase-A chunks: u2 = sp + tz*qz per block (2 ops each)
    for b in range(NBLK):
        chunks.append(lambda b=b: v.tensor_tensor(
            out=rcells(U_O, b, 1), in0=rskw(2, b, 1), in1=rsmb(QZ_O, b, 1),
            op=AL.mult))
        chunks.append(lambda b=b: v.tensor_tensor(
            out=rcells(U_O, b, 1), in0=rcells(U_O, b, 1),
            in1=rsmb(SPZ_O, b, 1), op=AL.add))

    # ---------------- phase A: one fused scan per row ----------------
    def arow(q0, cur, prevbase):
        _raw_scan(nc, out=slots[q0:q0 + 4, cur:cur + 82],
                  data0=_manual_ap(slots[q0:q0 + 4, 0:1], prevbase + 1,
                                   [[2, NB], [2, 2]]),
                  data1=slots[q0:q0 + 4, cur:cur + 82],
                  initial=BIGP, op0=AL.min, op1=AL.add)

    v.memset(megaRE[:, DRE_O:DRE_O + 1], BIGP)           # lead pad
    v.memset(megaRE[:, DREP_O:DREP_O + 1 + SP], BIGP)    # row-0 prev junk
    v.memset(megaRE[:, DREP_O + 1 + 32 * SP:DREP_O + 1 + 33 * SP], BIGP)

    def dshift(c0, cw):
        """DrePrev partition shift for DRE col range [c0, c0+cw)."""
        d0 = max(c0, 1 + SP)      # p=0 rows: dst col x <- src col x - 84
        for s in range(4):
            nc.sync.dma_start(
                out=megaRE[32 * s + 1:32 * s + 32, DREP_O + c0:DREP_O + c0 + cw],
                in_=megaRE[32 * s:32 * s + 31, DRE_O + c0:DRE_O + c0 + cw])
            nc.sync.dma_start(
                out=megaRE[32 * s:32 * s + 1,
                           DREP_O + d0:DREP_O + c0 + cw],
                in_=megaRE[32 * s + 31:32 * s + 32,
                           DRE_O + d0 - SP:DRE_O + c0 + cw - SP])

    for k in range(4):
        q0 = 32 * k
        r0 = max(1, 256 * k)
        r1 = N if k == 3 else 256 * k + 255
        if k > 0:
            arow(q0, _dslot(r0)[1], BSCQ[k])
            r0 += 1
        for r in range(r0, r1 + 1):
            arow(q0, _dslot(r)[1], _dslot(r - 1)[1])
            if r % 4 == 0 and r > 64:
                drain()
            if r == 1023:
                # quadrant-3 DRE/DREP built straight from slots (rows <=1023
                # suffice for everything except DRE block 32): overlaps the
                # last row + avoids the transpose -> shift serial chain.
                nc.sync.dma_start(       # DRE blocks 24..31
                    out=_manual_ap(megaRE[:, DRE_O + 1 + 24 * SP:
                                           DRE_O + 2 + 24 * SP], 0,
                                   [[1, 8 * SP]]),
                    in_=_manual_ap(slots[96:100, 0:1], 0,
                                   [[756, 32], [1, 8 * SP]]))
                for s in range(4):       # DrePrev mains, blocks 24..31
                    nc.sync.dma_start(
                        out=megaRE[32 * s + 1:32 * s + 32,
                                   DREP_O + 1 + 24 * SP:DREP_O + 1 + 32 * SP],
                        in_=_manual_ap(slots[96 + s:97 + s, 0:1], 0,
                                       [[756, 31], [1, 8 * SP]]))
                for s in range(4):       # DrePrev p=0 rows, blocks 25..32
                    nc.gpsimd.dma_start(
                        out=megaRE[32 * s:32 * s + 1,
                                   DREP_O + 1 + 25 * SP:DREP_O + 1 + 33 * SP],
                        in_=slots[96 + s:97 + s, 279 * SP:287 * SP])
        if k < 3:                                        # boundary to k+1
            _, pc = _dslot(r1)
            nc.sync.dma_start(
                out=slots[q0 + 32:q0 + 36, BSCQ[k + 1]:BSCQ[k + 1] + SP],
                in_=slots[q0:q0 + 4, pc:pc + SP])
        if k < 3:
            # transpose this quadrant's D out (overlaps next quadrant's DP)
            w = BKS[k] * SP
            nc.sync.dma_start(
                out=_manual_ap(megaRE[:, DRE_O + 1 + 8 * k * SP:
                                       DRE_O + 2 + 8 * k * SP], 0, [[1, w]]),
                in_=_manual_ap(slots[q0:q0 + 4, 0:1], 0, [[w, 32], [1, w]]))
        else:
            nc.sync.dma_start(           # DRE block 32 (needs row 1024)
                out=_manual_ap(megaRE[:, DRE_O + 1 + 32 * SP:
                                       DRE_O + 2 + 32 * SP], 0, [[1, SP]]),
                in_=_manual_ap(slots[96:100, 8 * SP:8 * SP + 1], 0,
                               [[756, 32], [1, SP]]))
        if k == 1:      # DrePrev shifts, blocks 0..15 (needs DRE b 0..15)
            dshift(0, 1 + 16 * SP)
        if k == 2:      # DrePrev shifts, blocks 16..23
            dshift(1 + 16 * SP, 8 * SP)
            for s in range(4):           # DrePrev p=0, block 24 (row 767)
                nc.sync.dma_start(
                    out=megaRE[32 * s:32 * s + 1,
                               DREP_O + 1 + 24 * SP:DREP_O + 1 + 25 * SP],
                    in_=slots[64 + s:65 + s, 255 * SP:256 * SP])
    drain(len(chunks))

    # ---------------- choice bits (min-trick, 8 ops per range) -------------
    def choice(b0, nb):
        diag = rcells84(DREP_O + 2, b0, nb)
        up = rcells84(DREP_O + 4, b0, nb)
        left = rcells84(DRE_O, b0, nb)
        sa, sb, sc = (rcells(o, b0, nb) for o in (SA_O, SB_O, SC_O))
        yield lambda: v.tensor_tensor(out=sb, in0=diag, in1=up, op=AL.min)
        yield lambda: v.tensor_tensor(out=sa, in0=left, in1=sb, op=AL.is_lt)
        yield lambda: v.tensor_tensor(out=sb, in0=up, in1=left, op=AL.min)
        yield lambda: v.tensor_tensor(out=sb, in0=diag, in1=sb, op=AL.is_le)
        yield lambda: v.tensor_single_scalar(out=sc, in_=sa, scalar=0.0,
                                             op=AL.is_equal)    # notleft
        yield lambda: v.scalar_tensor_tensor(out=sb, in0=sb, scalar=-1.0,
                                             in1=rocol(nb, 1), op0=AL.mult,
                                             op1=AL.add)        # oc+2-isdiag
        yield lambda: v.tensor_tensor(out=sb, in0=sb, in1=sc, op=AL.mult)
        yield lambda: v.tensor_tensor(out=sc, in0=rocol(nb), in1=sc,
                                      op=AL.mult)               # Lval

    def gscan(off, b0, nb):
        return lambda: v.tensor_tensor_scan(
            out=megaRE[:, off + 1 + b0 * PITCH:off + 1 + (b0 + nb) * PITCH],
            data0=megaRE[:, SA_O + 1 + b0 * PITCH:SA_O + 1 + (b0 + nb) * PITCH],
            data1=megaRE[:, off + 1 + b0 * PITCH:off + 1 + (b0 + nb) * PITCH],
            initial=0.0, op0=AL.mult, op1=AL.add)

    for f in choice(29, 4):                              # top blocks: critical
        f()
    gscan(SB_O, 29, 4)()                                 # g, blocks 29..32
    nc.sync.dma_start(    # g top -> walk slots (steps i >= 927)
        out=_manual_ap(slots[64:68, 546:547], 0, [[HIW, 32], [1, 168]]),
        in_=_manual_ap(megaRE[:, SB_O + 1219:SB_O + 1220], 0, [[1, 168]]))
    for f in choice(24, 5):                              # next blocks
        f()
    gscan(SB_O, 24, 5)()                                 # g, blocks 24..28
    nc.sync.dma_start(    # g -> walk slots (steps i >= 767)
        out=_manual_ap(slots[64:68, 336:337], 0, [[HIW, 32], [1, 210]]),
        in_=_manual_ap(megaRE[:, SB_O + 1009:SB_O + 1010], 0, [[1, 210]]))

    # -------- chunks for the hi walk: rest of choice/g/L + g DMAs ----------
    for b in range(16, 24):
        for f in choice(b, 1):
            chunks.append(f)
        chunks.append(gscan(SB_O, b, 1))
        chunks.append(gscan(SC_O, b, 1))                  # L scan
    chunks.append(lambda: nc.sync.dma_start(              # g mid -> walk slots
        out=_manual_ap(slots[64:68, 0:1], 0, [[HIW, 32], [1, 336]]),
        in_=_manual_ap(megaRE[:, SB_O + 1 + LOW:SB_O + 2 + LOW], 0,
                       [[1, 336]])))

    def gx_dmas():
        for s in range(4):    # lo extra: g row 512
            nc.sync.dma_start(
                out=slots[s:s + 1, GX:GX + NB],
                in_=megaRE[32 * s:32 * s + 1,
                           SB_O + 1 + LOW:SB_O + 1 + LOW + NB])
    chunks.append(gx_dmas)
    for b in range(16):
        for f in choice(b, 1):
            chunks.append(f)
        chunks.append(gscan(SB_O, b, 1))
        chunks.append(gscan(SC_O, b, 1))
    chunks.append(lambda: nc.sync.dma_start(              # g-lo -> walk slots
        out=_manual_ap(slots[0:4, 0:1], 0, [[LOW, 32], [1, LOW]]),
        in_=_manual_ap(megaRE[:, SB_O + 1:SB_O + 2], 0, [[1, LOW]])))
    for b in range(24, NBLK):                             # L scans, top blocks
        chunks.append(gscan(SC_O, b, 1))

    # ---------------- walk hi ----------------
    v.memset(xhh[64:68, 512:513], 21.0)                  # x_1023
    v.memset(xhh[64:68, 513:544], 0.0)                   # junk rows > 1023
    v.memset(xhl[0:4, 0:1], 0.0)                         # x_{-1} junk
    for i in range(1023, 511, -1):
        gc = _gslot_hi(i + 1)
        v.scalar_tensor_tensor(
            out=wks[64:68, 41:41 + NB], in0=wks[64:68, 0:NB],
            scalar=xhh[64:68, i - 511:i - 510],
            in1=slots[64:68, gc:gc + NB],
            op0=AL.is_equal, op1=AL.mult,
            accum_out=xhh[64:68, i - 512:i - 511])
        drain()
    drain(len(chunks))

    nc.sync.dma_start(out=xhl[0:4, 512:513], in_=xhh[64:68, 0:1])   # handoff
    nc.vector.tensor_scalar_add(                         # xcol repack hi
        out=_manual_ap(megaRE[64:68, SCX2_O:SCX2_O + 1], 0, [[17, 32], [1, 17]]),
        in0=_manual_ap(xhh[64:68, 0:1], 0, [[1, 32], [32, 17]]), scalar1=0.0)
    nc.sync.dma_start(
        out=_manual_ap(megaRE[:, XC_O + 16:XC_O + 17], 0, [[1, 17]]),
        in_=_manual_ap(megaRE[64:68, SCX2_O:SCX2_O + 1], 0, [[17, 32], [1, 17]]))

    # -------- chunks for the lo walk: hi-half olo/mask/products ------------
    b32 = SA_O + 1 + 32 * PITCH

    def metrics_blk(b):
        sa1 = rcells(SA_O, b, 1)
        yield lambda: v.tensor_tensor(out=sa1, in0=rocol(1),
                                      in1=rsmb(XC_O, b, 1), op=AL.is_equal)
        yield lambda: v.tensor_tensor(out=sa1, in0=sa1,
                                      in1=rcells(SC_O, b, 1), op=AL.mult)
        yield lambda: v.tensor_reduce(out=megaRE[:, OLO_O + b:OLO_O + b + 1],
                                      in_=rcells(SA_O, b, 1),
                                      axis=mybir.AxisListType.X, op=AL.add)
        if b == 32:
            yield lambda: v.memset(megaRE[:, b32:b32 + NB], 0.0)
            for s in range(4):                           # row 1023 mask
                q = 32 * s

                def rebuild(q=q):
                    ic0 = megaRE[q:q + 1, CONST_O + 1:CONST_O + 1 + NB]
                    wt = wks[q:q + 1, 41:41 + NB]
                    v.scalar_tensor_tensor(
                        out=wt, in0=ic0,
                        scalar=megaRE[q:q + 1, OLO_O + 32:OLO_O + 33],
                        in1=ic0, op0=AL.is_ge, op1=AL.bypass)
                    v.scalar_tensor_tensor(
                        out=megaRE[q:q + 1, b32:b32 + NB], in0=ic0,
                        scalar=megaRE[q:q + 1, XC_O + 32:XC_O + 33],
                        in1=wt, op0=AL.is_le, op1=AL.mult)
                yield rebuild
        else:
            yield lambda: v.tensor_tensor(out=sa1, in0=rocol(1),
                                          in1=rsmb(OLO_O, b, 1), op=AL.is_ge)
            yield lambda: v.tensor_tensor(out=rcells(SB_O, b, 1),
                                          in0=rocol(1), in1=rsmb(XC_O, b, 1),
                                          op=AL.is_le)
            yield lambda: v.tensor_tensor(out=sa1, in0=sa1,
                                          in1=rcells(SB_O, b, 1), op=AL.mult)
        for off in (R1_O, R2_O, U_O):                    # products, in place
            yield lambda off=off: v.tensor_tensor(
                out=rcells(off, b, 1), in0=rcells(off, b, 1), in1=sa1,
                op=AL.mult)
        for j, off in enumerate((R1_O, R2_O, U_O, SA_O)):  # partial reduces
            yield lambda j=j, off=off: v.tensor_reduce(
                out=megaRE[:, RSC_O + 33 * j + b:RSC_O + 33 * j + b + 1],
                in_=rcells(off, b, 1), axis=mybir.AxisListType.X, op=AL.add)

    for b in range(16, NBLK):
        for f in metrics_blk(b):
            chunks.append(f)

    # ---------------- walk lo ----------------
    chunks2 = []                # mid-walk group: blocks 8..15 metrics
    for b in range(8, 16):
        for f in metrics_blk(b):
            chunks2.append(f)
    for i in range(511, 0, -1):
        gc = GX if i == 511 else _gslot_lo(i + 1)
        v.scalar_tensor_tensor(
            out=wks[0:4, 41:41 + NB], in0=wks[0:4, 0:NB],
            scalar=xhl[0:4, i + 1:i + 2],
            in1=slots[0:4, gc:gc + NB],
            op0=AL.is_equal, op1=AL.mult,
            accum_out=xhl[0:4, i:i + 1])
        if i == 255:            # xcol for rows 256..511 is final
            nc.vector.tensor_scalar_add(
                out=_manual_ap(megaRE[0:4, SCX_O:SCX_O + 1], 0,
                               [[8, 32], [1, 8]]),
                in0=_manual_ap(xhl[0:4, 256:257], 0, [[1, 32], [32, 8]]),
                scalar1=0.0)
            nc.sync.dma_start(
                out=_manual_ap(megaRE[:, XC_O + 8:XC_O + 9], 0, [[1, 8]]),
                in_=_manual_ap(megaRE[0:4, SCX_O:SCX_O + 1], 0,
                               [[8, 32], [1, 8]]))
        if 225 < i < 490:
            drain()
        elif i <= 225 and chunks2:
            chunks2.pop(0)()
    drain(len(chunks))
    for f in chunks2:
        f()

    # ---------------- tail: blocks 0..7 olo/mask/products + reduces --------
    nc.vector.tensor_scalar_add(                         # xcol repack, b 0..7
        out=_manual_ap(megaRE[0:4, SCX_O:SCX_O + 1], 0, [[8, 32], [1, 8]]),
        in0=_manual_ap(xhl[0:4, 0:1], 0, [[1, 32], [32, 8]]), scalar1=0.0)
    nc.sync.dma_start(
        out=_manual_ap(megaRE[:, XC_O:XC_O + 1], 0, [[1, 8]]),
        in_=_manual_ap(megaRE[0:4, SCX_O:SCX_O + 1], 0, [[8, 32], [1, 8]]))
    salo = rcells(SA_O, 0, 8)
    v.tensor_tensor(out=salo, in0=rocol(8), in1=rsmb(XC_O, 0, 8),
                    op=AL.is_equal)
    v.tensor_tensor(out=salo, in0=salo, in1=rcells(SC_O, 0, 8), op=AL.mult)
    v.tensor_reduce(out=megaRE[:, OLO_O:OLO_O + 8], in_=salo,
                    axis=mybir.AxisListType.X, op=AL.add)
    v.tensor_tensor(out=salo, in0=rocol(8), in1=rsmb(OLO_O, 0, 8),
                    op=AL.is_ge)
    v.tensor_tensor(out=rcells(SB_O, 0, 8), in0=rocol(8),
                    in1=rsmb(XC_O, 0, 8), op=AL.is_le)
    v.tensor_tensor(out=salo, in0=salo, in1=rcells(SB_O, 0, 8), op=AL.mult)
    for s in range(4):                                   # virtual row 0
        v.memset(megaRE[32 * s:32 * s + 1, SA_O + 1:SA_O + 1 + NB], 0.0)
    for off in (R1_O, R2_O, U_O):
        v.tensor_tensor(out=rcells(off, 0, 8), in0=rcells(off, 0, 8),
                        in1=salo, op=AL.mult)
    for j, off in enumerate((R1_O, R2_O, U_O, SA_O)):
        v.tensor_reduce(out=megaRE[:, RSC_O + 33 * j:RSC_O + 33 * j + 8],
                        in_=rcells(off, 0, 8),
                        axis=mybir.AxisListType.X, op=AL.add)
    for j in range(4):
        v.tensor_reduce(out=megaRE[:, RED_O + j:RED_O + j + 1],
                        in_=megaRE[:, RSC_O + 33 * j:RSC_O + 33 * j + NBLK],
                        axis=mybir.AxisListType.X, op=AL.add)

    nc.sync.dma_start(out=partials[:], in_=megaRE[:, RED_O:RED_O + 4])


def _get_module():
    if "nc" not in _CACHE:
        _CACHE["nc"] = _build_module()
    return _CACHE["nc"]


def _make_inmaps(preds, targs):
    preds = np.ascontiguousarray(preds, dtype=np.float32)
    targs = np.ascontiguousarray(targs, dtype=np.float32)
    cst = np.tile(np.arange(44, dtype=np.float32), (128, 1))
    pp = np.arange(32)
    bb = np.arange(NBLK)
    r_idx = pp[:, None] + 32 * bb[None, :]              # [32, 33] = i + 1
    r_ok = (r_idx >= 1) & (r_idx <= N)
    r_cl = np.clip(r_idx - 1, 0, N - 1)
    uu = np.arange(SKW)
    t_idx = uu[None, :] + pp[:, None] - 21              # [32, SKW]
    t_ok = (t_idx >= 0) & (t_idx < N)
    t_cl = np.clip(t_idx, 0, N - 1)
    in_maps = []
    for c in range(NCORES):
        ps = preds[c * BC:(c + 1) * BC]
        ts = targs[c * BC:(c + 1) * BC]
        prev = np.zeros((BC, 32, 3 * NBLK), dtype=np.float32)
        tskv = np.zeros((BC, 32, 3 * SKW), dtype=np.float32)
        for k in range(3):
            vv = ps[:, :, k][:, r_cl]                   # [BC, 32, NBLK]
            prev[:, :, k * NBLK:(k + 1) * NBLK] = np.where(r_ok[None], vv, 0.0)
            fill = FILL if k < 2 else 0.0
            ww = ts[:, :, k][:, t_cl]                   # [BC, 32, SKW]
            tskv[:, :, k * SKW:(k + 1) * SKW] = np.where(t_ok[None], ww, fill)
        in_maps.append({"pre": prev.reshape(128, 3 * NBLK),
                        "tsk": tskv.reshape(128, 3 * SKW), "cst": cst})
    return in_maps


def _reduce_host(parts_list, subcoef):
    c0, c1 = float(subcoef[0]), float(subcoef[1])
    loss = 0.0
    for parts in parts_list:
        m = parts.reshape(BC, 32, 4).sum(axis=1)        # [BC, (Sx,Sy,Sb,cnt)]
        for s in range(BC):
            sx, sy, sb, cnt = (float(m[s, k]) for k in range(4))
            loss += c0 * sx + c1 * sy + 0.1 * sb / cnt
    return np.float32(loss)


def run(preds, targs, subcoef, trace=False):
    nc = _get_module()
    in_maps = _make_inmaps(preds, targs)
    res = run_bass_kernel_spmd(nc, in_maps, core_ids=list(range(NCORES)),
                               trace=trace)
    parts = [r["partials"] for r in res.results]
    return _reduce_host(parts, np.asarray(subcoef)), res


def kernel(preds, targs, subcoef):
    out, _ = run(preds, targs, subcoef)
    return out


# revision 28
# speedup vs baseline: 1.4754x; 1.0101x over previous
"""Banded DTW loss kernel for Trainium2 (Bass/Tile), 8-core data-parallel.

Layout: sample-major partitions q = 32*s + p (s = sample 0..3, p = row%32).

Phase A (forward DP) uses ONE fused 82-wide tensor_tensor_scan per row:
the row slot holds d interleaved with zeros (d[o] at odd col 2o+1), and
data0 is a 3-D overlapping-pair window over the previous row's slot
(odd cols hold D).  Scan semantics state = min(data0[t], state) + data1[t]
then give, per band cell o:
    t=2o   : m   = min(D'[o],   D[o-1]) + 0
    t=2o+1 : D[o]= min(D'[o+1], m     ) + d[o]
which is exactly the banded DTW recurrence.  The scan writes the slot in
place (d -> interleaved m/D).  Rows are distributed over 4 partition
quadrants (0/32/64/96) of 256 rows each, p-major slot order, so the
RE<->walk transposes are single rectangular DMAs per quadrant.

Band-invalid cells get cost ~2e15 via host-side target padding (1e15
fill outside [0,N)), standing in for the explicit +inf band mask.

Backtrack: choice bits + g/L tables built in RE layout (42-pitch cell
regions whose per-block pad column doubles as the band pad and as the
scan-state reset), two full-width scans, g copied to walk layout by two
rectangular DMAs, then two serial walk loops (one STT+accum per row).
Path masks + L1/BCE metrics reduce in RE; host sums the partials.
"""

import numpy as np

import concourse.bacc as bacc
import concourse.bass as bass
import concourse.mybir as mybir
import concourse.tile as tile
from concourse.bass_utils import run_bass_kernel_spmd

B, N = 32, 1024
NB = 41                   # band width
PITCH = 42                # RE cell-block pitch (41 cells + 1 pad)
SP = 84                   # phase-A slot pitch (interleaved, 82 + 2 pads)
NBLK = 33                 # RE blocks (r = i+1 in [0,1056))
SKW = 1066                # skewed targ row length
NCORES = 8
BC = B // NCORES
BIGP = 1e30               # pad / DP "infinity"
FILL = 1e15               # targ pad fill -> invalid-cell cost ~2e15

AL = mybir.AluOpType
DT = mybir.dt.float32

RW = 1388                 # 42-pitch region width (lead pad + 33*42 + spare)
RW84 = 2773               # 84-pitch region width (lead pad + 33*84)

# ---- megaRE column offsets ----
TSK_O = 0                         # 3 * 1066 skewed targs (x, y, z)
PRE_O = TSK_O + 3 * SKW           # 3 * 33 preds (x, y, z)
CONST_O = PRE_O + 99              # iota 0..43
R1_O = CONST_O + 44               # |dx| cells (42-pitch)
R2_O = R1_O + RW                  # |dy| cells
SA_O = R2_O + RW                  # scratch A (isleft / mask)
SB_O = SA_O + RW                  # scratch B (gval -> gfull)
SC_O = SB_O + RW                  # scratch C (Lval -> Lfull -> metric tmp)
DS_O = SC_O + RW                  # d staging, 84-pitch interleaved (no lead)
DRE_O = DS_O + NBLK * SP          # D cells, 84-pitch (lead pad)
DREP_O = DRE_O + RW84             # D prev-row cells, 84-pitch (lead pad)
XC_O = DREP_O + RW84              # xcol [33]
OLO_O = XC_O + 33
CLZ_O = OLO_O + 33
NG1_O = CLZ_O + 33
NG2_O = NG1_O + 33
SPZ_O = NG2_O + 33
SPN_O = SPZ_O + 33
QZ_O = SPN_O + 33
RED_O = QZ_O + 33                 # Sx, Sy, Sbce, cnt
SCX_O = RED_O + 4                 # xcol repack lo (512)
SCX2_O = SCX_O + 512              # xcol repack hi (544)
RSC_O = SCX2_O + 544              # per-block partial reduces (4 x 33)
REW = RSC_O + 132 + 4

# ---- slots tile ----
# Phase-A D slots: quadrant k on partitions 32k..32k+4 holds rows with
# r//32 in [8k, 8k+BK_k), BK = (8,8,8,9), p-major 84-pitch slots.
BKS = (8, 8, 8, 9)
BSCQ = tuple(bk * 32 * SP for bk in BKS)     # boundary scratch col per quad
# Walk g slots: lo (rows 0..511) on partitions 0:4, 16 blocks; hi (rows
# 512..1055) on 64:68, 17 blocks; 42-pitch p-major.
LOW = 16 * PITCH
HIW = 17 * PITCH
GX = 512 * PITCH          # lo extra slot for g row 512
SLOTW = BSCQ[3] + SP + 4

_CACHE = {}


def _manual_ap(base, extra_off, dims):
    ap0 = [list(base.ap[0])]
    return bass.AP(base.tensor, base.offset + extra_off,
                   ap0 + [list(d) for d in dims])


def _dslot(r):
    k = min(r // 256, 3)
    return k, ((r % 32) * BKS[k] + (r // 32 - 8 * k)) * SP


def _gslot_lo(r):
    return ((r % 32) * 16 + r // 32) * PITCH


def _gslot_hi(r):
    return ((r // 32 - 16) * 32 + (r % 32)) * PITCH      # block-major


def _build_module():
    nc = bacc.Bacc("TRN2", target_bir_lowering=False, debug=False,
                   num_devices=NCORES, detect_race_conditions=False)
    pre = nc.dram_tensor("pre", [128, 99], DT, kind="ExternalInput")
    tsk = nc.dram_tensor("tsk", [128, 3 * SKW], DT, kind="ExternalInput")
    cst = nc.dram_tensor("cst", [128, 44], DT, kind="ExternalInput")
    partials = nc.dram_tensor("partials", [128, 4], DT, kind="ExternalOutput")

    with tile.TileContext(nc) as tc:
        with tc.tile_pool(name="main", bufs=1) as pool:
            megaRE = pool.tile([128, REW], DT)
            slots = pool.tile([128, SLOTW], DT)
            xhl = pool.tile([128, 520], DT)
            xhh = pool.tile([128, 560], DT)
            wks = pool.tile([128, 96], DT)
            _emit(nc, tc, megaRE, slots, xhl, xhh, wks,
                  pre, tsk, cst, partials)
    nc.compile()
    return nc


def _raw_scan(nc, out, data0, data1, initial, op0, op1):
    eng = nc.vector
    return eng.add_instruction(mybir.InstTensorScalarPtr(
        name=eng.bass.get_next_instruction_name(),
        is_tensor_tensor_scan=True,
        is_scalar_tensor_tensor=True,
        op0=op0, op1=op1,
        ins=[eng.lower_ap(data0), eng.lower_ap_or_imm(initial),
             eng.lower_ap(data1)],
        outs=[eng.lower_ap(out)]))


def _emit(nc, tc, megaRE, slots, xhl, xhh, wks, pre, tsk, cst, partials):
    v = nc.vector
    U_O = DS_O        # u2 (bce per-cell) region, 42-pitch, reuses d staging

    def rcells(off, b0, nb, dc=0):
        return _manual_ap(megaRE[:, off + 1 + PITCH * b0 + dc:
                                 off + 2 + PITCH * b0 + dc], 0,
                          [[PITCH, nb], [1, NB]])

    def rsmb(off, b0, nb):
        return megaRE[:, off + b0:off + b0 + nb].unsqueeze(2) \
            .broadcast_to([128, nb, NB])

    def rskw(k, b0, nb):
        return _manual_ap(megaRE[:, TSK_O + k * SKW + 32 * b0:
                                 TSK_O + k * SKW + 32 * b0 + 1], 0,
                          [[32, nb], [1, NB]])

    def rocol(nb, dc=0):
        return megaRE[:, CONST_O + 1 + dc:CONST_O + 1 + dc + NB] \
            .unsqueeze(1).broadcast_to([128, nb, NB])

    def rcells84(base_col, b0, nb):
        return _manual_ap(megaRE[:, base_col + SP * b0:base_col + SP * b0 + 1],
                          0, [[SP, nb], [2, NB]])

    # ---------------- input DMAs (d-build deps first) ----------------
    nc.sync.dma_start(out=megaRE[:, TSK_O:TSK_O + 300], in_=tsk[:, 0:300])
    nc.sync.dma_start(out=megaRE[:, PRE_O:PRE_O + 99], in_=pre[:])
    nc.sync.dma_start(out=megaRE[:, TSK_O + SKW:TSK_O + SKW + 300],
                      in_=tsk[:, SKW:SKW + 300])
    nc.sync.dma_start(out=megaRE[:, TSK_O + 300:TSK_O + SKW],
                      in_=tsk[:, 300:SKW])
    nc.sync.dma_start(out=megaRE[:, TSK_O + SKW + 300:TSK_O + 2 * SKW],
                      in_=tsk[:, SKW + 300:2 * SKW])
    nc.sync.dma_start(out=megaRE[:, CONST_O:CONST_O + 44], in_=cst[:])
    nc.sync.dma_start(out=wks[:, 0:NB], in_=cst[:, 1:1 + NB])   # iota 1..41
    nc.sync.dma_start(out=megaRE[:, TSK_O + 2 * SKW:TSK_O + 3 * SKW],
                      in_=tsk[:, 2 * SKW:3 * SKW])

    # ---------------- bce scalars (Act engine; overlaps everything) --------
    v.tensor_scalar(out=megaRE[:, CLZ_O:CLZ_O + NBLK],
                    in0=megaRE[:, PRE_O + 66:PRE_O + 99],
                    scalar1=-4.0, scalar2=4.0, op0=AL.max, op1=AL.min)
    nc.scalar.activation(megaRE[:, NG1_O:NG1_O + NBLK],
                         megaRE[:, CLZ_O:CLZ_O + NBLK],
                         mybir.ActivationFunctionType.Exp)
    nc.scalar.activation(megaRE[:, NG2_O:NG2_O + NBLK],
                         megaRE[:, CLZ_O:CLZ_O + NBLK],
                         mybir.ActivationFunctionType.Exp, scale=-1.0)
    nc.scalar.activation(megaRE[:, SPZ_O:SPZ_O + NBLK],
                         megaRE[:, NG1_O:NG1_O + NBLK],
                         mybir.ActivationFunctionType.Ln, bias=1.0)
    nc.scalar.activation(megaRE[:, SPN_O:SPN_O + NBLK],
                         megaRE[:, NG2_O:NG2_O + NBLK],
                         mybir.ActivationFunctionType.Ln, bias=1.0)
    v.scalar_tensor_tensor(out=megaRE[:, QZ_O:QZ_O + NBLK],
                           in0=megaRE[:, SPN_O:SPN_O + NBLK], scalar=5.0,
                           in1=megaRE[:, SPZ_O:SPZ_O + NBLK],
                           op0=AL.mult, op1=AL.subtract)

    # scratch-region pads/leads: zeroed once (scan-state resets + uninit)
    for off in (SA_O, SB_O, SC_O):
        v.memset(_manual_ap(megaRE[:, off + 1 + NB:off + 2 + NB], 0,
                            [[PITCH, NBLK]]), 0.0)
        v.memset(megaRE[:, off:off + 1], 0.0)

    # ---------------- d build (84-pitch interleaved), quadrant 0 first -----
    def dbuild(b0, nb):
        w0 = DS_O + SP * b0
        v.memset(megaRE[:, w0:w0 + SP * nb], 0.0)
        v.memset(_manual_ap(megaRE[:, w0 + 82:w0 + 83], 0, [[SP, nb]]), BIGP)
        v.memset(_manual_ap(megaRE[:, w0 + 83:w0 + 84], 0, [[SP, nb]]), BIGP)
        v.tensor_tensor(out=rcells(SA_O, b0, nb), in0=rsmb(PRE_O, b0, nb),
                        in1=rskw(0, b0, nb), op=AL.subtract)
        v.scalar_tensor_tensor(out=rcells(R1_O, b0, nb),
                               in0=rcells(SA_O, b0, nb), scalar=-1.0,
                               in1=rcells(SA_O, b0, nb),
                               op0=AL.mult, op1=AL.max)
        v.tensor_tensor(out=rcells(SA_O, b0, nb), in0=rsmb(PRE_O + 33, b0, nb),
                        in1=rskw(1, b0, nb), op=AL.subtract)
        v.scalar_tensor_tensor(out=rcells(R2_O, b0, nb),
                               in0=rcells(SA_O, b0, nb), scalar=-1.0,
                               in1=rcells(SA_O, b0, nb),
                               op0=AL.mult, op1=AL.max)
        v.tensor_tensor(out=rcells84(DS_O + 1, b0, nb),
                        in0=rcells(R1_O, b0, nb), in1=rcells(R2_O, b0, nb),
                        op=AL.add)

    def stage(k):
        w = BKS[k] * SP
        nc.sync.dma_start(
            out=_manual_ap(slots[32 * k:32 * k + 4, 0:1], 0,
                           [[w, 32], [1, w]]),
            in_=_manual_ap(megaRE[:, DS_O + 8 * k * SP:DS_O + 8 * k * SP + 1],
                           0, [[1, w]]))

    dbuild(0, 8)
    stage(0)
    # virtual row r=0 (quadrant 0, slot 0): odd cols BIG, col 41 (o=20) = 0
    v.memset(slots[0:4, 0:SP], BIGP)
    v.memset(slots[0:4, 41:42], 0.0)
    dbuild(8, 25)
    for k in range(1, 4):
        stage(k)

    # ---------------- chunk machinery ----------------
    chunks = []

    def drain(n=1):
        for _ in range(n):
            if chunks:
                chunks.pop(0)()

    # phase-A chunks: u2 = sp + tz*qz per block (2 ops each)
    for b in range(NBLK):
        chunks.append(lambda b=b: v.tensor_tensor(
            out=rcells(U_O, b, 1), in0=rskw(2, b, 1), in1=rsmb(QZ_O, b, 1),
            op=AL.mult))
        chunks.append(lambda b=b: v.tensor_tensor(
            out=rcells(U_O, b, 1), in0=rcells(U_O, b, 1),
            in1=rsmb(SPZ_O, b, 1), op=AL.add))

    # ---------------- phase A: one fused scan per row ----------------
    def arow(q0, cur, prevbase):
        _raw_scan(nc, out=slots[q0:q0 + 4, cur:cur + 82],
                  data0=_manual_ap(slots[q0:q0 + 4, 0:1], prevbase + 1,
                                   [[2, NB], [2, 2]]),
                  data1=slots[q0:q0 + 4, cur:cur + 82],
                  initial=BIGP, op0=AL.min, op1=AL.add)

    v.memset(megaRE[:, DRE_O:DRE_O + 1], BIGP)           # lead pad
    v.memset(megaRE[:, DREP_O:DREP_O + 1 + SP], BIGP)    # row-0 prev junk
    v.memset(megaRE[:, DREP_O + 1 + 32 * SP:DREP_O + 1 + 33 * SP], BIGP)

    def dshift(c0, cw):
        """DrePrev partition shift for DRE col range [c0, c0+cw)."""
        d0 = max(c0, 1 + SP)      # p=0 rows: dst col x <- src col x - 84
        for s in range(4):
            nc.sync.dma_start(
                out=megaRE[32 * s + 1:32 * s + 32, DREP_O + c0:DREP_O + c0 + cw],
                in_=megaRE[32 * s:32 * s + 31, DRE_O + c0:DRE_O + c0 + cw])
            nc.sync.dma_start(
                out=megaRE[32 * s:32 * s + 1,
                           DREP_O + d0:DREP_O + c0 + cw],
                in_=megaRE[32 * s + 31:32 * s + 32,
                           DRE_O + d0 - SP:DRE_O + c0 + cw - SP])

    for k in range(4):
        q0 = 32 * k
        r0 = max(1, 256 * k)
        r1 = N if k == 3 else 256 * k + 255
        if k > 0:
            arow(q0, _dslot(r0)[1], BSCQ[k])
            r0 += 1
        for r in range(r0, r1 + 1):
            arow(q0, _dslot(r)[1], _dslot(r - 1)[1])
            if r % 4 == 0 and r > 64:
                drain()
            if r == 1023:
                # quadrant-3 DRE/DREP built straight from slots (rows <=1023
                # suffice for everything except DRE block 32): overlaps the
                # last row + avoids the transpose -> shift serial chain.
                nc.sync.dma_start(       # DRE blocks 24..31
                    out=_manual_ap(megaRE[:, DRE_O + 1 + 24 * SP:
                                           DRE_O + 2 + 24 * SP], 0,
                                   [[1, 8 * SP]]),
                    in_=_manual_ap(slots[96:100, 0:1], 0,
                                   [[756, 32], [1, 8 * SP]]))
                for s in range(4):       # DrePrev mains, blocks 24..31
                    nc.sync.dma_start(
                        out=megaRE[32 * s + 1:32 * s + 32,
                                   DREP_O + 1 + 24 * SP:DREP_O + 1 + 32 * SP],
                        in_=_manual_ap(slots[96 + s:97 + s, 0:1], 0,
                                       [[756, 31], [1, 8 * SP]]))
                for s in range(4):       # DrePrev p=0 rows, blocks 25..32
                    nc.gpsimd.dma_start(
                        out=megaRE[32 * s:32 * s + 1,
                                   DREP_O + 1 + 25 * SP:DREP_O + 1 + 33 * SP],
                        in_=slots[96 + s:97 + s, 279 * SP:287 * SP])
        if k < 3:                                        # boundary to k+1
            _, pc = _dslot(r1)
            nc.sync.dma_start(
                out=slots[q0 + 32:q0 + 36, BSCQ[k + 1]:BSCQ[k + 1] + SP],
                in_=slots[q0:q0 + 4, pc:pc + SP])
        if k < 3:
            # transpose this quadrant's D out (overlaps next quadrant's DP)
            w = BKS[k] * SP
            nc.sync.dma_start(
                out=_manual_ap(megaRE[:, DRE_O + 1 + 8 * k * SP:
                                       DRE_O + 2 + 8 * k * SP], 0, [[1, w]]),
                in_=_manual_ap(slots[q0:q0 + 4, 0:1], 0, [[w, 32], [1, w]]))
        else:
            nc.sync.dma_start(           # DRE block 32 (needs row 1024)
                out=_manual_ap(megaRE[:, DRE_O + 1 + 32 * SP:
                                       DRE_O + 2 + 32 * SP], 0, [[1, SP]]),
                in_=_manual_ap(slots[96:100, 8 * SP:8 * SP + 1], 0,
                               [[756, 32], [1, SP]]))
        if k == 1:      # DrePrev shifts, blocks 0..15 (needs DRE b 0..15)
            dshift(0, 1 + 16 * SP)
        if k == 2:      # DrePrev shifts, blocks 16..23
            dshift(1 + 16 * SP, 8 * SP)
            for s in range(4):           # DrePrev p=0, block 24 (row 767)
                nc.sync.dma_start(
                    out=megaRE[32 * s:32 * s + 1,
                               DREP_O + 1 + 24 * SP:DREP_O + 1 + 25 * SP],
                    in_=slots[64 + s:65 + s, 255 * SP:256 * SP])
    drain(len(chunks))

    # ---------------- choice bits (min-trick, 8 ops per range) -------------
    def choice(b0, nb):
        diag = rcells84(DREP_O + 2, b0, nb)
        up = rcells84(DREP_O + 4, b0, nb)
        left = rcells84(DRE_O, b0, nb)
        sa, sb, sc = (rcells(o, b0, nb) for o in (SA_O, SB_O, SC_O))
        yield lambda: v.tensor_tensor(out=sb, in0=diag, in1=up, op=AL.min)
        yield lambda: v.tensor_tensor(out=sa, in0=left, in1=sb, op=AL.is_lt)
        yield lambda: v.tensor_tensor(out=sb, in0=up, in1=left, op=AL.min)
        yield lambda: v.tensor_tensor(out=sb, in0=diag, in1=sb, op=AL.is_le)
        yield lambda: v.tensor_single_scalar(out=sc, in_=sa, scalar=0.0,
                                             op=AL.is_equal)    # notleft
        yield lambda: v.scalar_tensor_tensor(out=sb, in0=sb, scalar=-1.0,
                                             in1=rocol(nb, 1), op0=AL.mult,
                                             op1=AL.add)        # oc+2-isdiag
        yield lambda: v.tensor_tensor(out=sb, in0=sb, in1=sc, op=AL.mult)
        yield lambda: v.tensor_tensor(out=sc, in0=rocol(nb), in1=sc,
                                      op=AL.mult)               # Lval

    def gscan(off, b0, nb):
        return lambda: v.tensor_tensor_scan(
            out=megaRE[:, off + 1 + b0 * PITCH:off + 1 + (b0 + nb) * PITCH],
            data0=megaRE[:, SA_O + 1 + b0 * PITCH:SA_O + 1 + (b0 + nb) * PITCH],
            data1=megaRE[:, off + 1 + b0 * PITCH:off + 1 + (b0 + nb) * PITCH],
            initial=0.0, op0=AL.mult, op1=AL.add)

    def ghidma(b):        # per-block g DMA, block-major dst: disjoint bboxes
        return lambda: nc.sync.dma_start(
            out=_manual_ap(slots[64:68, (b - 16) * 32 * PITCH:
                                 (b - 16) * 32 * PITCH + 1], 0,
                           [[PITCH, 32], [1, PITCH]]),
            in_=_manual_ap(megaRE[:, SB_O + 1 + b * PITCH:
                                  SB_O + 2 + b * PITCH], 0, [[1, PITCH]]))

    for f in choice(29, 4):                              # top blocks: critical
        f()
    gscan(SB_O, 29, 4)()                                 # g, blocks 29..32
    for b in range(32, 28, -1):
        ghidma(b)()

    # -------- chunks for the hi walk: rest of choice/g/L + g DMAs ----------
    for b in range(28, 23, -1):                           # deadline order
        for f in choice(b, 1):
            chunks.append(f)
        chunks.append(gscan(SB_O, b, 1))
        chunks.append(ghidma(b))
    for b in range(16, 24):
        for f in choice(b, 1):
            chunks.append(f)
        chunks.append(gscan(SB_O, b, 1))
        chunks.append(ghidma(b))
        chunks.append(gscan(SC_O, b, 1))                  # L scan

    def gx_dmas():
        for s in range(4):    # lo extra: g row 512
            nc.sync.dma_start(
                out=slots[s:s + 1, GX:GX + NB],
                in_=megaRE[32 * s:32 * s + 1,
                           SB_O + 1 + LOW:SB_O + 1 + LOW + NB])
    chunks.append(gx_dmas)
    for b in range(16):
        for f in choice(b, 1):
            chunks.append(f)
        chunks.append(gscan(SB_O, b, 1))
        chunks.append(gscan(SC_O, b, 1))
    chunks.append(lambda: nc.sync.dma_start(              # g-lo -> walk slots
        out=_manual_ap(slots[0:4, 0:1], 0, [[LOW, 32], [1, LOW]]),
        in_=_manual_ap(megaRE[:, SB_O + 1:SB_O + 2], 0, [[1, LOW]])))
    for b in range(24, NBLK):                             # L scans, top blocks
        chunks.append(gscan(SC_O, b, 1))

    # ---------------- walk hi ----------------
    v.memset(xhh[64:68, 512:513], 21.0)                  # x_1023
    v.memset(xhh[64:68, 513:544], 0.0)                   # junk rows > 1023
    v.memset(xhl[0:4, 0:1], 0.0)                         # x_{-1} junk
    for i in range(1023, 511, -1):
        gc = _gslot_hi(i + 1)
        v.scalar_tensor_tensor(
            out=wks[64:68, 41:41 + NB], in0=wks[64:68, 0:NB],
            scalar=xhh[64:68, i - 511:i - 510],
            in1=slots[64:68, gc:gc + NB],
            op0=AL.is_equal, op1=AL.mult,
            accum_out=xhh[64:68, i - 512:i - 511])
        drain()
    drain(len(chunks))

    nc.sync.dma_start(out=xhl[0:4, 512:513], in_=xhh[64:68, 0:1])   # handoff
    nc.vector.tensor_scalar_add(                         # xcol repack hi
        out=_manual_ap(megaRE[64:68, SCX2_O:SCX2_O + 1], 0, [[17, 32], [1, 17]]),
        in0=_manual_ap(xhh[64:68, 0:1], 0, [[1, 32], [32, 17]]), scalar1=0.0)
    nc.sync.dma_start(
        out=_manual_ap(megaRE[:, XC_O + 16:XC_O + 17], 0, [[1, 17]]),
        in_=_manual_ap(megaRE[64:68, SCX2_O:SCX2_O + 1], 0, [[17, 32], [1, 17]]))

    # -------- chunks for the lo walk: hi-half olo/mask/products ------------
    b32 = SA_O + 1 + 32 * PITCH

    def metrics_blk(b):
        sa1 = rcells(SA_O, b, 1)
        yield lambda: v.tensor_tensor(out=sa1, in0=rocol(1),
                                      in1=rsmb(XC_O, b, 1), op=AL.is_equal)
        yield lambda: v.tensor_tensor(out=sa1, in0=sa1,
                                      in1=rcells(SC_O, b, 1), op=AL.mult)
        yield lambda: v.tensor_reduce(out=megaRE[:, OLO_O + b:OLO_O + b + 1],
                                      in_=rcells(SA_O, b, 1),
                                      axis=mybir.AxisListType.X, op=AL.add)
        if b == 32:
            yield lambda: v.memset(megaRE[:, b32:b32 + NB], 0.0)
            for s in range(4):                           # row 1023 mask
                q = 32 * s

                def rebuild(q=q):
                    ic0 = megaRE[q:q + 1, CONST_O + 1:CONST_O + 1 + NB]
                    wt = wks[q:q + 1, 41:41 + NB]
                    v.scalar_tensor_tensor(
                        out=wt, in0=ic0,
                        scalar=megaRE[q:q + 1, OLO_O + 32:OLO_O + 33],
                        in1=ic0, op0=AL.is_ge, op1=AL.bypass)
                    v.scalar_tensor_tensor(
                        out=megaRE[q:q + 1, b32:b32 + NB], in0=ic0,
                        scalar=megaRE[q:q + 1, XC_O + 32:XC_O + 33],
                        in1=wt, op0=AL.is_le, op1=AL.mult)
                yield rebuild
        else:
            yield lambda: v.tensor_tensor(out=sa1, in0=rocol(1),
                                          in1=rsmb(OLO_O, b, 1), op=AL.is_ge)
            yield lambda: v.tensor_tensor(out=rcells(SB_O, b, 1),
                                          in0=rocol(1), in1=rsmb(XC_O, b, 1),
                                          op=AL.is_le)
            yield lambda: v.tensor_tensor(out=sa1, in0=sa1,
                                          in1=rcells(SB_O, b, 1), op=AL.mult)
        for off in (R1_O, R2_O, U_O):                    # products, in place
            yield lambda off=off: v.tensor_tensor(
                out=rcells(off, b, 1), in0=rcells(off, b, 1), in1=sa1,
                op=AL.mult)
        for j, off in enumerate((R1_O, R2_O, U_O, SA_O)):  # partial reduces
            yield lambda j=j, off=off: v.tensor_reduce(
                out=megaRE[:, RSC_O + 33 * j + b:RSC_O + 33 * j + b + 1],
                in_=rcells(off, b, 1), axis=mybir.AxisListType.X, op=AL.add)

    for b in range(16, NBLK):
        for f in metrics_blk(b):
            chunks.append(f)

    # ---------------- walk lo ----------------
    chunks2 = []                # mid-walk group: blocks 8..15 metrics
    for b in range(8, 16):
        for f in metrics_blk(b):
            chunks2.append(f)
    for i in range(511, 0, -1):
        gc = GX if i == 511 else _gslot_lo(i + 1)
        v.scalar_tensor_tensor(
            out=wks[0:4, 41:41 + NB], in0=wks[0:4, 0:NB],
            scalar=xhl[0:4, i + 1:i + 2],
            in1=slots[0:4, gc:gc + NB],
            op0=AL.is_equal, op1=AL.mult,
            accum_out=xhl[0:4, i:i + 1])
        if i == 255:            # xcol for rows 256..511 is final
            nc.vector.tensor_scalar_add(
                out=_manual_ap(megaRE[0:4, SCX_O:SCX_O + 1], 0,
                               [[8, 32], [1, 8]]),
                in0=_manual_ap(xhl[0:4, 256:257], 0, [[1, 32], [32, 8]]),
                scalar1=0.0)
            nc.sync.dma_start(
                out=_manual_ap(megaRE[:, XC_O + 8:XC_O + 9], 0, [[1, 8]]),
                in_=_manual_ap(megaRE[0:4, SCX_O:SCX_O + 1], 0,
                               [[8, 32], [1, 8]]))
        if 225 < i < 490:
            drain()
        elif i <= 225 and chunks2:
            chunks2.pop(0)()
    drain(len(chunks))
    for f in chunks2:
        f()

    # ---------------- tail: blocks 0..7 olo/mask/products + reduces --------
    nc.vector.tensor_scalar_add(                         # xcol repack, b 0..7
        out=_manual_ap(megaRE[0:4, SCX_O:SCX_O + 1], 0, [[8, 32], [1, 8]]),
        in0=_manual_ap(xhl[0:4, 0:1], 0, [[1, 32], [32, 8]]), scalar1=0.0)
    nc.sync.dma_start(
        out=_manual_ap(megaRE[:, XC_O:XC_O + 1], 0, [[1, 8]]),
        in_=_manual_ap(megaRE[0:4, SCX_O:SCX_O + 1], 0, [[8, 32], [1, 8]]))
    salo = rcells(SA_O, 0, 8)
    v.tensor_tensor(out=salo, in0=rocol(8), in1=rsmb(XC_O, 0, 8),
                    op=AL.is_equal)
    v.tensor_tensor(out=salo, in0=salo, in1=rcells(SC_O, 0, 8), op=AL.mult)
    v.tensor_reduce(out=megaRE[:, OLO_O:OLO_O + 8], in_=salo,
                    axis=mybir.AxisListType.X, op=AL.add)
    v.tensor_tensor(out=salo, in0=rocol(8), in1=rsmb(OLO_O, 0, 8),
                    op=AL.is_ge)
    v.tensor_tensor(out=rcells(SB_O, 0, 8), in0=rocol(8),
                    in1=rsmb(XC_O, 0, 8), op=AL.is_le)
    v.tensor_tensor(out=salo, in0=salo, in1=rcells(SB_O, 0, 8), op=AL.mult)
    for s in range(4):                                   # virtual row 0
        v.memset(megaRE[32 * s:32 * s + 1, SA_O + 1:SA_O + 1 + NB], 0.0)
    for off in (R1_O, R2_O, U_O):
        v.tensor_tensor(out=rcells(off, 0, 8), in0=rcells(off, 0, 8),
                        in1=salo, op=AL.mult)
    for j, off in enumerate((R1_O, R2_O, U_O, SA_O)):
        v.tensor_reduce(out=megaRE[:, RSC_O + 33 * j:RSC_O + 33 * j + 8],
                        in_=rcells(off, 0, 8),
                        axis=mybir.AxisListType.X, op=AL.add)
    for j in range(4):
        v.tensor_reduce(out=megaRE[:, RED_O + j:RED_O + j + 1],
                        in_=megaRE[:, RSC_O + 33 * j:RSC_O + 33 * j + NBLK],
                        axis=mybir.AxisListType.X, op=AL.add)

    nc.sync.dma_start(out=partials[:], in_=megaRE[:, RED_O:RED_O + 4])


def _get_module():
    if "nc" not in _CACHE:
        _CACHE["nc"] = _build_module()
    return _CACHE["nc"]


def _make_inmaps(preds, targs):
    preds = np.ascontiguousarray(preds, dtype=np.float32)
    targs = np.ascontiguousarray(targs, dtype=np.float32)
    cst = np.tile(np.arange(44, dtype=np.float32), (128, 1))
    pp = np.arange(32)
    bb = np.arange(NBLK)
    r_idx = pp[:, None] + 32 * bb[None, :]              # [32, 33] = i + 1
    r_ok = (r_idx >= 1) & (r_idx <= N)
    r_cl = np.clip(r_idx - 1, 0, N - 1)
    uu = np.arange(SKW)
    t_idx = uu[None, :] + pp[:, None] - 21              # [32, SKW]
    t_ok = (t_idx >= 0) & (t_idx < N)
    t_cl = np.clip(t_idx, 0, N - 1)
    in_maps = []
    for c in range(NCORES):
        ps = preds[c * BC:(c + 1) * BC]
        ts = targs[c * BC:(c + 1) * BC]
        prev = np.zeros((BC, 32, 3 * NBLK), dtype=np.float32)
        tskv = np.zeros((BC, 32, 3 * SKW), dtype=np.float32)
        for k in range(3):
            vv = ps[:, :, k][:, r_cl]                   # [BC, 32, NBLK]
            prev[:, :, k * NBLK:(k + 1) * NBLK] = np.where(r_ok[None], vv, 0.0)
            fill = FILL if k < 2 else 0.0
            ww = ts[:, :, k][:, t_cl]                   # [BC, 32, SKW]
            tskv[:, :, k * SKW:(k + 1) * SKW] = np.where(t_ok[None], ww, fill)
        in_maps.append({"pre": prev.reshape(128, 3 * NBLK),
                        "tsk": tskv.reshape(128, 3 * SKW), "cst": cst})
    return in_maps


def _reduce_host(parts_list, subcoef):
    c0, c1 = float(subcoef[0]), float(subcoef[1])
    loss = 0.0
    for parts in parts_list:
        m = parts.reshape(BC, 32, 4).sum(axis=1)        # [BC, (Sx,Sy,Sb,cnt)]
        for s in range(BC):
            sx, sy, sb, cnt = (float(m[s, k]) for k in range(4))
            loss += c0 * sx + c1 * sy + 0.1 * sb / cnt
    return np.float32(loss)


def run(preds, targs, subcoef, trace=False):
    nc = _get_module()
    in_maps = _make_inmaps(preds, targs)
    res = run_bass_kernel_spmd(nc, in_maps, core_ids=list(range(NCORES)),
                               trace=trace)
    parts = [r["partials"] for r in res.results]
    return _reduce_host(parts, np.asarray(subcoef)), res


def kernel(preds, targs, subcoef):
    out, _ = run(preds, targs, subcoef)
    return out
